# revision 1
# baseline (speedup 1.0000x reference)
"""Trainium2 Bass kernel for nn_Block_17033840296551 (GNN message passing block).

Data-parallel over batch: 16 images -> 8 cores x 2 images. Each core runs the
full block (g1 conv -> kNN top-9 -> EdgeConv max-agg -> g2 -> FFN -> bottleneck
-> final BN) on its 2 images with no cross-core communication.

Key algorithmic points:
  * All BNs folded into conv weights/biases on host.
  * EdgeConv decomposed: e[n,k] = p[n] + q[idx[n,k]], p = (Wa-Wb)@feat + b,
    q = Wb@feat; max_k relu(e) = relu(p + max_k q).
  * kNN: scores s[n,m] = <feat_n, feat_m/||feat_m||> are rank-equivalent to
    cosine sim per row; self (rank-1 always) removed via -BIG on the diagonal
    (extra identity matmul into PSUM), leaving a top-8 problem that maps to
    the DVE's native max/max_index (Max8) ops.
  * Neighbor gather: SBUF-source transposed dma_gather of q rows (fp16),
    output channels-on-partitions; 9-way max folded with fp16 TT-max trees.
  * fp16 matmul inputs (PE full speed, 11-bit mantissa), f32 PSUM accum,
    f32 residual stream. Validated on host: l2 rel err ~3e-3.
"""

import os
import numpy as np

_DBG_SKIP = os.environ.get("KBENCH_SKIP", "")

# problem constants (hardcoded per harness contract)
B, C, H, W = 16, 256, 32, 32
N = H * W           # 1024 pixels per image
K = 9
EPS = 1e-5
IMGS_PER_CORE = 2
N_CORES = 8
NEG_BIG = -30000.0

_cache = {}


# --------------------------------------------------------------------------
# host-side preprocessing
# --------------------------------------------------------------------------
def _bn_fold(p):
    g, b, m, v = np.asarray(p, np.float32)
    s = g / np.sqrt(v + EPS)
    t = b - m * s
    return s, t


def _pack_kxm(w_t, part=128):
    """[K, M] -> [part, K//part, M] (partition-major K tiling)."""
    Kd, M = w_t.shape
    kt = Kd // part
    return np.ascontiguousarray(w_t.reshape(kt, part, M).transpose(1, 0, 2))


def _pack_bias(b, part=128):
    n = b.shape[0]
    t = n // part
    return np.ascontiguousarray(b.reshape(t, part).T)  # [part, t]


def _make_skeleton():
    """Wrapped idx buffer skeleton [16, 576] i16 with the k=0 (self) column
    pattern filled in; cols for k>=1 are filled on-device."""
    skel = np.zeros((16, 576), np.int16)
    for s in range(4):
        for ip in range(2):
            for g in range(8):
                col = 144 * s + 0 + 8 * ip + g
                for p16 in range(16):
                    n = 256 * s + 128 * ip + 16 * g + p16
                    skel[p16, col] = n
    return skel


def _prep_weights(inp):
    f16 = np.float16
    s1, t1 = _bn_fold(inp['g1_bn'])
    Wg1 = s1[:, None] * inp['g1_w']
    s2, t2 = _bn_fold(inp['g2_bn'])
    Wg2 = s2[:, None] * inp['g2_w']
    sf1, tf1 = _bn_fold(inp['f1_bn'])
    Wf1 = sf1[:, None] * inp['f1_w']
    bf1 = sf1 * inp['f1_b'] + tf1
    sf2, tf2 = _bn_fold(inp['f2_bn'])
    Wf2 = sf2[:, None] * inp['f2_w']
    bf2 = sf2 * inp['f2_b'] + tf2
    sb1, tb1 = _bn_fold(inp['b1_bn'])
    Wb1 = sb1[:, None] * inp['b1_w']
    sb2, tb2 = _bn_fold(inp['b2_bn'])
    Wb2 = sb2[:, None, None, None] * inp['b2_w']
    sb3, tb3 = _bn_fold(inp['b3_bn'])
    Wb3 = sb3[:, None] * inp['b3_w']
    sf, tf = _bn_fold(inp['bnf'])

    A = inp['edge_w'][:, :C]
    Bm = inp['edge_w'][:, C:]
    Wp = A - Bm
    Wq = Bm
    bp = inp['edge_b']

    wb2_t = np.zeros((64, 9, 64), f16)
    for dy in range(3):
        for dx in range(3):
            wb2_t[:, dy * 3 + dx, :] = Wb2[:, :, dy, dx].T.astype(f16)

    return {
        'wg1': _pack_kxm(Wg1.T.astype(f16)),           # [128,2,256]
        'wp': _pack_kxm(Wp.T.astype(f16)),             # [128,2,512]
        'wq': _pack_kxm(Wq.T.astype(f16)),             # [128,2,512]
        'wg2': _pack_kxm(Wg2.T.astype(f16)),           # [128,4,256]
        'wf1': _pack_kxm(Wf1.T.astype(f16)),           # [128,2,1024]
        'wf2': _pack_kxm(Wf2.T.astype(f16)),           # [128,8,256]
        'wb1': _pack_kxm(Wb1.T.astype(f16)),           # [128,2,64]
        'wb2': wb2_t,                                   # [64,9,64]
        'wb3': Wb3.T.astype(f16),                       # [64,256]
        'bt1': _pack_bias(t1),                          # [128,2] f32
        'bt2': _pack_bias(t2),
        'bbp': _pack_bias(bp),                          # [128,4]
        'bbf1': _pack_bias(bf1),                        # [128,8]
        'bbf2': _pack_bias(bf2),
        'btb1': np.ascontiguousarray(tb1[:, None].astype(np.float32)),  # [64,1]
        'btb2': np.ascontiguousarray(tb2[:, None].astype(np.float32)),
        'btb3': _pack_bias(tb3),
        'bsf': _pack_bias(sf),
        'btf': _pack_bias(tf),
        'skel': _make_skeleton(),
    }


# --------------------------------------------------------------------------
# device kernel builder
# --------------------------------------------------------------------------
def _build_bass():
    import concourse.bass as bass
    import concourse.mybir as mybir
    from concourse import bacc
    from concourse.tile import TileContext
    from concourse.masks import make_identity

    dt = mybir.dt
    F16 = dt.float16
    F32 = dt.float32
    AF = mybir.ActivationFunctionType
    OP = mybir.AluOpType

    nc = bacc.Bacc()

    # ---- DRAM parameters ----
    x_d = nc.declare_dram_parameter("x", [IMGS_PER_CORE, C, N], F32, isOutput=False)
    wg1_d = nc.declare_dram_parameter("wg1", [128, 2, 256], F16, isOutput=False)
    wp_d = nc.declare_dram_parameter("wp", [128, 2, 512], F16, isOutput=False)
    wq_d = nc.declare_dram_parameter("wq", [128, 2, 512], F16, isOutput=False)
    wg2_d = nc.declare_dram_parameter("wg2", [128, 4, 256], F16, isOutput=False)
    wf1_d = nc.declare_dram_parameter("wf1", [128, 2, 1024], F16, isOutput=False)
    wf2_d = nc.declare_dram_parameter("wf2", [128, 8, 256], F16, isOutput=False)
    wb1_d = nc.declare_dram_parameter("wb1", [128, 2, 64], F16, isOutput=False)
    wb2_d = nc.declare_dram_parameter("wb2", [64, 9, 64], F16, isOutput=False)
    wb3_d = nc.declare_dram_parameter("wb3", [64, 256], F16, isOutput=False)
    bt1_d = nc.declare_dram_parameter("bt1", [128, 2], F32, isOutput=False)
    bt2_d = nc.declare_dram_parameter("bt2", [128, 2], F32, isOutput=False)
    bbp_d = nc.declare_dram_parameter("bbp", [128, 4], F32, isOutput=False)
    bbf1_d = nc.declare_dram_parameter("bbf1", [128, 8], F32, isOutput=False)
    bbf2_d = nc.declare_dram_parameter("bbf2", [128, 2], F32, isOutput=False)
    btb1_d = nc.declare_dram_parameter("btb1", [64, 1], F32, isOutput=False)
    btb2_d = nc.declare_dram_parameter("btb2", [64, 1], F32, isOutput=False)
    btb3_d = nc.declare_dram_parameter("btb3", [128, 2], F32, isOutput=False)
    bsf_d = nc.declare_dram_parameter("bsf", [128, 2], F32, isOutput=False)
    btf_d = nc.declare_dram_parameter("btf", [128, 2], F32, isOutput=False)
    skel_d = nc.declare_dram_parameter("skel", [16, 576], dt.int16, isOutput=False)
    q_drams = [nc.dram_tensor(f"q_dram{i}", [N, 512], F16)
               for i in range(IMGS_PER_CORE)]
    out_d = nc.declare_dram_parameter("out", [IMGS_PER_CORE, C, N], F32, isOutput=True)

    with TileContext(nc) as tc:
        import contextlib
        ctx = contextlib.ExitStack()
        with ctx:
            consts = ctx.enter_context(tc.tile_pool(name="consts", bufs=1))
            pool_x = ctx.enter_context(tc.tile_pool(name="x", bufs=2))
            pool_xc = ctx.enter_context(tc.tile_pool(name="xc", bufs=2))
            pool_feat = ctx.enter_context(tc.tile_pool(name="feat", bufs=2))
            pool_fx = ctx.enter_context(tc.tile_pool(name="fx", bufs=1))
            pool_big = ctx.enter_context(tc.tile_pool(name="big", bufs=1))
            pool_gath = ctx.enter_context(tc.tile_pool(name="gath", bufs=2))
            pool_mid = ctx.enter_context(tc.tile_pool(name="mid", bufs=1))
            pool_sm1 = ctx.enter_context(tc.tile_pool(name="sm1", bufs=1))
            pool_sm2 = ctx.enter_context(tc.tile_pool(name="sm2", bufs=3))
            pool_idx = ctx.enter_context(tc.tile_pool(name="idx", bufs=2))
            psum = ctx.enter_context(tc.tile_pool(name="psum", bufs=4, space="PSUM"))
            psum1 = ctx.enter_context(tc.tile_pool(name="psum1", bufs=1, space="PSUM"))
            psum64 = ctx.enter_context(tc.tile_pool(name="psum64", bufs=2, space="PSUM"))

            # ---- constants / weights (loaded once) ----
            def load(name, shape, dtype, src):
                t = consts.tile(shape, dtype, name=name)
                nc.sync.dma_start(out=t[:], in_=src[:])
                return t

            wg1 = load("wg1s", [128, 2, 256], F16, wg1_d)
            wp = load("wps", [128, 2, 512], F16, wp_d)
            wq = load("wqs", [128, 2, 512], F16, wq_d)
            wg2 = load("wg2s", [128, 4, 256], F16, wg2_d)
            wf1 = load("wf1s", [128, 2, 1024], F16, wf1_d)
            wf2 = load("wf2s", [128, 8, 256], F16, wf2_d)
            wb1 = load("wb1s", [128, 2, 64], F16, wb1_d)
            wb2 = load("wb2s", [64, 9, 64], F16, wb2_d)
            wb3 = load("wb3s", [64, 256], F16, wb3_d)
            bt1 = load("bt1s", [128, 2], F32, bt1_d)
            bt2 = load("bt2s", [128, 2], F32, bt2_d)
            bbp = load("bbps", [128, 4], F32, bbp_d)
            bbf1 = load("bbf1s", [128, 8], F32, bbf1_d)
            bbf2 = load("bbf2s", [128, 2], F32, bbf2_d)
            btb1 = load("btb1s", [64, 1], F32, btb1_d)
            btb2 = load("btb2s", [64, 1], F32, btb2_d)
            btb3 = load("btb3s", [128, 2], F32, btb3_d)
            bsf = load("bsfs", [128, 2], F32, bsf_d)
            btf = load("btfs", [128, 2], F32, btf_d)

            ident = consts.tile([128, 128], F16, name="ident")
            make_identity(nc, ident[:])
            negid = consts.tile([128, 128], F16, name="negid")
            nc.scalar.activation(out=negid[:], in_=ident[:], func=AF.Copy,
                                 scale=NEG_BIG)
            ones = consts.tile([128, 128], F16, name="ones")
            nc.gpsimd.memset(ones[:], 1.0)
            # idbig[k, f] = 1 iff f == k + 384 (shifted identity for diag-kill)
            idbig = consts.tile([128, 1024], F16, name="idbig")
            nc.gpsimd.memset(idbig[:], 0.0)
            nc.gpsimd.affine_select(
                out=idbig[:], in_=idbig[:],
                compare_op=mybir.AluOpType.not_equal, fill=1.0,
                base=384, pattern=[[-1, 1024]], channel_multiplier=1)

            for img in range(IMGS_PER_CORE):
                # ==== load x ====
                x32 = pool_x.tile([128, 2, N], F32, name="x32")
                xc = pool_xc.tile([128, 2, N], F16, name="xc")
                for t in range(2):
                    nc.sync.dma_start(out=x32[:, t, :],
                                      in_=x_d[img, t * 128:(t + 1) * 128, :])
                    # cast f32->f16 during DMA (SWDGE)
                    nc.gpsimd.dma_start(out=xc[:, t, :],
                                        in_=x_d[img, t * 128:(t + 1) * 128, :])

                # ==== g1: featT [128, 2, N] f16 ====
                featT = pool_feat.tile([128, 2, N], F16, name="featT")
                for to in range(2):
                    for nb in range(2):
                        ps = psum.tile([128, 512], F32, name="ps_g1", tag="ps")
                        for kt in range(2):
                            nc.tensor.matmul(
                                ps[:], lhsT=wg1[:, kt, to * 128:(to + 1) * 128],
                                rhs=xc[:, kt, nb * 512:(nb + 1) * 512],
                                start=(kt == 0), stop=(kt == 1))
                        nc.scalar.activation(
                            out=featT[:, to, nb * 512:(nb + 1) * 512], in_=ps[:],
                            func=AF.Identity, bias=bt1[:, to:to + 1])

                # ==== norms: invnb [128, N] f16 (broadcast of 1/||f_px||) ====
                fsq = pool_fx.tile([128, 2, N], F16, name="fsq", tag="fx")
                for t in range(2):
                    nc.vector.tensor_mul(fsq[:, t, :], featT[:, t, :], featT[:, t, :])
                n2 = pool_sm1.tile([1, N], F32, name="n2")
                for nb in range(2):
                    ps1 = psum1.tile([1, 512], F32, name="ps_n2")
                    for kt in range(2):
                        nc.tensor.matmul(
                            ps1[:], lhsT=ones[:, 0:1],
                            rhs=fsq[:, kt, nb * 512:(nb + 1) * 512],
                            start=(kt == 0), stop=(kt == 1))
                    nc.scalar.activation(out=n2[:, nb * 512:(nb + 1) * 512],
                                         in_=ps1[:], func=AF.Copy)
                rn2 = pool_sm1.tile([1, N], F32, name="rn2")
                nc.vector.reciprocal(out=rn2[:], in_=n2[:])
                invn = pool_sm1.tile([1, N], F16, name="invn")
                nc.scalar.activation(out=invn[:], in_=rn2[:], func=AF.Sqrt)
                invnb = pool_fx.tile([128, N], F16, name="invnb")
                for nb in range(2):
                    psb = psum.tile([128, 512], F32, name="ps_bc", tag="ps")
                    nc.tensor.matmul(psb[:], lhsT=ones[0:1, :],
                                     rhs=invn[:, nb * 512:(nb + 1) * 512],
                                     start=True, stop=True)
                    nc.scalar.activation(out=invnb[:, nb * 512:(nb + 1) * 512],
                                         in_=psb[:], func=AF.Copy)
                xnT = pool_fx.tile([128, 2, N], F16, name="xnT", tag="fx")
                for t in range(2):
                    nc.vector.tensor_mul(xnT[:, t, :], featT[:, t, :], invnb[:])

                # ==== sim scores [128, 8, N] f32, diagonal killed ====
                sim = pool_big.tile([128, 8, N], F16, name="sim", tag="simbig")
                for I in range(8):
                    for cb in range(2):
                        has_diag = (cb == I // 4)
                        ps = psum.tile([128, 512], F32, name="ps_sim", tag="ps")
                        for kt in range(2):
                            nc.tensor.matmul(
                                ps[:], lhsT=featT[:, kt, I * 128:(I + 1) * 128],
                                rhs=xnT[:, kt, cb * 512:(cb + 1) * 512],
                                start=(kt == 0),
                                stop=(kt == 1 and not has_diag))
                        if has_diag:
                            off = I * 128 - cb * 512
                            nc.tensor.matmul(ps[:], lhsT=negid[:],
                                             rhs=idbig[:, 384 - off:896 - off],
                                             start=False, stop=True)
                        nc.scalar.activation(
                            out=sim[:, I, cb * 512:(cb + 1) * 512], in_=ps[:],
                            func=AF.Copy)

                # ==== selection: top-8 neighbor indices per row ====
                # ixbuf layout: [128, s(4), k(8), i(2)] so the wrap-DMA
                # source flattens to one contiguous run per partition.
                ixbuf = pool_idx.tile([128, 4, 8, 2], dt.uint16, name="ixbuf")
                if "sel" in _DBG_SKIP:
                    nc.vector.memset(ixbuf[:], 3)
                else:
                    for I in range(8):
                        mx = pool_sm2.tile([128, 8], F16, name="mx")
                        nc.vector.max(out=mx[:], in_=sim[:, I, :])
                        nc.vector.max_index(out=ixbuf[:, I // 2, :, I % 2],
                                            in_max=mx[:],
                                            in_values=sim[:, I, :])

                # ==== wrapped idx buffer [128, 576] i16 ====
                wrapped = pool_idx.tile([128, 576], dt.int16, name="wrapped")
                nc.sync.dma_start(out=wrapped[0:16, :], in_=skel_d[:])
                wview = wrapped[0:16, :].rearrange(
                    "p (s k i g) -> p s k i g", s=4, k=9, i=2, g=8)
                ixi = ixbuf[:].bitcast(dt.int16)
                if "wrap" not in _DBG_SKIP:
                    for g in range(8):
                        for s in range(4):
                            src = ixi[16 * g:16 * (g + 1), s, :, :]
                            nc.sync.dma_start(out=wview[:, s, 1:9, :, g], in_=src)
                    for g in range(1, 8):
                        nc.sync.dma_start(out=wrapped[16 * g:16 * (g + 1), :],
                                          in_=wrapped[0:16, :])

                # ==== q [128 n-part, 8, 512] f16 ====
                q_sb = pool_mid.tile([128, 8, 512], F16, name="q_sb", tag="qe")
                q_dram = q_drams[img]
                for nt in range(8):
                    ps = psum.tile([128, 512], F32, name="ps_q", tag="ps")
                    for kt in range(2):
                        nc.tensor.matmul(
                            ps[:], lhsT=featT[:, kt, nt * 128:(nt + 1) * 128],
                            rhs=wq[:, kt, :], start=(kt == 0), stop=(kt == 1))
                    nc.scalar.activation(out=q_sb[:, nt, :], in_=ps[:], func=AF.Copy)
                    nc.sync.dma_start(out=q_dram[nt * 128:(nt + 1) * 128, :],
                                      in_=q_sb[:, nt, :])

                # ==== p^T [128, 4, N] f16 (ch-part, bias folded) ====
                pT = pool_mid.tile([128, 4, N], F16, name="pT", tag="po")
                for to in range(4):
                    for nb in range(2):
                        ps = psum.tile([128, 512], F32, name="ps_p", tag="ps")
                        for kt in range(2):
                            nc.tensor.matmul(
                                ps[:], lhsT=wp[:, kt, to * 128:(to + 1) * 128],
                                rhs=featT[:, kt, nb * 512:(nb + 1) * 512],
                                start=(kt == 0), stop=(kt == 1))
                        nc.scalar.activation(
                            out=pT[:, to, nb * 512:(nb + 1) * 512], in_=ps[:],
                            func=AF.Identity, bias=bbp[:, to:to + 1])

                # ==== gather + 9-way max fold -> maxqT [128, 4, N] f16 ====
                maxqT = pool_mid.tile([128, 4, N], F16, name="maxqT", tag="mf")
                qflat = q_sb[:].rearrange("p a b -> p (a b)")
                for s in range(4):
                    go = pool_gath.tile([128, 4, 2304], F16, name="go")
                    if "gather" in _DBG_SKIP:
                        nc.vector.memset(go[:], 0)
                        gv = go[:].rearrange("p a (k n) -> p a k n", k=9)
                        nc.vector.tensor_max(maxqT[:, :, 256 * s:256 * (s + 1)],
                                             gv[:, :, 8, :], gv[:, :, 0, :])
                        continue
                    if os.environ.get("KBENCH_SBUFGATHER"):
                        nc.gpsimd.dma_gather(
                            out_ap=go[:], in_ap=qflat,
                            idxs_ap=wrapped[:, 144 * s:144 * (s + 1)],
                            num_idxs=2304, num_idxs_reg=2304, elem_size=512,
                            transpose=True, sbuf_tokens_per_rank=128,
                            sbuf_free_dim_per_rank=1024,
                            single_packet=False)
                    else:
                        nc.gpsimd.dma_gather(
                            out_ap=go[:], in_ap=q_dram[:],
                            idxs_ap=wrapped[:, 144 * s:144 * (s + 1)],
                            num_idxs=2304, num_idxs_reg=2304, elem_size=512,
                            transpose=True, single_packet=False)
                    gv = go[:].rearrange("p a (k n) -> p a k n", k=9)
                    nc.vector.tensor_max(gv[:, :, 5:9, :], gv[:, :, 1:5, :],
                                         gv[:, :, 5:9, :])
                    nc.vector.tensor_max(gv[:, :, 7:9, :], gv[:, :, 5:7, :],
                                         gv[:, :, 7:9, :])
                    nc.vector.tensor_max(gv[:, :, 8, :], gv[:, :, 7, :],
                                         gv[:, :, 8, :])
                    nc.vector.tensor_max(maxqT[:, :, 256 * s:256 * (s + 1)],
                                         gv[:, :, 8, :], gv[:, :, 0, :])

                # ==== e = relu(p + maxq) [128, 4, N] f16 ====
                eT = pool_mid.tile([128, 4, N], F16, name="eT", tag="qe")
                nc.vector.tensor_add(eT[:], pT[:], maxqT[:])
                nc.vector.tensor_scalar_max(eT[:], eT[:], 0.0)

                # ==== g2 + residual -> h32 f32, hc f16 ====
                h32 = pool_mid.tile([128, 2, N], F32, name="h32", tag="ho")
                hc = pool_mid.tile([128, 2, N], F16, name="hc")
                for to in range(2):
                    for nb in range(2):
                        ps = psum.tile([128, 512], F32, name="ps_g2", tag="ps")
                        for kt in range(4):
                            nc.tensor.matmul(
                                ps[:], lhsT=wg2[:, kt, to * 128:(to + 1) * 128],
                                rhs=eT[:, kt, nb * 512:(nb + 1) * 512],
                                start=(kt == 0), stop=(kt == 3))
                        tmp = pool_sm2.tile([128, 512], F32, name="g2tmp", tag="evtmp")
                        nc.scalar.activation(out=tmp[:], in_=ps[:],
                                             func=AF.Identity,
                                             bias=bt2[:, to:to + 1])
                        sl = slice(nb * 512, (nb + 1) * 512)
                        nc.vector.tensor_add(h32[:, to, sl], tmp[:], x32[:, to, sl])
                for t in range(2):
                    nc.vector.tensor_copy(hc[:, t, :], h32[:, t, :])

                # ==== FFN ====
                f1o = pool_big.tile([128, 8, N], F16, name="f1o", tag="simbig")
                for to in range(8):
                    for nb in range(2):
                        ps = psum.tile([128, 512], F32, name="ps_f1", tag="ps")
                        for kt in range(2):
                            nc.tensor.matmul(
                                ps[:], lhsT=wf1[:, kt, to * 128:(to + 1) * 128],
                                rhs=hc[:, kt, nb * 512:(nb + 1) * 512],
                                start=(kt == 0), stop=(kt == 1))
                        nc.scalar.activation(
                            out=f1o[:, to, nb * 512:(nb + 1) * 512], in_=ps[:],
                            func=AF.Relu, bias=bbf1[:, to:to + 1])
                h232 = pool_mid.tile([128, 2, N], F32, name="h232")
                h2c = pool_mid.tile([128, 2, N], F16, name="h2c")
                for to in range(2):
                    for nb in range(2):
                        ps = psum.tile([128, 512], F32, name="ps_f2", tag="ps")
                        for kt in range(8):
                            nc.tensor.matmul(
                                ps[:], lhsT=wf2[:, kt, to * 128:(to + 1) * 128],
                                rhs=f1o[:, kt, nb * 512:(nb + 1) * 512],
                                start=(kt == 0), stop=(kt == 7))
                        tmp = pool_sm2.tile([128, 512], F32, name="f2tmp", tag="evtmp")
                        nc.scalar.activation(out=tmp[:], in_=ps[:],
                                             func=AF.Identity,
                                             bias=bbf2[:, to:to + 1])
                        sl = slice(nb * 512, (nb + 1) * 512)
                        nc.vector.tensor_add(h232[:, to, sl], tmp[:], h32[:, to, sl])
                for t in range(2):
                    nc.vector.tensor_copy(h2c[:, t, :], h232[:, t, :])

                # ==== bottleneck ====
                b1o = pool_mid.tile([64, N], F16, name="b1o")
                for nb in range(2):
                    ps = psum64.tile([64, 512], F32, name="ps_b1", tag="ps64")
                    for kt in range(2):
                        nc.tensor.matmul(
                            ps[:], lhsT=wb1[:, kt, :],
                            rhs=h2c[:, kt, nb * 512:(nb + 1) * 512],
                            start=(kt == 0), stop=(kt == 1))
                    nc.scalar.activation(out=b1o[:, nb * 512:(nb + 1) * 512],
                                         in_=ps[:], func=AF.Relu, bias=btb1[:, 0:1])
                pad = pool_mid.tile([64, 34 * 34], F16, name="pad")
                nc.gpsimd.memset(pad[:], 0.0)
                pad3 = pad[:].rearrange("p (r c) -> p r c", r=34)
                b1v = b1o[:].rearrange("p (r c) -> p r c", r=32)
                nc.vector.tensor_copy(pad3[:, 1:33, 1:33], b1v)
                b2o = pool_mid.tile([64, N], F16, name="b2o")
                for nb in range(2):
                    ps = psum64.tile([64, 512], F32, name="ps_b2", tag="ps64")
                    for tap in range(9):
                        dy, dx = tap // 3, tap % 3
                        rhs = pad3[:, 16 * nb + dy:16 * nb + dy + 16, dx:dx + 32]
                        nc.tensor.matmul(ps[:], lhsT=wb2[:, tap, :], rhs=rhs,
                                         start=(tap == 0), stop=(tap == 8))
                    nc.scalar.activation(out=b2o[:, nb * 512:(nb + 1) * 512],
                                         in_=ps[:], func=AF.Relu, bias=btb2[:, 0:1])
                o332 = pool_mid.tile([128, 2, N], F32, name="o332", tag="po")
                for to in range(2):
                    for nb in range(2):
                        ps = psum.tile([128, 512], F32, name="ps_b3", tag="ps")
                        nc.tensor.matmul(
                            ps[:], lhsT=wb3[:, to * 128:(to + 1) * 128],
                            rhs=b2o[:, nb * 512:(nb + 1) * 512],
                            start=True, stop=True)
                        tmp = pool_sm2.tile([128, 512], F32, name="b3tmp", tag="evtmp")
                        nc.scalar.activation(out=tmp[:], in_=ps[:],
                                             func=AF.Identity,
                                             bias=btb3[:, to:to + 1])
                        sl = slice(nb * 512, (nb + 1) * 512)
                        nc.vector.tensor_add(o332[:, to, sl], tmp[:],
                                             h232[:, to, sl])

                # ==== final: out = sf*(o3 + x) + tf ====
                fin = pool_mid.tile([128, 2, N], F32, name="fin", tag="mf")
                out32 = pool_mid.tile([128, 2, N], F32, name="out32", tag="ho")
                for t in range(2):
                    nc.vector.tensor_add(fin[:, t, :], o332[:, t, :], x32[:, t, :])
                    nc.scalar.activation(out=out32[:, t, :], in_=fin[:, t, :],
                                         func=AF.Identity, scale=bsf[:, t:t + 1],
                                         bias=btf[:, t:t + 1])
                    nc.sync.dma_start(out=out_d[img, t * 128:(t + 1) * 128, :],
                                      in_=out32[:, t, :])

    nc.finalize()
    return nc


# --------------------------------------------------------------------------
# entry point
# --------------------------------------------------------------------------
def kernel(**inputs):
    inp = {k: np.asarray(v) for k, v in inputs.items()}
    w = _prep_weights(inp)

    if 'nc' not in _cache:
        _cache['nc'] = _build_bass()
    nc = _cache['nc']

    x = inp['x'].astype(np.float32).reshape(B, C, N)
    in_maps = []
    for c in range(N_CORES):
        m = {'x': np.ascontiguousarray(x[c * 2:(c + 1) * 2])}
        m.update({k: v for k, v in w.items() if k != 'skel'})
        m['skel'] = w['skel']
        in_maps.append(m)

    from concourse.bass_utils import run_bass_kernel_spmd
    trace = bool(os.environ.get("KBENCH_TRACE"))
    res = run_bass_kernel_spmd(nc, in_maps, core_ids=list(range(N_CORES)),
                               trace=trace)
    _cache['exec_time_ns'] = res.exec_time_ns
    _cache['results'] = res
    out = np.zeros((B, C, N), np.float32)
    for c in range(N_CORES):
        out[c * 2:(c + 1) * 2] = res.results[c]['out']
    return out.reshape(B, C, H, W)



# revision 31
# speedup vs baseline: 1.8258x; 1.8258x over previous
"""Trainium2 Bass kernel for nn_Block_17033840296551 (GNN message passing block).

Data-parallel over batch: 16 images -> 8 cores x 2 images. Each core runs the
full block (g1 conv -> kNN top-9 -> EdgeConv max-agg -> g2 -> FFN -> bottleneck
-> final BN) on its 2 images with no cross-core communication.

v2 design (LSE EdgeConv — no neighbor gather):
  * All BNs folded into conv weights/biases on host.
  * EdgeConv decomposed: e[n,k] = p[n] + q[idx[n,k]], p = (Wa-Wb)@feat + b,
    q = Wb@feat; max_k relu(e) = relu(p + max_k q).
  * max_k q replaced by log-sum-exp: max_k q ~= c + ln(sum_k exp(t(q_k-c)))/t
    with t=30, c=2.0. The sum over the 9-hot neighbor set is a matmul
    S^T.T @ expq on the PE (S built by gpsimd local_scatter from the top-8
    indices; S^T via tiled xbar dma transpose). Kills the descriptor-
    generation-bound dma_gather (~160us/core) plus the DVE max-fold tree.
  * kNN: scores s[n,m] = <feat_n, feat_m/||feat_m||> rank-equivalent to cosine
    per row; self removed via -BIG diagonal (extra identity matmul into PSUM);
    DVE Max8/FindIndex8 read scores straight from PSUM (no SBUF sim buffer).
  * f16 matmul inputs (bf16 for the exp path: f16 overflows at e^11), f32
    PSUM, f16 residual stream, [128,1024] two-bank PSUM drains.
  * Two-phase emission (head: g1..sim..sel..q..p, tail: agg..FFN..bottleneck)
    interleaved across the 2 images so PE/DVE/ACT/DMA overlap.
"""

import os
import numpy as np

# problem constants (hardcoded per harness contract)
B, C, H, W = 16, 256, 32, 32
N = H * W           # 1024 pixels per image
K = 9
EPS = 1e-5
IMGS_PER_CORE = 2
N_CORES = 8
NEG_BIG = -30000.0
T_LSE = 30.0
C_LSE = 2.0
LN2 = 0.6931471805599453
# ln(x) ~= LN2 * (bitcast_int32(x) * 2^-23 - 126.957)  (max err ~0.03 in ln)
LN_ALPHA = LN2 / T_LSE / (1 << 23)
LN_BETA = -126.957 * LN2 / T_LSE

_cache = {}


# --------------------------------------------------------------------------
# host-side preprocessing
# --------------------------------------------------------------------------
def _bn_fold(p):
    g, b, m, v = np.asarray(p, np.float32)
    s = g / np.sqrt(v + EPS)
    t = b - m * s
    return s, t


def _pack_kxm(w_t, part=128):
    """[K, M] -> [part, K//part, M] (partition-major K tiling)."""
    Kd, M = w_t.shape
    kt = Kd // part
    return np.ascontiguousarray(w_t.reshape(kt, part, M).transpose(1, 0, 2))


def _pack_bias(b, part=128):
    n = b.shape[0]
    t = n // part
    return np.ascontiguousarray(b.reshape(t, part).T)  # [part, t]


def _make_selftpl():
    """ixbuf init template [128, 8, 10] uint16: col0 = self node id, col9 =
    0xFFFF (-1 as int16: ignored by local_scatter), cols 1..8 overwritten by
    find_index8."""
    tpl = np.zeros((128, 8, 10), np.uint16)
    for I in range(8):
        tpl[:, I, 0] = I * 128 + np.arange(128)
        tpl[:, I, 9] = 0xFFFF
    return np.ascontiguousarray(tpl.reshape(128, 80))


def _prep_weights(inp):
    f16 = np.float16
    s1, t1 = _bn_fold(inp['g1_bn'])
    Wg1 = s1[:, None] * inp['g1_w']
    s2, t2 = _bn_fold(inp['g2_bn'])
    Wg2 = s2[:, None] * inp['g2_w']
    sf1, tf1 = _bn_fold(inp['f1_bn'])
    Wf1 = sf1[:, None] * inp['f1_w']
    bf1 = sf1 * inp['f1_b'] + tf1
    sf2, tf2 = _bn_fold(inp['f2_bn'])
    Wf2 = sf2[:, None] * inp['f2_w']
    bf2 = sf2 * inp['f2_b'] + tf2
    sb1, tb1 = _bn_fold(inp['b1_bn'])
    Wb1 = sb1[:, None] * inp['b1_w']
    sb2, tb2 = _bn_fold(inp['b2_bn'])
    Wb2 = sb2[:, None, None, None] * inp['b2_w']
    sb3, tb3 = _bn_fold(inp['b3_bn'])
    Wb3 = sb3[:, None] * inp['b3_w']
    sf, tf = _bn_fold(inp['bnf'])

    A = inp['edge_w'][:, :C]
    Bm = inp['edge_w'][:, C:]
    Wp = A - Bm
    Wq = Bm
    bp = inp['edge_b'] + C_LSE          # LSE shift folded into the p bias

    wb2_t = np.zeros((64, 9, 64), f16)
    for dy in range(3):
        for dx in range(3):
            wb2_t[:, dy * 3 + dx, :] = Wb2[:, :, dy, dx].T.astype(f16)

    return {
        'wg1': _pack_kxm(Wg1.T.astype(f16)),                # [128,2,256]
        'wp': _pack_kxm(Wp.T.astype(f16)),                  # [128,2,512]
        'wq': _pack_kxm((T_LSE * Wq).T.astype(f16)),        # [128,2,512] (t*Wq)
        'wg2': _pack_kxm(Wg2.T.astype(f16)),                # [128,4,256]
        'wf1': _pack_kxm(Wf1.T.astype(f16)),                # [128,2,1024]
        'wf2': _pack_kxm(Wf2.T.astype(f16)),                # [128,8,256]
        'wb1': _pack_kxm(Wb1.T.astype(f16)),                # [128,2,64]
        'wb2': wb2_t,                                        # [64,9,64]
        'wb3': Wb3.T.astype(f16),                            # [64,256]
        'bt1': _pack_bias(t1),                               # [128,2] f32
        'bt2': _pack_bias(t2),
        'bbp': _pack_bias(bp),                               # [128,4]
        'bbf1': _pack_bias(bf1),                             # [128,8]
        'bbf2': _pack_bias(bf2),
        'btb1': np.ascontiguousarray(tb1[:, None].astype(np.float32)),  # [64,1]
        'btb2': np.ascontiguousarray(tb2[:, None].astype(np.float32)),
        'btb3': _pack_bias(tb3),
        'bsf': _pack_bias(sf),
        'btf': _pack_bias(tf),
        'expb': np.full((128, 1), -T_LSE * C_LSE, np.float32),
        'lnb': np.full((128, 1), 1e-30, np.float32),
        'selftpl': _make_selftpl(),                          # [128,80] u16
    }


# --------------------------------------------------------------------------
# device kernel builder
# --------------------------------------------------------------------------
def _build_bass():
    import concourse.bass as bass
    import concourse.mybir as mybir
    from concourse import bacc
    from concourse.tile import TileContext
    from concourse.masks import make_identity

    dt = mybir.dt
    F16 = dt.float16
    BF16 = dt.bfloat16
    F32 = dt.float32
    AF = mybir.ActivationFunctionType

    nc = bacc.Bacc()

    # ---- DRAM parameters ----
    x_d = nc.declare_dram_parameter("x", [IMGS_PER_CORE, C, N], F32, isOutput=False)
    wg1_d = nc.declare_dram_parameter("wg1", [128, 2, 256], F16, isOutput=False)
    wp_d = nc.declare_dram_parameter("wp", [128, 2, 512], F16, isOutput=False)
    wq_d = nc.declare_dram_parameter("wq", [128, 2, 512], F16, isOutput=False)
    wg2_d = nc.declare_dram_parameter("wg2", [128, 4, 256], F16, isOutput=False)
    wf1_d = nc.declare_dram_parameter("wf1", [128, 2, 1024], F16, isOutput=False)
    wf2_d = nc.declare_dram_parameter("wf2", [128, 8, 256], F16, isOutput=False)
    wb1_d = nc.declare_dram_parameter("wb1", [128, 2, 64], F16, isOutput=False)
    wb2_d = nc.declare_dram_parameter("wb2", [64, 9, 64], F16, isOutput=False)
    wb3_d = nc.declare_dram_parameter("wb3", [64, 256], F16, isOutput=False)
    bt1_d = nc.declare_dram_parameter("bt1", [128, 2], F32, isOutput=False)
    bt2_d = nc.declare_dram_parameter("bt2", [128, 2], F32, isOutput=False)
    bbp_d = nc.declare_dram_parameter("bbp", [128, 4], F32, isOutput=False)
    bbf1_d = nc.declare_dram_parameter("bbf1", [128, 8], F32, isOutput=False)
    bbf2_d = nc.declare_dram_parameter("bbf2", [128, 2], F32, isOutput=False)
    btb1_d = nc.declare_dram_parameter("btb1", [64, 1], F32, isOutput=False)
    btb2_d = nc.declare_dram_parameter("btb2", [64, 1], F32, isOutput=False)
    btb3_d = nc.declare_dram_parameter("btb3", [128, 2], F32, isOutput=False)
    bsf_d = nc.declare_dram_parameter("bsf", [128, 2], F32, isOutput=False)
    btf_d = nc.declare_dram_parameter("btf", [128, 2], F32, isOutput=False)
    expb_d = nc.declare_dram_parameter("expb", [128, 1], F32, isOutput=False)
    lnb_d = nc.declare_dram_parameter("lnb", [128, 1], F32, isOutput=False)
    selftpl_d = nc.declare_dram_parameter("selftpl", [128, 80], dt.uint16,
                                          isOutput=False)
    out_d = nc.declare_dram_parameter("out", [IMGS_PER_CORE, C, N], F32,
                                      isOutput=True)
    dbg = bool(os.environ.get("KBENCH_DEBUG"))
    if dbg:
        dbg_ix = nc.declare_dram_parameter("dbg_ix", [128, 80], dt.uint16,
                                           isOutput=True)
        dbg_S = nc.declare_dram_parameter("dbg_S", [128, N], BF16, isOutput=True)
        dbg_STt = nc.declare_dram_parameter("dbg_STt", [128, 8, 8, 128], BF16,
                                            isOutput=True)
        dbg_e = nc.declare_dram_parameter("dbg_e", [128, 4, N], F16,
                                          isOutput=True)
        dbg_expq = nc.declare_dram_parameter("dbg_expq", [128, 4, N], BF16,
                                             isOutput=True)
        dbg_lnq = nc.declare_dram_parameter("dbg_lnq", [128, 4, N], F16,
                                            isOutput=True)
        dbg_feat = nc.declare_dram_parameter("dbg_feat", [128, 2, N], F16,
                                             isOutput=True)

    with TileContext(nc) as tc:
        import contextlib
        ctx = contextlib.ExitStack()
        with ctx:
            consts = ctx.enter_context(tc.tile_pool(name="consts", bufs=1))
            p_xc = ctx.enter_context(tc.tile_pool(name="xc", bufs=2))
            p_feat = ctx.enter_context(tc.tile_pool(name="feat", bufs=2))
            p_xn = ctx.enter_context(tc.tile_pool(name="xn", bufs=2))
            p_sc = ctx.enter_context(tc.tile_pool(name="sc", bufs=3))
            p_qe = ctx.enter_context(tc.tile_pool(name="qe", bufs=2))
            p_S = ctx.enter_context(tc.tile_pool(name="S", bufs=2))
            p_STt = ctx.enter_context(tc.tile_pool(name="STt", bufs=1))
            p_lnq = ctx.enter_context(tc.tile_pool(name="lnq", bufs=2))
            p_p = ctx.enter_context(tc.tile_pool(name="p", bufs=2))
            p_h = ctx.enter_context(tc.tile_pool(name="h", bufs=3))
            p_f1o = ctx.enter_context(tc.tile_pool(name="f1o", bufs=1))
            p_b = ctx.enter_context(tc.tile_pool(name="b", bufs=3))
            p_out = ctx.enter_context(tc.tile_pool(name="out", bufs=2))
            p_ix = ctx.enter_context(tc.tile_pool(name="ix", bufs=2))
            p_mx = ctx.enter_context(tc.tile_pool(name="mx", bufs=3))
            ps1024 = ctx.enter_context(
                tc.tile_pool(name="ps1024", bufs=3, space="PSUM"))
            psmall = ctx.enter_context(
                tc.tile_pool(name="psmall", bufs=2, space="PSUM"))

            # ---- constants / weights (loaded once) ----
            def load(name, shape, dtype, src):
                t = consts.tile(shape, dtype, name=name)
                nc.sync.dma_start(out=t[:], in_=src[:])
                return t

            wg1 = load("wg1s", [128, 2, 256], F16, wg1_d)
            wp = load("wps", [128, 2, 512], F16, wp_d)
            wq = load("wqs", [128, 2, 512], F16, wq_d)
            wg2 = load("wg2s", [128, 4, 256], F16, wg2_d)
            wf1 = load("wf1s", [128, 2, 1024], F16, wf1_d)
            wf2 = load("wf2s", [128, 8, 256], F16, wf2_d)
            wb1 = load("wb1s", [128, 2, 64], F16, wb1_d)
            wb2 = load("wb2s", [64, 9, 64], F16, wb2_d)
            wb3 = load("wb3s", [64, 256], F16, wb3_d)
            bt1 = load("bt1s", [128, 2], F32, bt1_d)
            bt2 = load("bt2s", [128, 2], F32, bt2_d)
            bbp = load("bbps", [128, 4], F32, bbp_d)
            bbf1 = load("bbf1s", [128, 8], F32, bbf1_d)
            bbf2 = load("bbf2s", [128, 2], F32, bbf2_d)
            btb1 = load("btb1s", [64, 1], F32, btb1_d)
            btb2 = load("btb2s", [64, 1], F32, btb2_d)
            btb3 = load("btb3s", [128, 2], F32, btb3_d)
            bsf = load("bsfs", [128, 2], F32, bsf_d)
            btf = load("btfs", [128, 2], F32, btf_d)
            expb = load("expbs", [128, 1], F32, expb_d)
            lnb = load("lnbs", [128, 1], F32, lnb_d)
            selftpl = load("selftpls", [128, 80], dt.uint16, selftpl_d)

            ident = consts.tile([128, 128], F16, name="ident")
            make_identity(nc, ident[:])
            negid = consts.tile([128, 128], F16, name="negid")
            nc.scalar.activation(out=negid[:], in_=ident[:], func=AF.Copy,
                                 scale=NEG_BIG)
            ones = consts.tile([128, 128], F16, name="ones")
            nc.gpsimd.memset(ones[:], 1.0)
            onesk = consts.tile([128, 16], BF16, name="onesk")
            nc.gpsimd.memset(onesk[:], 1.0)
            # idbig[k, f] = 1 iff f == k + 384 (shifted identity for diag-kill)
            idbig = consts.tile([128, 1024], F16, name="idbig")
            nc.gpsimd.memset(idbig[:], 0.0)
            nc.gpsimd.affine_select(
                out=idbig[:], in_=idbig[:],
                compare_op=mybir.AluOpType.not_equal, fill=1.0,
                base=384, pattern=[[-1, 1024]], channel_multiplier=1)

            # per-image state carried from head to tail
            st = [{} for _ in range(IMGS_PER_CORE)]

            # ============== HEAD: load, g1, norms, sim/top8/S, q, p =======
            def head(img):
                s = st[img]
                xc = p_xc.tile([128, 2, N], F16, name="xc")
                for t in range(2):
                    # cast f32->f16 during DMA (SWDGE)
                    nc.gpsimd.dma_start(out=xc[:, t, :],
                                        in_=x_d[img, t * 128:(t + 1) * 128, :])
                s['xc'] = xc

                # ---- g1: featT [128, 2, N] f16 ----
                # (kt-outer loops everywhere: one LDWEIGHTS serves both
                # nb-halves, so matmuls stream back-to-back)
                featT = p_feat.tile([128, 2, N], F16, name="featT")
                for to in range(2):
                    ps = ps1024.tile([128, N], F32, name="ps_g1", tag="ps")
                    for kt in range(2):
                        for nb in range(2):
                            nc.tensor.matmul(
                                ps[:, nb * 512:(nb + 1) * 512],
                                lhsT=wg1[:, kt, to * 128:(to + 1) * 128],
                                rhs=xc[:, kt, nb * 512:(nb + 1) * 512],
                                start=(kt == 0), stop=(kt == 1))
                    nc.scalar.activation(out=featT[:, to, :], in_=ps[:],
                                         func=AF.Identity, bias=bt1[:, to:to + 1])

                # ---- q -> expq (t*Wq folded; exp bias = -t*c) ----
                # (emitted before the norm chain: q/p only need featT, and
                # they keep the PE busy through the fsq/rsqrt/xnT stalls so
                # HAM stays warm)
                expq = p_qe.tile([128, 4, N], BF16, name="expq", tag="qe")
                for pair in range(4):
                    ps = ps1024.tile([128, N], F32, name="ps_q", tag="ps")
                    for sub in range(2):
                        nt = 2 * pair + sub
                        for kt in range(2):
                            nc.tensor.matmul(
                                ps[:, sub * 512:(sub + 1) * 512],
                                lhsT=featT[:, kt, nt * 128:(nt + 1) * 128],
                                rhs=wq[:, kt, :], start=(kt == 0), stop=(kt == 1))
                    nc.scalar.activation(out=expq[:, pair, :], in_=ps[:],
                                         func=AF.Exp, bias=expb[:, 0:1])
                if dbg and img == 0:
                    nc.sync.dma_start(out=dbg_expq[:], in_=expq[:])
                s['expq'] = expq

                # ---- p^T [128, 4, N] f16 (ch-part, bias + c folded) ----
                pT = p_p.tile([128, 4, N], F16, name="pT")
                for to in range(4):
                    ps = ps1024.tile([128, N], F32, name="ps_p", tag="ps")
                    for kt in range(2):
                        for nb in range(2):
                            nc.tensor.matmul(
                                ps[:, nb * 512:(nb + 1) * 512],
                                lhsT=wp[:, kt, to * 128:(to + 1) * 128],
                                rhs=featT[:, kt, nb * 512:(nb + 1) * 512],
                                start=(kt == 0), stop=(kt == 1))
                    nc.scalar.activation(out=pT[:, to, :], in_=ps[:],
                                         func=AF.Identity, bias=bbp[:, to:to + 1])
                s['pT'] = pT

                # ---- norms -> xnT (rhs-side normalized) ----
                fsq = p_sc.tile([128, 2, N], F16, name="fsq", tag="sc")
                nc.vector.tensor_mul(fsq[:], featT[:], featT[:])
                # rsqrt only feeds the kNN ranking (monotone use) — the
                # gated-accuracy LUT is fine here; n2 >= O(100), no eps
                invn = p_mx.tile([1, N], F16, name="invn", tag="invn")
                for nb in range(2):
                    ps1 = psmall.tile([1, 512], F32, name="ps_n2")
                    for kt in range(2):
                        nc.tensor.matmul(
                            ps1[:], lhsT=ones[:, 0:1],
                            rhs=fsq[:, kt, nb * 512:(nb + 1) * 512],
                            start=(kt == 0), stop=(kt == 1))
                    nc.scalar.activation(out=invn[:, nb * 512:(nb + 1) * 512],
                                         in_=ps1[:], func=AF.Abs_reciprocal_sqrt)
                invnb = p_sc.tile([128, N], F16, name="invnb", tag="sc")
                psb = ps1024.tile([128, N], F32, name="ps_bc", tag="ps")
                for nb in range(2):
                    nc.tensor.matmul(psb[:, nb * 512:(nb + 1) * 512],
                                     lhsT=ones[0:1, :],
                                     rhs=invn[:, nb * 512:(nb + 1) * 512],
                                     start=True, stop=True)
                nc.scalar.activation(out=invnb[:], in_=psb[:], func=AF.Copy)
                xnT = p_xn.tile([128, 2, N], F16, name="xnT")
                for t in range(2):
                    nc.vector.tensor_mul(xnT[:, t, :], featT[:, t, :], invnb[:])

                # ---- sim + top8 + S + S^T, per 128-node block ----
                ixbuf = p_ix.tile([128, 8, 10], dt.uint16, name="ixbuf")
                nc.vector.tensor_copy(
                    ixbuf[:].rearrange("p a b -> p (a b)"), selftpl[:])
                STt = p_STt.tile([128, 8, 8, 128], BF16, name="STt")
                s['STt'] = STt
                for I in range(8):
                    ps = ps1024.tile([128, N], F32, name="ps_sim", tag="ps")
                    dcb = I // 4
                    for kt in range(2):
                        for cb in range(2):
                            nc.tensor.matmul(
                                ps[:, cb * 512:(cb + 1) * 512],
                                lhsT=featT[:, kt, I * 128:(I + 1) * 128],
                                rhs=xnT[:, kt, cb * 512:(cb + 1) * 512],
                                start=(kt == 0),
                                stop=(kt == 1 and cb != dcb))
                    off2 = (I % 4) * 128
                    nc.tensor.matmul(
                        ps[:, dcb * 512:(dcb + 1) * 512], lhsT=negid[:],
                        rhs=idbig[:, 384 - off2:896 - off2],
                        start=False, stop=True)
                    # mx must be f32: find_index8 matches exact values, so
                    # in_max and in_values (PSUM f32) must share precision
                    mx = p_mx.tile([128, 8], F32, name="mx", tag="mx")
                    nc.vector.max(out=mx[:], in_=ps[:])
                    nc.vector.max_index(out=ixbuf[:, I, 1:9], in_max=mx[:],
                                        in_values=ps[:])
                    S_I = p_S.tile([128, N], BF16, name="S_I")
                    nc.gpsimd.local_scatter(
                        out_ap=S_I[:], data_ap=onesk[:, 0:10],
                        idxs_ap=ixbuf[:, I, :].bitcast(dt.int16),
                        channels=128, num_elems=N, num_idxs=10)
                    nc.sync.dma_start_transpose(out=STt[:, I], in_=S_I[:])
                    if dbg and img == 0 and I == 0:
                        nc.sync.dma_start(out=dbg_S[:], in_=S_I[:])
                if dbg and img == 0:
                    nc.sync.dma_start(out=dbg_STt[:], in_=STt[:])
                    nc.sync.dma_start(out=dbg_ix[:],
                                      in_=ixbuf[:].rearrange("p a b -> p (a b)"))
                    nc.sync.dma_start(out=dbg_feat[:], in_=featT[:])

            # ============== TAIL: agg, e, g2, FFN, bottleneck, out ========
            def tail(img):
                s = st[img]
                xc, expq, pT, STt = s['xc'], s['expq'], s['pT'], s['STt']

                # ---- agg: lnqT [128, 4, N] f16 = ln(expq^T @ S^T) ----
                # lnqT holds ln(agg)/t, computed on the DVE from the f32
                # exponent bits (the ACT Ln LUT clamps below ~2^-66, which
                # floors 20% of entries)
                lnqT = p_lnq.tile([128, 4, N], F16, name="lnqT")
                for cb in range(4):
                    ps = ps1024.tile([128, N], F32, name="ps_agg", tag="ps")
                    for kt in range(8):
                        for half in range(2):
                            nc.tensor.matmul(
                                ps[:, half * 512:(half + 1) * 512],
                                lhsT=expq[:, kt // 2,
                                          (kt % 2) * 512 + cb * 128:
                                          (kt % 2) * 512 + cb * 128 + 128],
                                rhs=STt[:, half * 4:(half + 1) * 4, kt, :],
                                start=(kt == 0), stop=(kt == 7))
                    cast32 = p_sc.tile([128, N], F32, name="lncast", tag="sc")
                    nc.vector.tensor_copy(cast32[:], ps[:].bitcast(dt.int32))
                    nc.vector.tensor_scalar(
                        out=lnqT[:, cb, :], in0=cast32[:],
                        scalar1=LN_ALPHA, scalar2=LN_BETA,
                        op0=mybir.AluOpType.mult, op1=mybir.AluOpType.add)

                if dbg and img == 0:
                    nc.sync.dma_start(out=dbg_lnq[:], in_=lnqT[:])
                # ---- e = relu(pT + lnqT) (lnqT already scaled by 1/t) ----
                eT = p_qe.tile([128, 4, N], F16, name="eT", tag="qe")
                nc.vector.tensor_add(eT[:], lnqT[:], pT[:])
                nc.vector.tensor_scalar_max(eT[:], eT[:], 0.0)
                if dbg and img == 0:
                    nc.sync.dma_start(out=dbg_e[:], in_=eT[:])

                # ---- g2 + residual -> h f16 ----
                h = p_h.tile([128, 2, N], F16, name="h", tag="h")
                for to in range(2):
                    ps = ps1024.tile([128, N], F32, name="ps_g2", tag="ps")
                    for kt in range(4):
                        for nb in range(2):
                            nc.tensor.matmul(
                                ps[:, nb * 512:(nb + 1) * 512],
                                lhsT=wg2[:, kt, to * 128:(to + 1) * 128],
                                rhs=eT[:, kt, nb * 512:(nb + 1) * 512],
                                start=(kt == 0), stop=(kt == 3))
                    tmp = p_sc.tile([128, N], F32, name="g2tmp", tag="sc")
                    nc.scalar.activation(out=tmp[:], in_=ps[:],
                                         func=AF.Identity, bias=bt2[:, to:to + 1])
                    nc.vector.tensor_add(h[:, to, :], tmp[:], xc[:, to, :])

                # ---- FFN ----
                f1o = p_f1o.tile([128, 8, N], F16, name="f1o")
                for to in range(8):
                    ps = ps1024.tile([128, N], F32, name="ps_f1", tag="ps")
                    for kt in range(2):
                        for nb in range(2):
                            nc.tensor.matmul(
                                ps[:, nb * 512:(nb + 1) * 512],
                                lhsT=wf1[:, kt, to * 128:(to + 1) * 128],
                                rhs=h[:, kt, nb * 512:(nb + 1) * 512],
                                start=(kt == 0), stop=(kt == 1))
                    nc.scalar.activation(out=f1o[:, to, :], in_=ps[:],
                                         func=AF.Relu, bias=bbf1[:, to:to + 1])
                h2 = p_h.tile([128, 2, N], F16, name="h2", tag="h")
                for to in range(2):
                    ps = ps1024.tile([128, N], F32, name="ps_f2", tag="ps")
                    for kt in range(8):
                        for nb in range(2):
                            nc.tensor.matmul(
                                ps[:, nb * 512:(nb + 1) * 512],
                                lhsT=wf2[:, kt, to * 128:(to + 1) * 128],
                                rhs=f1o[:, kt, nb * 512:(nb + 1) * 512],
                                start=(kt == 0), stop=(kt == 7))
                    tmp = p_sc.tile([128, N], F32, name="f2tmp", tag="sc")
                    nc.scalar.activation(out=tmp[:], in_=ps[:],
                                         func=AF.Identity, bias=bbf2[:, to:to + 1])
                    nc.vector.tensor_add(h2[:, to, :], tmp[:], h[:, to, :])

                # ---- bottleneck ----
                b1o = p_b.tile([64, N], F16, name="b1o", tag="b")
                psb1 = ps1024.tile([128, N], F32, name="ps_b1", tag="ps")
                for kt in range(2):
                    for nb in range(2):
                        nc.tensor.matmul(
                            psb1[0:64, nb * 512:(nb + 1) * 512],
                            lhsT=wb1[:, kt, :],
                            rhs=h2[:, kt, nb * 512:(nb + 1) * 512],
                            start=(kt == 0), stop=(kt == 1))
                nc.scalar.activation(out=b1o[:], in_=psb1[0:64, :],
                                     func=AF.Relu, bias=btb1[:, 0:1])
                pad = p_b.tile([64, 34 * 34], F16, name="pad", tag="b")
                nc.vector.memset(pad[:], 0.0)
                pad3 = pad[:].rearrange("p (r c) -> p r c", r=34)
                b1v = b1o[:].rearrange("p (r c) -> p r c", r=32)
                nc.vector.tensor_copy(pad3[:, 1:33, 1:33], b1v)
                b2o = p_b.tile([64, N], F16, name="b2o", tag="b")
                psb2 = ps1024.tile([128, N], F32, name="ps_b2", tag="ps")
                for tap in range(9):
                    dy, dx = tap // 3, tap % 3
                    for nb in range(2):
                        rhs = pad3[:, 16 * nb + dy:16 * nb + dy + 16, dx:dx + 32]
                        nc.tensor.matmul(psb2[0:64, nb * 512:(nb + 1) * 512],
                                         lhsT=wb2[:, tap, :], rhs=rhs,
                                         start=(tap == 0), stop=(tap == 8))
                nc.scalar.activation(out=b2o[:], in_=psb2[0:64, :],
                                     func=AF.Relu, bias=btb2[:, 0:1])
                b3o = p_sc.tile([128, 2, N], F16, name="b3o", tag="sc")
                for to in range(2):
                    ps = ps1024.tile([128, N], F32, name="ps_b3", tag="ps")
                    for nb in range(2):
                        nc.tensor.matmul(
                            ps[:, nb * 512:(nb + 1) * 512],
                            lhsT=wb3[:, to * 128:(to + 1) * 128],
                            rhs=b2o[:, nb * 512:(nb + 1) * 512],
                            start=True, stop=True)
                    nc.scalar.activation(out=b3o[:, to, :], in_=ps[:],
                                         func=AF.Identity, bias=btb3[:, to:to + 1])

                # ---- o3 = b3o + h2; fin = o3 + x; out = sf*fin + tf ----
                o3 = p_h.tile([128, 2, N], F16, name="o3", tag="h")
                nc.vector.tensor_add(o3[:], b3o[:], h2[:])
                fin = p_lnq.tile([128, 2, N], F16, name="fin", tag="fin")
                nc.vector.tensor_add(fin[:], o3[:], xc[:])
                for t in range(2):
                    out32 = p_out.tile([128, N], F32, name="out32")
                    nc.scalar.activation(out=out32[:], in_=fin[:, t, :],
                                         func=AF.Identity, scale=bsf[:, t:t + 1],
                                         bias=btf[:, t:t + 1])
                    nc.sync.dma_start(out=out_d[img, t * 128:(t + 1) * 128, :],
                                      in_=out32[:])

            for img in range(IMGS_PER_CORE):
                head(img)
            for img in range(IMGS_PER_CORE):
                tail(img)

    nc.finalize()
    return nc


# --------------------------------------------------------------------------
# entry point
# --------------------------------------------------------------------------
def kernel(**inputs):
    inp = {k: np.asarray(v) for k, v in inputs.items()}
    w = _prep_weights(inp)

    if 'nc' not in _cache:
        _cache['nc'] = _build_bass()
    nc = _cache['nc']

    x = inp['x'].astype(np.float32).reshape(B, C, N)
    in_maps = []
    for c in range(N_CORES):
        m = {'x': np.ascontiguousarray(x[c * 2:(c + 1) * 2])}
        m.update({k: v for k, v in w.items()})
        in_maps.append(m)

    from concourse.bass_utils import run_bass_kernel_spmd
    trace = bool(os.environ.get("KBENCH_TRACE"))
    res = run_bass_kernel_spmd(nc, in_maps, core_ids=list(range(N_CORES)),
                               trace=trace)
    _cache['exec_time_ns'] = res.exec_time_ns
    _cache['results'] = res
    out = np.zeros((B, C, N), np.float32)
    for c in range(N_CORES):
        out[c * 2:(c + 1) * 2] = res.results[c]['out']
    return out.reshape(B, C, H, W)


# revision 41
# speedup vs baseline: 2.4376x; 1.3351x over previous
"""Trainium2 Bass kernel for nn_Block_17033840296551 (GNN message passing block).

Data-parallel over batch: 16 images -> 8 cores x 2 images. Each core runs the
full block (g1 conv -> kNN top-9 -> EdgeConv max-agg -> g2 -> FFN -> bottleneck
-> final BN) on its 2 images with no cross-core communication.

v2 design (LSE EdgeConv — no neighbor gather):
  * All BNs folded into conv weights/biases on host.
  * EdgeConv decomposed: e[n,k] = p[n] + q[idx[n,k]], p = (Wa-Wb)@feat + b,
    q = Wb@feat; max_k relu(e) = relu(p + max_k q).
  * max_k q replaced by log-sum-exp: max_k q ~= c + ln(sum_k exp(t(q_k-c)))/t
    with t=30, c=2.0. The sum over the 9-hot neighbor set is a matmul
    S^T.T @ expq on the PE (S built by gpsimd local_scatter from the top-8
    indices; S^T via tiled xbar dma transpose). Kills the descriptor-
    generation-bound dma_gather (~160us/core) plus the DVE max-fold tree.
  * kNN: scores s[n,m] = <feat_n, feat_m/||feat_m||> rank-equivalent to cosine
    per row; self removed via -BIG diagonal (extra identity matmul into PSUM);
    DVE Max8/FindIndex8 read scores straight from PSUM (no SBUF sim buffer).
  * f16 matmul inputs (bf16 for the exp path: f16 overflows at e^11), f32
    PSUM, f16 residual stream, [128,1024] two-bank PSUM drains.
  * Two-phase emission (head: g1..sim..sel..q..p, tail: agg..FFN..bottleneck)
    interleaved across the 2 images so PE/DVE/ACT/DMA overlap.
"""

import os
import numpy as np

# problem constants (hardcoded per harness contract)
B, C, H, W = 16, 256, 32, 32
N = H * W           # 1024 pixels per image
K = 9
EPS = 1e-5
IMGS_PER_CORE = 2
N_CORES = 8
NEG_BIG = -30000.0
T_LSE = 30.0
C_LSE = 2.0
LN2 = 0.6931471805599453
# ln(x) ~= LN2 * (bitcast_int32(x) * 2^-23 - 126.957)  (max err ~0.03 in ln)
LN_ALPHA = LN2 / T_LSE / (1 << 23)
LN_BETA = -126.957 * LN2 / T_LSE

_cache = {}


# --------------------------------------------------------------------------
# host-side preprocessing
# --------------------------------------------------------------------------
def _bn_fold(p):
    g, b, m, v = np.asarray(p, np.float32)
    s = g / np.sqrt(v + EPS)
    t = b - m * s
    return s, t


def _pack_kxm(w_t, part=128):
    """[K, M] -> [part, K//part, M] (partition-major K tiling)."""
    Kd, M = w_t.shape
    kt = Kd // part
    return np.ascontiguousarray(w_t.reshape(kt, part, M).transpose(1, 0, 2))


def _pack_bias(b, part=128):
    n = b.shape[0]
    t = n // part
    return np.ascontiguousarray(b.reshape(t, part).T)  # [part, t]


def _make_selftpl():
    """ixbuf init template [128, 8, 10] uint16: col0 = self node id, col9 =
    0xFFFF (-1 as int16: ignored by local_scatter), cols 1..8 overwritten by
    find_index8."""
    tpl = np.zeros((128, 8, 10), np.uint16)
    for I in range(8):
        tpl[:, I, 0] = I * 128 + np.arange(128)
        tpl[:, I, 9] = 0xFFFF
    return np.ascontiguousarray(tpl.reshape(128, 80))


def _prep_weights(inp):
    f16 = np.float16
    s1, t1 = _bn_fold(inp['g1_bn'])
    Wg1 = s1[:, None] * inp['g1_w']
    s2, t2 = _bn_fold(inp['g2_bn'])
    Wg2 = s2[:, None] * inp['g2_w']
    sf1, tf1 = _bn_fold(inp['f1_bn'])
    Wf1 = sf1[:, None] * inp['f1_w']
    bf1 = sf1 * inp['f1_b'] + tf1
    sf2, tf2 = _bn_fold(inp['f2_bn'])
    Wf2 = sf2[:, None] * inp['f2_w']
    bf2 = sf2 * inp['f2_b'] + tf2
    sb1, tb1 = _bn_fold(inp['b1_bn'])
    Wb1 = sb1[:, None] * inp['b1_w']
    sb2, tb2 = _bn_fold(inp['b2_bn'])
    Wb2 = sb2[:, None, None, None] * inp['b2_w']
    sb3, tb3 = _bn_fold(inp['b3_bn'])
    Wb3 = sb3[:, None] * inp['b3_w']
    sf, tf = _bn_fold(inp['bnf'])

    A = inp['edge_w'][:, :C]
    Bm = inp['edge_w'][:, C:]
    Wp = A - Bm
    Wq = Bm
    bp = inp['edge_b'] + C_LSE          # LSE shift folded into the p bias

    wb2_t = np.zeros((64, 9, 64), f16)
    for dy in range(3):
        for dx in range(3):
            wb2_t[:, dy * 3 + dx, :] = Wb2[:, :, dy, dx].T.astype(f16)

    return {
        'wg1': _pack_kxm(Wg1.T.astype(f16)),                # [128,2,256]
        'wp': _pack_kxm(Wp.T.astype(f16)),                  # [128,2,512]
        'wq': _pack_kxm((T_LSE * Wq).T.astype(f16)),        # [128,2,512] (t*Wq)
        'wg2': _pack_kxm(Wg2.T.astype(f16)),                # [128,4,256]
        'wf1': _pack_kxm(Wf1.T.astype(f16)),                # [128,2,1024]
        'wf2': _pack_kxm(Wf2.T.astype(f16)),                # [128,8,256]
        'wb1': _pack_kxm(Wb1.T.astype(f16)),                # [128,2,64]
        'wb2': wb2_t,                                        # [64,9,64]
        'wb3': Wb3.T.astype(f16),                            # [64,256]
        'bt1': _pack_bias(t1),                               # [128,2] f32
        'bt2': _pack_bias(t2),
        'bbp': _pack_bias(bp),                               # [128,4]
        'bbf1': _pack_bias(bf1),                             # [128,8]
        'bbf2': _pack_bias(bf2),
        'btb1': np.ascontiguousarray(tb1[:, None].astype(np.float32)),  # [64,1]
        'btb2': np.ascontiguousarray(tb2[:, None].astype(np.float32)),
        'btb3': _pack_bias(tb3),
        'bsf': _pack_bias(sf),
        'btf': _pack_bias(tf),
        'expb': np.full((128, 1), -T_LSE * C_LSE, np.float32),
        'lnb': np.full((128, 1), 1e-30, np.float32),
        'selftpl': _make_selftpl(),                          # [128,80] u16
    }


# --------------------------------------------------------------------------
# device kernel builder
# --------------------------------------------------------------------------
def _build_bass():
    import concourse.bass as bass
    import concourse.mybir as mybir
    from concourse import bacc
    from concourse.tile import TileContext
    from concourse.masks import make_identity

    dt = mybir.dt
    F16 = dt.float16
    BF16 = dt.bfloat16
    F32 = dt.float32
    AF = mybir.ActivationFunctionType

    nc = bacc.Bacc()

    # ---- DRAM parameters ----
    x_d = nc.declare_dram_parameter("x", [IMGS_PER_CORE, C, N], F32, isOutput=False)
    wg1_d = nc.declare_dram_parameter("wg1", [128, 2, 256], F16, isOutput=False)
    wp_d = nc.declare_dram_parameter("wp", [128, 2, 512], F16, isOutput=False)
    wq_d = nc.declare_dram_parameter("wq", [128, 2, 512], F16, isOutput=False)
    wg2_d = nc.declare_dram_parameter("wg2", [128, 4, 256], F16, isOutput=False)
    wf1_d = nc.declare_dram_parameter("wf1", [128, 2, 1024], F16, isOutput=False)
    wf2_d = nc.declare_dram_parameter("wf2", [128, 8, 256], F16, isOutput=False)
    wb1_d = nc.declare_dram_parameter("wb1", [128, 2, 64], F16, isOutput=False)
    wb2_d = nc.declare_dram_parameter("wb2", [64, 9, 64], F16, isOutput=False)
    wb3_d = nc.declare_dram_parameter("wb3", [64, 256], F16, isOutput=False)
    bt1_d = nc.declare_dram_parameter("bt1", [128, 2], F32, isOutput=False)
    bt2_d = nc.declare_dram_parameter("bt2", [128, 2], F32, isOutput=False)
    bbp_d = nc.declare_dram_parameter("bbp", [128, 4], F32, isOutput=False)
    bbf1_d = nc.declare_dram_parameter("bbf1", [128, 8], F32, isOutput=False)
    bbf2_d = nc.declare_dram_parameter("bbf2", [128, 2], F32, isOutput=False)
    btb1_d = nc.declare_dram_parameter("btb1", [64, 1], F32, isOutput=False)
    btb2_d = nc.declare_dram_parameter("btb2", [64, 1], F32, isOutput=False)
    btb3_d = nc.declare_dram_parameter("btb3", [128, 2], F32, isOutput=False)
    bsf_d = nc.declare_dram_parameter("bsf", [128, 2], F32, isOutput=False)
    btf_d = nc.declare_dram_parameter("btf", [128, 2], F32, isOutput=False)
    expb_d = nc.declare_dram_parameter("expb", [128, 1], F32, isOutput=False)
    lnb_d = nc.declare_dram_parameter("lnb", [128, 1], F32, isOutput=False)
    selftpl_d = nc.declare_dram_parameter("selftpl", [128, 80], dt.uint16,
                                          isOutput=False)
    out_d = nc.declare_dram_parameter("out", [IMGS_PER_CORE, C, N], F32,
                                      isOutput=True)
    dbg = bool(os.environ.get("KBENCH_DEBUG"))
    if dbg:
        dbg_ix = nc.declare_dram_parameter("dbg_ix", [128, 80], dt.uint16,
                                           isOutput=True)
        dbg_S = nc.declare_dram_parameter("dbg_S", [128, N], BF16, isOutput=True)
        dbg_STt = nc.declare_dram_parameter("dbg_STt", [128, 8, 8, 128], BF16,
                                            isOutput=True)
        dbg_e = nc.declare_dram_parameter("dbg_e", [128, 4, N], F16,
                                          isOutput=True)
        dbg_expq = nc.declare_dram_parameter("dbg_expq", [128, 4, N], BF16,
                                             isOutput=True)
        dbg_lnq = nc.declare_dram_parameter("dbg_lnq", [128, 4, N], F16,
                                            isOutput=True)
        dbg_feat = nc.declare_dram_parameter("dbg_feat", [128, 2, N], F16,
                                             isOutput=True)

    with TileContext(nc) as tc:
        import contextlib
        ctx = contextlib.ExitStack()
        with ctx:
            consts = ctx.enter_context(tc.tile_pool(name="consts", bufs=1))
            p_xc = ctx.enter_context(tc.tile_pool(name="xc", bufs=2))
            p_feat = ctx.enter_context(tc.tile_pool(name="feat", bufs=2))
            p_xn = ctx.enter_context(tc.tile_pool(name="xn", bufs=2))
            p_sc = ctx.enter_context(tc.tile_pool(name="sc", bufs=3))
            p_qe = ctx.enter_context(tc.tile_pool(name="qe", bufs=2))
            p_S = ctx.enter_context(tc.tile_pool(name="S", bufs=2))
            p_STt = ctx.enter_context(tc.tile_pool(name="STt", bufs=2))
            p_lnq = ctx.enter_context(tc.tile_pool(name="lnq", bufs=2))
            p_p = ctx.enter_context(tc.tile_pool(name="p", bufs=2))
            p_h = ctx.enter_context(tc.tile_pool(name="h", bufs=4))
            p_f1o = ctx.enter_context(tc.tile_pool(name="f1o", bufs=1))
            p_b = ctx.enter_context(tc.tile_pool(name="b", bufs=3))
            p_out = ctx.enter_context(tc.tile_pool(name="out", bufs=2))
            p_ix = ctx.enter_context(tc.tile_pool(name="ix", bufs=2))
            p_mx = ctx.enter_context(tc.tile_pool(name="mx", bufs=2))
            ps1024 = ctx.enter_context(
                tc.tile_pool(name="ps1024", bufs=3, space="PSUM"))
            psmall = ctx.enter_context(
                tc.tile_pool(name="psmall", bufs=2, space="PSUM"))

            # ---- constants / weights (loaded once) ----
            def load(name, shape, dtype, src):
                t = consts.tile(shape, dtype, name=name)
                nc.sync.dma_start(out=t[:], in_=src[:])
                return t

            wg1 = load("wg1s", [128, 2, 256], F16, wg1_d)
            wp = load("wps", [128, 2, 512], F16, wp_d)
            wq = load("wqs", [128, 2, 512], F16, wq_d)
            wg2 = load("wg2s", [128, 4, 256], F16, wg2_d)
            wf1 = load("wf1s", [128, 2, 1024], F16, wf1_d)
            wf2 = load("wf2s", [128, 8, 256], F16, wf2_d)
            wb1 = load("wb1s", [128, 2, 64], F16, wb1_d)
            wb2 = load("wb2s", [64, 9, 64], F16, wb2_d)
            wb3 = load("wb3s", [64, 256], F16, wb3_d)
            bt1 = load("bt1s", [128, 2], F32, bt1_d)
            bt2 = load("bt2s", [128, 2], F32, bt2_d)
            bbp = load("bbps", [128, 4], F32, bbp_d)
            bbf1 = load("bbf1s", [128, 8], F32, bbf1_d)
            bbf2 = load("bbf2s", [128, 2], F32, bbf2_d)
            btb1 = load("btb1s", [64, 1], F32, btb1_d)
            btb2 = load("btb2s", [64, 1], F32, btb2_d)
            btb3 = load("btb3s", [128, 2], F32, btb3_d)
            bsf = load("bsfs", [128, 2], F32, bsf_d)
            btf = load("btfs", [128, 2], F32, btf_d)
            expb = load("expbs", [128, 1], F32, expb_d)
            lnb = load("lnbs", [128, 1], F32, lnb_d)
            selftpl = load("selftpls", [128, 80], dt.uint16, selftpl_d)

            ident = consts.tile([128, 128], F16, name="ident")
            make_identity(nc, ident[:])
            negid = consts.tile([128, 128], F16, name="negid")
            nc.scalar.activation(out=negid[:], in_=ident[:], func=AF.Copy,
                                 scale=NEG_BIG)
            ones = consts.tile([128, 128], F16, name="ones")
            nc.gpsimd.memset(ones[:], 1.0)
            onesk = consts.tile([128, 16], BF16, name="onesk")
            nc.gpsimd.memset(onesk[:], 1.0)
            # idbig[k, f] = 1 iff f == k + 384 (shifted identity for diag-kill)
            idbig = consts.tile([128, 1024], F16, name="idbig")
            nc.gpsimd.memset(idbig[:], 0.0)
            nc.gpsimd.affine_select(
                out=idbig[:], in_=idbig[:],
                compare_op=mybir.AluOpType.not_equal, fill=1.0,
                base=384, pattern=[[-1, 1024]], channel_multiplier=1)

            # per-image state carried from head to tail
            st = [{} for _ in range(IMGS_PER_CORE)]

            # ============== HEAD: load, g1, norms, sim/top8/S, q, p =======
            def head(img):
                s = st[img]
                xc = p_xc.tile([128, 2, N], F16, name="xc")
                for t in range(2):
                    # cast f32->f16 during DMA (SWDGE)
                    nc.gpsimd.dma_start(out=xc[:, t, :],
                                        in_=x_d[img, t * 128:(t + 1) * 128, :])
                s['xc'] = xc

                # ---- g1: featT [128, 2, N] f16 ----
                # (kt-outer loops everywhere: one LDWEIGHTS serves both
                # nb-halves, so matmuls stream back-to-back)
                featT = p_feat.tile([128, 2, N], F16, name="featT")
                for to in range(2):
                    ps = ps1024.tile([128, N], F32, name="ps_g1", tag="ps")
                    for kt in range(2):
                        for nb in range(2):
                            nc.tensor.matmul(
                                ps[:, nb * 512:(nb + 1) * 512],
                                lhsT=wg1[:, kt, to * 128:(to + 1) * 128],
                                rhs=xc[:, kt, nb * 512:(nb + 1) * 512],
                                start=(kt == 0), stop=(kt == 1))
                    nc.scalar.activation(out=featT[:, to, :], in_=ps[:],
                                         func=AF.Identity, bias=bt1[:, to:to + 1])

                # ---- row norms first: the rsqrt ACT-table swap and the n2
                # matmuls run while the PE then chews q/p, so invnb is ready
                # by the time the bcast matmul needs it ----
                fsq = p_sc.tile([128, 2, N], F16, name="fsq", tag="sc")
                nc.vector.tensor_mul(fsq[:], featT[:], featT[:])
                invn = p_mx.tile([1, N], F16, name="invn", tag="invn")
                for nb in range(2):
                    ps1 = psmall.tile([1, 512], F32, name="ps_n2")
                    for kt in range(2):
                        nc.tensor.matmul(
                            ps1[:], lhsT=ones[:, 0:1],
                            rhs=fsq[:, kt, nb * 512:(nb + 1) * 512],
                            start=(kt == 0), stop=(kt == 1))
                    # rank-only use; the gated-accuracy LUT is fine here
                    nc.scalar.activation(out=invn[:, nb * 512:(nb + 1) * 512],
                                         in_=ps1[:], func=AF.Abs_reciprocal_sqrt)

                # ---- q -> expq (t*Wq folded; exp bias = -t*c) ----
                expq = p_qe.tile([128, 4, N], BF16, name="expq", tag="qe")
                for pair in range(4):
                    ps = ps1024.tile([128, N], F32, name="ps_q", tag="ps")
                    for sub in range(2):
                        nt = 2 * pair + sub
                        for kt in range(2):
                            nc.tensor.matmul(
                                ps[:, sub * 512:(sub + 1) * 512],
                                lhsT=featT[:, kt, nt * 128:(nt + 1) * 128],
                                rhs=wq[:, kt, :], start=(kt == 0), stop=(kt == 1))
                    nc.scalar.activation(out=expq[:, pair, :], in_=ps[:],
                                         func=AF.Exp, bias=expb[:, 0:1])
                if dbg and img == 0:
                    nc.sync.dma_start(out=dbg_expq[:], in_=expq[:])
                s['expq'] = expq

                # ---- p^T [128, 4, N] f16 (ch-part, bias + c folded) ----
                pT = p_p.tile([128, 4, N], F16, name="pT")
                for to in range(4):
                    ps = ps1024.tile([128, N], F32, name="ps_p", tag="ps")
                    for kt in range(2):
                        for nb in range(2):
                            nc.tensor.matmul(
                                ps[:, nb * 512:(nb + 1) * 512],
                                lhsT=wp[:, kt, to * 128:(to + 1) * 128],
                                rhs=featT[:, kt, nb * 512:(nb + 1) * 512],
                                start=(kt == 0), stop=(kt == 1))
                    nc.scalar.activation(out=pT[:, to, :], in_=ps[:],
                                         func=AF.Identity, bias=bbp[:, to:to + 1])
                s['pT'] = pT

                # ---- broadcast invn -> xnT (rhs-side normalized) ----
                invnb = p_sc.tile([128, N], F16, name="invnb", tag="sc")
                psb = ps1024.tile([128, N], F32, name="ps_bc", tag="ps")
                for nb in range(2):
                    nc.tensor.matmul(psb[:, nb * 512:(nb + 1) * 512],
                                     lhsT=ones[0:1, :],
                                     rhs=invn[:, nb * 512:(nb + 1) * 512],
                                     start=True, stop=True)
                nc.scalar.activation(out=invnb[:], in_=psb[:], func=AF.Copy)
                xnT = p_xn.tile([128, 2, N], F16, name="xnT")
                for t in range(2):
                    nc.vector.tensor_mul(xnT[:, t, :], featT[:, t, :], invnb[:])

                # ---- sim + top8 + S + S^T, per 128-node block ----
                ixbuf = p_ix.tile([128, 8, 10], dt.uint16, name="ixbuf")
                nc.vector.tensor_copy(
                    ixbuf[:].rearrange("p a b -> p (a b)"), selftpl[:])
                STt = p_STt.tile([128, 8, 8, 128], BF16, name="STt")
                s['STt'] = STt
                for I in range(8):
                    ps = ps1024.tile([128, N], F32, name="ps_sim", tag="ps")
                    dcb = I // 4
                    for kt in range(2):
                        for cb in range(2):
                            nc.tensor.matmul(
                                ps[:, cb * 512:(cb + 1) * 512],
                                lhsT=featT[:, kt, I * 128:(I + 1) * 128],
                                rhs=xnT[:, kt, cb * 512:(cb + 1) * 512],
                                start=(kt == 0),
                                stop=(kt == 1 and cb != dcb))
                    off2 = (I % 4) * 128
                    nc.tensor.matmul(
                        ps[:, dcb * 512:(dcb + 1) * 512], lhsT=negid[:],
                        rhs=idbig[:, 384 - off2:896 - off2],
                        start=False, stop=True)
                    # mx must be f32: find_index8 matches exact values, so
                    # in_max and in_values (PSUM f32) must share precision
                    mx = p_mx.tile([128, 8], F32, name="mx", tag="mx")
                    nc.vector.max(out=mx[:], in_=ps[:])
                    nc.vector.max_index(out=ixbuf[:, I, 1:9], in_max=mx[:],
                                        in_values=ps[:])
                    S_I = p_S.tile([128, N], BF16, name="S_I")
                    nc.gpsimd.local_scatter(
                        out_ap=S_I[:], data_ap=onesk[:, 0:10],
                        idxs_ap=ixbuf[:, I, :].bitcast(dt.int16),
                        channels=128, num_elems=N, num_idxs=10)
                    nc.sync.dma_start_transpose(out=STt[:, I], in_=S_I[:])
                    if dbg and img == 0 and I == 0:
                        nc.sync.dma_start(out=dbg_S[:], in_=S_I[:])
                if dbg and img == 0:
                    nc.sync.dma_start(out=dbg_STt[:], in_=STt[:])
                    nc.sync.dma_start(out=dbg_ix[:],
                                      in_=ixbuf[:].rearrange("p a b -> p (a b)"))
                    nc.sync.dma_start(out=dbg_feat[:], in_=featT[:])

            # ============== TAIL phase 1: agg + e =========================
            def agg_phase(img):
                s = st[img]
                expq, pT, STt = s['expq'], s['pT'], s['STt']

                # ---- agg: lnqT [128, 4, N] f16 = ln(expq^T @ S^T) ----
                # lnqT holds ln(agg)/t, computed on the DVE from the f32
                # exponent bits (the ACT Ln LUT clamps below ~2^-66, which
                # floors 20% of entries)
                lnqT = p_lnq.tile([128, 4, N], F16, name="lnqT")
                for cb in range(4):
                    ps = ps1024.tile([128, N], F32, name="ps_agg", tag="ps")
                    for kt in range(8):
                        for half in range(2):
                            nc.tensor.matmul(
                                ps[:, half * 512:(half + 1) * 512],
                                lhsT=expq[:, kt // 2,
                                          (kt % 2) * 512 + cb * 128:
                                          (kt % 2) * 512 + cb * 128 + 128],
                                rhs=STt[:, half * 4:(half + 1) * 4, kt, :],
                                start=(kt == 0), stop=(kt == 7))
                    nc.vector.tensor_scalar(
                        out=lnqT[:, cb, :], in0=ps[:].bitcast(dt.int32),
                        scalar1=LN_ALPHA, scalar2=LN_BETA,
                        op0=mybir.AluOpType.mult, op1=mybir.AluOpType.add)

                if dbg and img == 0:
                    nc.sync.dma_start(out=dbg_lnq[:], in_=lnqT[:])
                # ---- e = relu(pT + lnqT) (lnqT already scaled by 1/t) ----
                eT = p_qe.tile([128, 4, N], F16, name="eT", tag="qe")
                nc.vector.tensor_add(eT[:], lnqT[:], pT[:])
                nc.vector.tensor_scalar_max(eT[:], eT[:], 0.0)
                if dbg and img == 0:
                    nc.sync.dma_start(out=dbg_e[:], in_=eT[:])
                s['eT'] = eT

            # ============== TAIL phase 2: g2 + FFN ========================
            def ffn_phase(img):
                s = st[img]
                xc, eT = s['xc'], s['eT']

                # ---- g2 + residual -> h f16 ----
                h = p_h.tile([128, 2, N], F16, name="h", tag="h")
                for to in range(2):
                    ps = ps1024.tile([128, N], F32, name="ps_g2", tag="ps")
                    for kt in range(4):
                        for nb in range(2):
                            nc.tensor.matmul(
                                ps[:, nb * 512:(nb + 1) * 512],
                                lhsT=wg2[:, kt, to * 128:(to + 1) * 128],
                                rhs=eT[:, kt, nb * 512:(nb + 1) * 512],
                                start=(kt == 0), stop=(kt == 3))
                    tmp = p_sc.tile([128, N], F32, name="g2tmp", tag="sc")
                    nc.scalar.activation(out=tmp[:], in_=ps[:],
                                         func=AF.Identity, bias=bt2[:, to:to + 1])
                    nc.vector.tensor_add(h[:, to, :], tmp[:], xc[:, to, :])

                # ---- FFN ----
                f1o = p_f1o.tile([128, 8, N], F16, name="f1o")
                for to in range(8):
                    ps = ps1024.tile([128, N], F32, name="ps_f1", tag="ps")
                    for kt in range(2):
                        for nb in range(2):
                            nc.tensor.matmul(
                                ps[:, nb * 512:(nb + 1) * 512],
                                lhsT=wf1[:, kt, to * 128:(to + 1) * 128],
                                rhs=h[:, kt, nb * 512:(nb + 1) * 512],
                                start=(kt == 0), stop=(kt == 1))
                    nc.scalar.activation(out=f1o[:, to, :], in_=ps[:],
                                         func=AF.Relu, bias=bbf1[:, to:to + 1])
                h2 = p_h.tile([128, 2, N], F16, name="h2", tag="h")
                for to in range(2):
                    ps = ps1024.tile([128, N], F32, name="ps_f2", tag="ps")
                    for kt in range(8):
                        for nb in range(2):
                            nc.tensor.matmul(
                                ps[:, nb * 512:(nb + 1) * 512],
                                lhsT=wf2[:, kt, to * 128:(to + 1) * 128],
                                rhs=f1o[:, kt, nb * 512:(nb + 1) * 512],
                                start=(kt == 0), stop=(kt == 7))
                    tmp = p_sc.tile([128, N], F32, name="f2tmp", tag="sc")
                    nc.scalar.activation(out=tmp[:], in_=ps[:],
                                         func=AF.Identity, bias=bbf2[:, to:to + 1])
                    nc.vector.tensor_add(h2[:, to, :], tmp[:], h[:, to, :])
                s['h2'] = h2

            # ============== TAIL phase 3: bottleneck + out ================
            def bott_phase(img):
                s = st[img]
                xc, h2 = s['xc'], s['h2']

                b1o = p_b.tile([64, N], F16, name="b1o", tag="b")
                psb1 = ps1024.tile([128, N], F32, name="ps_b1", tag="ps")
                for kt in range(2):
                    for nb in range(2):
                        nc.tensor.matmul(
                            psb1[0:64, nb * 512:(nb + 1) * 512],
                            lhsT=wb1[:, kt, :],
                            rhs=h2[:, kt, nb * 512:(nb + 1) * 512],
                            start=(kt == 0), stop=(kt == 1))
                nc.scalar.activation(out=b1o[:], in_=psb1[0:64, :],
                                     func=AF.Relu, bias=btb1[:, 0:1])
                pad = p_b.tile([64, 34 * 34], F16, name="pad", tag="b")
                nc.vector.memset(pad[:], 0.0)
                pad3 = pad[:].rearrange("p (r c) -> p r c", r=34)
                b1v = b1o[:].rearrange("p (r c) -> p r c", r=32)
                nc.vector.tensor_copy(pad3[:, 1:33, 1:33], b1v)
                b2o = p_b.tile([64, N], F16, name="b2o", tag="b")
                psb2 = ps1024.tile([128, N], F32, name="ps_b2", tag="ps")
                for tap in range(9):
                    dy, dx = tap // 3, tap % 3
                    for nb in range(2):
                        rhs = pad3[:, 16 * nb + dy:16 * nb + dy + 16, dx:dx + 32]
                        nc.tensor.matmul(psb2[0:64, nb * 512:(nb + 1) * 512],
                                         lhsT=wb2[:, tap, :], rhs=rhs,
                                         start=(tap == 0), stop=(tap == 8))
                nc.scalar.activation(out=b2o[:], in_=psb2[0:64, :],
                                     func=AF.Relu, bias=btb2[:, 0:1])
                b3o = p_sc.tile([128, 2, N], F16, name="b3o", tag="sc")
                for to in range(2):
                    ps = ps1024.tile([128, N], F32, name="ps_b3", tag="ps")
                    for nb in range(2):
                        nc.tensor.matmul(
                            ps[:, nb * 512:(nb + 1) * 512],
                            lhsT=wb3[:, to * 128:(to + 1) * 128],
                            rhs=b2o[:, nb * 512:(nb + 1) * 512],
                            start=True, stop=True)
                    nc.scalar.activation(out=b3o[:, to, :], in_=ps[:],
                                         func=AF.Identity, bias=btb3[:, to:to + 1])

                # ---- o3 = b3o + h2; fin = o3 + x; out = sf*fin + tf ----
                o3 = p_h.tile([128, 2, N], F16, name="o3", tag="h")
                nc.vector.tensor_add(o3[:], b3o[:], h2[:])
                fin = p_lnq.tile([128, 2, N], F16, name="fin", tag="fin")
                nc.vector.tensor_add(fin[:], o3[:], xc[:])
                for t in range(2):
                    out32 = p_out.tile([128, N], F32, name="out32")
                    nc.scalar.activation(out=out32[:], in_=fin[:, t, :],
                                         func=AF.Identity, scale=bsf[:, t:t + 1],
                                         bias=btf[:, t:t + 1])
                    nc.sync.dma_start(out=out_d[img, t * 128:(t + 1) * 128, :],
                                      in_=out32[:])

            for img in range(IMGS_PER_CORE):
                head(img)
            for img in range(IMGS_PER_CORE):
                agg_phase(img)
            for img in range(IMGS_PER_CORE):
                ffn_phase(img)
            for img in range(IMGS_PER_CORE):
                bott_phase(img)

    nc.finalize()
    return nc


# --------------------------------------------------------------------------
# entry point
# --------------------------------------------------------------------------
def kernel(**inputs):
    inp = {k: np.asarray(v) for k, v in inputs.items()}
    w = _prep_weights(inp)

    if 'nc' not in _cache:
        _cache['nc'] = _build_bass()
    nc = _cache['nc']

    x = inp['x'].astype(np.float32).reshape(B, C, N)
    in_maps = []
    for c in range(N_CORES):
        m = {'x': np.ascontiguousarray(x[c * 2:(c + 1) * 2])}
        m.update({k: v for k, v in w.items()})
        in_maps.append(m)

    from concourse.bass_utils import run_bass_kernel_spmd
    trace = bool(os.environ.get("KBENCH_TRACE"))
    res = run_bass_kernel_spmd(nc, in_maps, core_ids=list(range(N_CORES)),
                               trace=trace)
    _cache['exec_time_ns'] = res.exec_time_ns
    _cache['results'] = res
    out = np.zeros((B, C, N), np.float32)
    for c in range(N_CORES):
        out[c * 2:(c + 1) * 2] = res.results[c]['out']
    return out.reshape(B, C, H, W)


# revision 43
# speedup vs baseline: 2.5362x; 1.0404x over previous
"""Trainium2 Bass kernel for nn_Block_17033840296551 (GNN message passing block).

Data-parallel over batch: 16 images -> 8 cores x 2 images. Each core runs the
full block (g1 conv -> kNN top-9 -> EdgeConv max-agg -> g2 -> FFN -> bottleneck
-> final BN) on its 2 images with no cross-core communication.

v2 design (LSE EdgeConv — no neighbor gather):
  * All BNs folded into conv weights/biases on host.
  * EdgeConv decomposed: e[n,k] = p[n] + q[idx[n,k]], p = (Wa-Wb)@feat + b,
    q = Wb@feat; max_k relu(e) = relu(p + max_k q).
  * max_k q replaced by log-sum-exp: max_k q ~= c + ln(sum_k exp(t(q_k-c)))/t
    with t=30, c=2.0. The sum over the 9-hot neighbor set is a matmul
    S^T.T @ expq on the PE (S built by gpsimd local_scatter from the top-8
    indices; S^T via tiled xbar dma transpose). Kills the descriptor-
    generation-bound dma_gather (~160us/core) plus the DVE max-fold tree.
  * kNN: scores s[n,m] = <feat_n, feat_m/||feat_m||> rank-equivalent to cosine
    per row; self removed via -BIG diagonal (extra identity matmul into PSUM);
    DVE Max8/FindIndex8 read scores straight from PSUM (no SBUF sim buffer).
  * f16 matmul inputs (bf16 for the exp path: f16 overflows at e^11), f32
    PSUM, f16 residual stream, [128,1024] two-bank PSUM drains.
  * Two-phase emission (head: g1..sim..sel..q..p, tail: agg..FFN..bottleneck)
    interleaved across the 2 images so PE/DVE/ACT/DMA overlap.
"""

import os
import numpy as np

# problem constants (hardcoded per harness contract)
B, C, H, W = 16, 256, 32, 32
N = H * W           # 1024 pixels per image
K = 9
EPS = 1e-5
IMGS_PER_CORE = 2
N_CORES = 8
NEG_BIG = -30000.0
T_LSE = 30.0
C_LSE = 2.0
LN2 = 0.6931471805599453
# ln(x) ~= LN2 * (bitcast_int32(x) * 2^-23 - 126.957)  (max err ~0.03 in ln)
LN_ALPHA = LN2 / T_LSE / (1 << 23)
LN_BETA = -126.957 * LN2 / T_LSE

_cache = {}


# --------------------------------------------------------------------------
# host-side preprocessing
# --------------------------------------------------------------------------
def _bn_fold(p):
    g, b, m, v = np.asarray(p, np.float32)
    s = g / np.sqrt(v + EPS)
    t = b - m * s
    return s, t


def _pack_kxm(w_t, part=128):
    """[K, M] -> [part, K//part, M] (partition-major K tiling)."""
    Kd, M = w_t.shape
    kt = Kd // part
    return np.ascontiguousarray(w_t.reshape(kt, part, M).transpose(1, 0, 2))


def _pack_bias(b, part=128):
    n = b.shape[0]
    t = n // part
    return np.ascontiguousarray(b.reshape(t, part).T)  # [part, t]


def _make_selftpl():
    """ixbuf init template [128, 8, 10] uint16: col0 = self node id, col9 =
    0xFFFF (-1 as int16: ignored by local_scatter), cols 1..8 overwritten by
    find_index8."""
    tpl = np.zeros((128, 8, 10), np.uint16)
    for I in range(8):
        tpl[:, I, 0] = I * 128 + np.arange(128)
        tpl[:, I, 9] = 0xFFFF
    return np.ascontiguousarray(tpl.reshape(128, 80))


def _prep_weights(inp):
    f16 = np.float16
    s1, t1 = _bn_fold(inp['g1_bn'])
    Wg1 = s1[:, None] * inp['g1_w']
    s2, t2 = _bn_fold(inp['g2_bn'])
    Wg2 = s2[:, None] * inp['g2_w']
    sf1, tf1 = _bn_fold(inp['f1_bn'])
    Wf1 = sf1[:, None] * inp['f1_w']
    bf1 = sf1 * inp['f1_b'] + tf1
    sf2, tf2 = _bn_fold(inp['f2_bn'])
    Wf2 = sf2[:, None] * inp['f2_w']
    bf2 = sf2 * inp['f2_b'] + tf2
    sb1, tb1 = _bn_fold(inp['b1_bn'])
    Wb1 = sb1[:, None] * inp['b1_w']
    sb2, tb2 = _bn_fold(inp['b2_bn'])
    Wb2 = sb2[:, None, None, None] * inp['b2_w']
    sb3, tb3 = _bn_fold(inp['b3_bn'])
    Wb3 = sb3[:, None] * inp['b3_w']
    sf, tf = _bn_fold(inp['bnf'])

    A = inp['edge_w'][:, :C]
    Bm = inp['edge_w'][:, C:]
    Wp = A - Bm
    Wq = Bm
    bp = inp['edge_b'] + C_LSE          # LSE shift folded into the p bias

    wb2_t = np.zeros((64, 9, 64), f16)
    for dy in range(3):
        for dx in range(3):
            wb2_t[:, dy * 3 + dx, :] = Wb2[:, :, dy, dx].T.astype(f16)

    return {
        'wg1': _pack_kxm(Wg1.T.astype(f16)),                # [128,2,256]
        'wp': _pack_kxm(Wp.T.astype(f16)),                  # [128,2,512]
        'wq': _pack_kxm((T_LSE * Wq).T.astype(f16)),        # [128,2,512] (t*Wq)
        'wg2': _pack_kxm(Wg2.T.astype(f16)),                # [128,4,256]
        'wf1': _pack_kxm(Wf1.T.astype(f16)),                # [128,2,1024]
        'wf2': _pack_kxm(Wf2.T.astype(f16)),                # [128,8,256]
        'wb1': _pack_kxm(Wb1.T.astype(f16)),                # [128,2,64]
        'wb2': wb2_t,                                        # [64,9,64]
        'wb3': Wb3.T.astype(f16),                            # [64,256]
        'bt1': _pack_bias(t1),                               # [128,2] f32
        'bt2': _pack_bias(t2),
        'bbp': _pack_bias(bp),                               # [128,4]
        'bbf1': _pack_bias(bf1),                             # [128,8]
        'bbf2': _pack_bias(bf2),
        'btb1': np.ascontiguousarray(tb1[:, None].astype(np.float32)),  # [64,1]
        'btb2': np.ascontiguousarray(tb2[:, None].astype(np.float32)),
        'btb3': _pack_bias(tb3),
        'bsf': _pack_bias(sf),
        'btf': _pack_bias(tf),
        'expb': np.full((128, 1), -T_LSE * C_LSE, np.float32),
        'lnb': np.full((128, 1), 1e-30, np.float32),
        'selftpl': _make_selftpl(),                          # [128,80] u16
    }


# --------------------------------------------------------------------------
# device kernel builder
# --------------------------------------------------------------------------
def _build_bass():
    import concourse.bass as bass
    import concourse.mybir as mybir
    from concourse import bacc
    from concourse.tile import TileContext
    from concourse.masks import make_identity

    dt = mybir.dt
    F16 = dt.float16
    BF16 = dt.bfloat16
    F32 = dt.float32
    AF = mybir.ActivationFunctionType

    nc = bacc.Bacc()

    # ---- DRAM parameters ----
    x_d = nc.declare_dram_parameter("x", [IMGS_PER_CORE, C, N], F32, isOutput=False)
    wg1_d = nc.declare_dram_parameter("wg1", [128, 2, 256], F16, isOutput=False)
    wp_d = nc.declare_dram_parameter("wp", [128, 2, 512], F16, isOutput=False)
    wq_d = nc.declare_dram_parameter("wq", [128, 2, 512], F16, isOutput=False)
    wg2_d = nc.declare_dram_parameter("wg2", [128, 4, 256], F16, isOutput=False)
    wf1_d = nc.declare_dram_parameter("wf1", [128, 2, 1024], F16, isOutput=False)
    wf2_d = nc.declare_dram_parameter("wf2", [128, 8, 256], F16, isOutput=False)
    wb1_d = nc.declare_dram_parameter("wb1", [128, 2, 64], F16, isOutput=False)
    wb2_d = nc.declare_dram_parameter("wb2", [64, 9, 64], F16, isOutput=False)
    wb3_d = nc.declare_dram_parameter("wb3", [64, 256], F16, isOutput=False)
    bt1_d = nc.declare_dram_parameter("bt1", [128, 2], F32, isOutput=False)
    bt2_d = nc.declare_dram_parameter("bt2", [128, 2], F32, isOutput=False)
    bbp_d = nc.declare_dram_parameter("bbp", [128, 4], F32, isOutput=False)
    bbf1_d = nc.declare_dram_parameter("bbf1", [128, 8], F32, isOutput=False)
    bbf2_d = nc.declare_dram_parameter("bbf2", [128, 2], F32, isOutput=False)
    btb1_d = nc.declare_dram_parameter("btb1", [64, 1], F32, isOutput=False)
    btb2_d = nc.declare_dram_parameter("btb2", [64, 1], F32, isOutput=False)
    btb3_d = nc.declare_dram_parameter("btb3", [128, 2], F32, isOutput=False)
    bsf_d = nc.declare_dram_parameter("bsf", [128, 2], F32, isOutput=False)
    btf_d = nc.declare_dram_parameter("btf", [128, 2], F32, isOutput=False)
    expb_d = nc.declare_dram_parameter("expb", [128, 1], F32, isOutput=False)
    lnb_d = nc.declare_dram_parameter("lnb", [128, 1], F32, isOutput=False)
    selftpl_d = nc.declare_dram_parameter("selftpl", [128, 80], dt.uint16,
                                          isOutput=False)
    out_d = nc.declare_dram_parameter("out", [IMGS_PER_CORE, C, N], F32,
                                      isOutput=True)
    dbg = bool(os.environ.get("KBENCH_DEBUG"))
    if dbg:
        dbg_ix = nc.declare_dram_parameter("dbg_ix", [128, 80], dt.uint16,
                                           isOutput=True)
        dbg_S = nc.declare_dram_parameter("dbg_S", [128, N], BF16, isOutput=True)
        dbg_STt = nc.declare_dram_parameter("dbg_STt", [128, 8, 8, 128], BF16,
                                            isOutput=True)
        dbg_e = nc.declare_dram_parameter("dbg_e", [128, 4, N], F16,
                                          isOutput=True)
        dbg_expq = nc.declare_dram_parameter("dbg_expq", [128, 4, N], BF16,
                                             isOutput=True)
        dbg_lnq = nc.declare_dram_parameter("dbg_lnq", [128, 4, N], F16,
                                            isOutput=True)
        dbg_feat = nc.declare_dram_parameter("dbg_feat", [128, 2, N], F16,
                                             isOutput=True)

    with TileContext(nc) as tc:
        import contextlib
        ctx = contextlib.ExitStack()
        with ctx:
            consts = ctx.enter_context(tc.tile_pool(name="consts", bufs=1))
            p_xc = ctx.enter_context(tc.tile_pool(name="xc", bufs=2))
            p_feat = ctx.enter_context(tc.tile_pool(name="feat", bufs=2))
            p_xn = ctx.enter_context(tc.tile_pool(name="xn", bufs=2))
            p_sc = ctx.enter_context(tc.tile_pool(name="sc", bufs=3))
            p_qe = ctx.enter_context(tc.tile_pool(name="qe", bufs=2))
            p_S = ctx.enter_context(tc.tile_pool(name="S", bufs=2))
            p_STt = ctx.enter_context(tc.tile_pool(name="STt", bufs=2))
            p_lnq = ctx.enter_context(tc.tile_pool(name="lnq", bufs=2))
            p_p = ctx.enter_context(tc.tile_pool(name="p", bufs=2))
            p_h = ctx.enter_context(tc.tile_pool(name="h", bufs=4))
            p_f1o = ctx.enter_context(tc.tile_pool(name="f1o", bufs=1))
            p_b = ctx.enter_context(tc.tile_pool(name="b", bufs=3))
            p_out = ctx.enter_context(tc.tile_pool(name="out", bufs=2))
            p_ix = ctx.enter_context(tc.tile_pool(name="ix", bufs=2))
            p_mx = ctx.enter_context(tc.tile_pool(name="mx", bufs=2))
            ps1024 = ctx.enter_context(
                tc.tile_pool(name="ps1024", bufs=3, space="PSUM"))
            psmall = ctx.enter_context(
                tc.tile_pool(name="psmall", bufs=2, space="PSUM"))

            # ---- constants / weights (loaded once) ----
            def load(name, shape, dtype, src):
                t = consts.tile(shape, dtype, name=name)
                nc.sync.dma_start(out=t[:], in_=src[:])
                return t

            wg1 = load("wg1s", [128, 2, 256], F16, wg1_d)
            wp = load("wps", [128, 2, 512], F16, wp_d)
            wq = load("wqs", [128, 2, 512], F16, wq_d)
            wg2 = load("wg2s", [128, 4, 256], F16, wg2_d)
            wf1 = load("wf1s", [128, 2, 1024], F16, wf1_d)
            wf2 = load("wf2s", [128, 8, 256], F16, wf2_d)
            wb1 = load("wb1s", [128, 2, 64], F16, wb1_d)
            wb2 = load("wb2s", [64, 9, 64], F16, wb2_d)
            wb3 = load("wb3s", [64, 256], F16, wb3_d)
            bt1 = load("bt1s", [128, 2], F32, bt1_d)
            bt2 = load("bt2s", [128, 2], F32, bt2_d)
            bbp = load("bbps", [128, 4], F32, bbp_d)
            bbf1 = load("bbf1s", [128, 8], F32, bbf1_d)
            bbf2 = load("bbf2s", [128, 2], F32, bbf2_d)
            btb1 = load("btb1s", [64, 1], F32, btb1_d)
            btb2 = load("btb2s", [64, 1], F32, btb2_d)
            btb3 = load("btb3s", [128, 2], F32, btb3_d)
            bsf = load("bsfs", [128, 2], F32, bsf_d)
            btf = load("btfs", [128, 2], F32, btf_d)
            expb = load("expbs", [128, 1], F32, expb_d)
            lnb = load("lnbs", [128, 1], F32, lnb_d)
            selftpl = load("selftpls", [128, 80], dt.uint16, selftpl_d)

            ident = consts.tile([128, 128], F16, name="ident")
            make_identity(nc, ident[:])
            negid = consts.tile([128, 128], F16, name="negid")
            nc.scalar.activation(out=negid[:], in_=ident[:], func=AF.Copy,
                                 scale=NEG_BIG)
            ones = consts.tile([128, 128], F16, name="ones")
            nc.gpsimd.memset(ones[:], 1.0)
            onesk = consts.tile([128, 16], BF16, name="onesk")
            nc.gpsimd.memset(onesk[:], 1.0)
            # idbig[k, f] = 1 iff f == k + 384 (shifted identity for diag-kill)
            idbig = consts.tile([128, 1024], F16, name="idbig")
            nc.gpsimd.memset(idbig[:], 0.0)
            nc.gpsimd.affine_select(
                out=idbig[:], in_=idbig[:],
                compare_op=mybir.AluOpType.not_equal, fill=1.0,
                base=384, pattern=[[-1, 1024]], channel_multiplier=1)

            # per-image state carried from head to tail
            st = [{} for _ in range(IMGS_PER_CORE)]

            # ============== HEAD: load, g1, norms, sim/top8/S, q, p =======
            def head(img):
                s = st[img]
                xc = p_xc.tile([128, 2, N], F16, name="xc")
                for t in range(2):
                    # cast f32->f16 during DMA (SWDGE)
                    nc.gpsimd.dma_start(out=xc[:, t, :],
                                        in_=x_d[img, t * 128:(t + 1) * 128, :])
                s['xc'] = xc

                # ---- g1: featT [128, 2, N] f16 ----
                # (kt-outer loops everywhere: one LDWEIGHTS serves both
                # nb-halves, so matmuls stream back-to-back)
                featT = p_feat.tile([128, 2, N], F16, name="featT")
                for to in range(2):
                    ps = ps1024.tile([128, N], F32, name="ps_g1", tag="ps")
                    for kt in range(2):
                        for nb in range(2):
                            nc.tensor.matmul(
                                ps[:, nb * 512:(nb + 1) * 512],
                                lhsT=wg1[:, kt, to * 128:(to + 1) * 128],
                                rhs=xc[:, kt, nb * 512:(nb + 1) * 512],
                                start=(kt == 0), stop=(kt == 1))
                    nc.scalar.activation(out=featT[:, to, :], in_=ps[:],
                                         func=AF.Identity, bias=bt1[:, to:to + 1])

                # ---- row norms first: the rsqrt ACT-table swap and the n2
                # matmuls run while the PE then chews q/p, so invnb is ready
                # by the time the bcast matmul needs it ----
                fsq = p_sc.tile([128, 2, N], F16, name="fsq", tag="sc")
                nc.vector.tensor_mul(fsq[:], featT[:], featT[:])
                invn = p_mx.tile([1, N], F16, name="invn", tag="invn")
                for nb in range(2):
                    ps1 = psmall.tile([1, 512], F32, name="ps_n2")
                    for kt in range(2):
                        nc.tensor.matmul(
                            ps1[:], lhsT=ones[:, 0:1],
                            rhs=fsq[:, kt, nb * 512:(nb + 1) * 512],
                            start=(kt == 0), stop=(kt == 1))
                    # rank-only use; the gated-accuracy LUT is fine here
                    nc.scalar.activation(out=invn[:, nb * 512:(nb + 1) * 512],
                                         in_=ps1[:], func=AF.Abs_reciprocal_sqrt)

                # ---- q -> expq (t*Wq folded; exp bias = -t*c) ----
                expq = p_qe.tile([128, 4, N], BF16, name="expq", tag="qe")
                for pair in range(4):
                    ps = ps1024.tile([128, N], F32, name="ps_q", tag="ps")
                    for sub in range(2):
                        nt = 2 * pair + sub
                        for kt in range(2):
                            nc.tensor.matmul(
                                ps[:, sub * 512:(sub + 1) * 512],
                                lhsT=featT[:, kt, nt * 128:(nt + 1) * 128],
                                rhs=wq[:, kt, :], start=(kt == 0), stop=(kt == 1))
                    nc.scalar.activation(out=expq[:, pair, :], in_=ps[:],
                                         func=AF.Exp, bias=expb[:, 0:1])
                if dbg and img == 0:
                    nc.sync.dma_start(out=dbg_expq[:], in_=expq[:])
                s['expq'] = expq

                # ---- p^T [128, 4, N] f16 (ch-part, bias + c folded) ----
                pT = p_p.tile([128, 4, N], F16, name="pT")
                for to in range(4):
                    ps = ps1024.tile([128, N], F32, name="ps_p", tag="ps")
                    for kt in range(2):
                        for nb in range(2):
                            nc.tensor.matmul(
                                ps[:, nb * 512:(nb + 1) * 512],
                                lhsT=wp[:, kt, to * 128:(to + 1) * 128],
                                rhs=featT[:, kt, nb * 512:(nb + 1) * 512],
                                start=(kt == 0), stop=(kt == 1))
                    # split PSUM drains between ACT and DVE: the PE fills a
                    # two-bank tile in ~0.9-1.8us; one ACT alone (1.3us/tile)
                    # is the pipeline bottleneck
                    if to < 2:
                        nc.scalar.activation(out=pT[:, to, :], in_=ps[:],
                                             func=AF.Identity,
                                             bias=bbp[:, to:to + 1])
                    else:
                        nc.vector.tensor_scalar(
                            out=pT[:, to, :], in0=ps[:],
                            scalar1=bbp[:, to:to + 1], scalar2=None,
                            op0=mybir.AluOpType.add)
                s['pT'] = pT

                # ---- broadcast invn -> xnT (rhs-side normalized) ----
                invnb = p_sc.tile([128, N], F16, name="invnb", tag="sc")
                psb = ps1024.tile([128, N], F32, name="ps_bc", tag="ps")
                for nb in range(2):
                    nc.tensor.matmul(psb[:, nb * 512:(nb + 1) * 512],
                                     lhsT=ones[0:1, :],
                                     rhs=invn[:, nb * 512:(nb + 1) * 512],
                                     start=True, stop=True)
                nc.scalar.activation(out=invnb[:], in_=psb[:], func=AF.Copy)
                xnT = p_xn.tile([128, 2, N], F16, name="xnT")
                for t in range(2):
                    nc.vector.tensor_mul(xnT[:, t, :], featT[:, t, :], invnb[:])

                # ---- sim + top8 + S + S^T, per 128-node block ----
                ixbuf = p_ix.tile([128, 8, 10], dt.uint16, name="ixbuf")
                nc.vector.tensor_copy(
                    ixbuf[:].rearrange("p a b -> p (a b)"), selftpl[:])
                STt = p_STt.tile([128, 8, 8, 128], BF16, name="STt")
                s['STt'] = STt
                for I in range(8):
                    ps = ps1024.tile([128, N], F32, name="ps_sim", tag="ps")
                    dcb = I // 4
                    for kt in range(2):
                        for cb in range(2):
                            nc.tensor.matmul(
                                ps[:, cb * 512:(cb + 1) * 512],
                                lhsT=featT[:, kt, I * 128:(I + 1) * 128],
                                rhs=xnT[:, kt, cb * 512:(cb + 1) * 512],
                                start=(kt == 0),
                                stop=(kt == 1 and cb != dcb))
                    off2 = (I % 4) * 128
                    nc.tensor.matmul(
                        ps[:, dcb * 512:(dcb + 1) * 512], lhsT=negid[:],
                        rhs=idbig[:, 384 - off2:896 - off2],
                        start=False, stop=True)
                    # mx must be f32: find_index8 matches exact values, so
                    # in_max and in_values (PSUM f32) must share precision
                    mx = p_mx.tile([128, 8], F32, name="mx", tag="mx")
                    nc.vector.max(out=mx[:], in_=ps[:])
                    nc.vector.max_index(out=ixbuf[:, I, 1:9], in_max=mx[:],
                                        in_values=ps[:])
                    S_I = p_S.tile([128, N], BF16, name="S_I")
                    nc.gpsimd.local_scatter(
                        out_ap=S_I[:], data_ap=onesk[:, 0:10],
                        idxs_ap=ixbuf[:, I, :].bitcast(dt.int16),
                        channels=128, num_elems=N, num_idxs=10)
                    nc.sync.dma_start_transpose(out=STt[:, I], in_=S_I[:])
                    if dbg and img == 0 and I == 0:
                        nc.sync.dma_start(out=dbg_S[:], in_=S_I[:])
                if dbg and img == 0:
                    nc.sync.dma_start(out=dbg_STt[:], in_=STt[:])
                    nc.sync.dma_start(out=dbg_ix[:],
                                      in_=ixbuf[:].rearrange("p a b -> p (a b)"))
                    nc.sync.dma_start(out=dbg_feat[:], in_=featT[:])

            # ============== TAIL phase 1: agg + e =========================
            def agg_phase(img):
                s = st[img]
                expq, pT, STt = s['expq'], s['pT'], s['STt']

                # ---- agg: lnqT [128, 4, N] f16 = ln(expq^T @ S^T) ----
                # lnqT holds ln(agg)/t, computed on the DVE from the f32
                # exponent bits (the ACT Ln LUT clamps below ~2^-66, which
                # floors 20% of entries)
                lnqT = p_lnq.tile([128, 4, N], F16, name="lnqT")
                for cb in range(4):
                    ps = ps1024.tile([128, N], F32, name="ps_agg", tag="ps")
                    for kt in range(8):
                        for half in range(2):
                            nc.tensor.matmul(
                                ps[:, half * 512:(half + 1) * 512],
                                lhsT=expq[:, kt // 2,
                                          (kt % 2) * 512 + cb * 128:
                                          (kt % 2) * 512 + cb * 128 + 128],
                                rhs=STt[:, half * 4:(half + 1) * 4, kt, :],
                                start=(kt == 0), stop=(kt == 7))
                    nc.vector.tensor_scalar(
                        out=lnqT[:, cb, :], in0=ps[:].bitcast(dt.int32),
                        scalar1=LN_ALPHA, scalar2=LN_BETA,
                        op0=mybir.AluOpType.mult, op1=mybir.AluOpType.add)

                if dbg and img == 0:
                    nc.sync.dma_start(out=dbg_lnq[:], in_=lnqT[:])
                # ---- e = relu(pT + lnqT) (lnqT already scaled by 1/t) ----
                eT = p_qe.tile([128, 4, N], F16, name="eT", tag="qe")
                nc.vector.tensor_add(eT[:], lnqT[:], pT[:])
                nc.vector.tensor_scalar_max(eT[:], eT[:], 0.0)
                if dbg and img == 0:
                    nc.sync.dma_start(out=dbg_e[:], in_=eT[:])
                s['eT'] = eT

            # ============== TAIL phase 2: g2 + FFN ========================
            def ffn_phase(img):
                s = st[img]
                xc, eT = s['xc'], s['eT']

                # ---- g2 + residual -> h f16 ----
                h = p_h.tile([128, 2, N], F16, name="h", tag="h")
                for to in range(2):
                    ps = ps1024.tile([128, N], F32, name="ps_g2", tag="ps")
                    for kt in range(4):
                        for nb in range(2):
                            nc.tensor.matmul(
                                ps[:, nb * 512:(nb + 1) * 512],
                                lhsT=wg2[:, kt, to * 128:(to + 1) * 128],
                                rhs=eT[:, kt, nb * 512:(nb + 1) * 512],
                                start=(kt == 0), stop=(kt == 3))
                    tmp = p_sc.tile([128, N], F32, name="g2tmp", tag="sc")
                    nc.scalar.activation(out=tmp[:], in_=ps[:],
                                         func=AF.Identity, bias=bt2[:, to:to + 1])
                    nc.vector.tensor_add(h[:, to, :], tmp[:], xc[:, to, :])

                # ---- FFN ----
                f1o = p_f1o.tile([128, 8, N], F16, name="f1o")
                for to in range(8):
                    ps = ps1024.tile([128, N], F32, name="ps_f1", tag="ps")
                    for kt in range(2):
                        for nb in range(2):
                            nc.tensor.matmul(
                                ps[:, nb * 512:(nb + 1) * 512],
                                lhsT=wf1[:, kt, to * 128:(to + 1) * 128],
                                rhs=h[:, kt, nb * 512:(nb + 1) * 512],
                                start=(kt == 0), stop=(kt == 1))
                    if to % 2 == 0:
                        nc.scalar.activation(out=f1o[:, to, :], in_=ps[:],
                                             func=AF.Relu, bias=bbf1[:, to:to + 1])
                    else:
                        nc.vector.tensor_scalar(
                            out=f1o[:, to, :], in0=ps[:],
                            scalar1=bbf1[:, to:to + 1], scalar2=0.0,
                            op0=mybir.AluOpType.add, op1=mybir.AluOpType.max)
                h2 = p_h.tile([128, 2, N], F16, name="h2", tag="h")
                for to in range(2):
                    ps = ps1024.tile([128, N], F32, name="ps_f2", tag="ps")
                    for kt in range(8):
                        for nb in range(2):
                            nc.tensor.matmul(
                                ps[:, nb * 512:(nb + 1) * 512],
                                lhsT=wf2[:, kt, to * 128:(to + 1) * 128],
                                rhs=f1o[:, kt, nb * 512:(nb + 1) * 512],
                                start=(kt == 0), stop=(kt == 7))
                    tmp = p_sc.tile([128, N], F32, name="f2tmp", tag="sc")
                    nc.scalar.activation(out=tmp[:], in_=ps[:],
                                         func=AF.Identity, bias=bbf2[:, to:to + 1])
                    nc.vector.tensor_add(h2[:, to, :], tmp[:], h[:, to, :])
                s['h2'] = h2

            # ============== TAIL phase 3: bottleneck + out ================
            def bott_phase(img):
                s = st[img]
                xc, h2 = s['xc'], s['h2']

                b1o = p_b.tile([64, N], F16, name="b1o", tag="b")
                psb1 = ps1024.tile([128, N], F32, name="ps_b1", tag="ps")
                for kt in range(2):
                    for nb in range(2):
                        nc.tensor.matmul(
                            psb1[0:64, nb * 512:(nb + 1) * 512],
                            lhsT=wb1[:, kt, :],
                            rhs=h2[:, kt, nb * 512:(nb + 1) * 512],
                            start=(kt == 0), stop=(kt == 1))
                nc.scalar.activation(out=b1o[:], in_=psb1[0:64, :],
                                     func=AF.Relu, bias=btb1[:, 0:1])
                pad = p_b.tile([64, 34 * 34], F16, name="pad", tag="b")
                nc.vector.memset(pad[:], 0.0)
                pad3 = pad[:].rearrange("p (r c) -> p r c", r=34)
                b1v = b1o[:].rearrange("p (r c) -> p r c", r=32)
                nc.vector.tensor_copy(pad3[:, 1:33, 1:33], b1v)
                b2o = p_b.tile([64, N], F16, name="b2o", tag="b")
                psb2 = ps1024.tile([128, N], F32, name="ps_b2", tag="ps")
                for tap in range(9):
                    dy, dx = tap // 3, tap % 3
                    for nb in range(2):
                        rhs = pad3[:, 16 * nb + dy:16 * nb + dy + 16, dx:dx + 32]
                        nc.tensor.matmul(psb2[0:64, nb * 512:(nb + 1) * 512],
                                         lhsT=wb2[:, tap, :], rhs=rhs,
                                         start=(tap == 0), stop=(tap == 8))
                nc.scalar.activation(out=b2o[:], in_=psb2[0:64, :],
                                     func=AF.Relu, bias=btb2[:, 0:1])
                b3o = p_sc.tile([128, 2, N], F16, name="b3o", tag="sc")
                for to in range(2):
                    ps = ps1024.tile([128, N], F32, name="ps_b3", tag="ps")
                    for nb in range(2):
                        nc.tensor.matmul(
                            ps[:, nb * 512:(nb + 1) * 512],
                            lhsT=wb3[:, to * 128:(to + 1) * 128],
                            rhs=b2o[:, nb * 512:(nb + 1) * 512],
                            start=True, stop=True)
                    nc.scalar.activation(out=b3o[:, to, :], in_=ps[:],
                                         func=AF.Identity, bias=btb3[:, to:to + 1])

                # ---- o3 = b3o + h2; fin = o3 + x; out = sf*fin + tf ----
                o3 = p_h.tile([128, 2, N], F16, name="o3", tag="h")
                nc.vector.tensor_add(o3[:], b3o[:], h2[:])
                fin = p_lnq.tile([128, 2, N], F16, name="fin", tag="fin")
                nc.vector.tensor_add(fin[:], o3[:], xc[:])
                for t in range(2):
                    out32 = p_out.tile([128, N], F32, name="out32")
                    nc.scalar.activation(out=out32[:], in_=fin[:, t, :],
                                         func=AF.Identity, scale=bsf[:, t:t + 1],
                                         bias=btf[:, t:t + 1])
                    nc.sync.dma_start(out=out_d[img, t * 128:(t + 1) * 128, :],
                                      in_=out32[:])

            for img in range(IMGS_PER_CORE):
                head(img)
            for img in range(IMGS_PER_CORE):
                agg_phase(img)
            for img in range(IMGS_PER_CORE):
                ffn_phase(img)
            for img in range(IMGS_PER_CORE):
                bott_phase(img)

    nc.finalize()
    return nc


# --------------------------------------------------------------------------
# entry point
# --------------------------------------------------------------------------
def kernel(**inputs):
    inp = {k: np.asarray(v) for k, v in inputs.items()}
    w = _prep_weights(inp)

    if 'nc' not in _cache:
        _cache['nc'] = _build_bass()
    nc = _cache['nc']

    x = inp['x'].astype(np.float32).reshape(B, C, N)
    in_maps = []
    for c in range(N_CORES):
        m = {'x': np.ascontiguousarray(x[c * 2:(c + 1) * 2])}
        m.update({k: v for k, v in w.items()})
        in_maps.append(m)

    from concourse.bass_utils import run_bass_kernel_spmd
    trace = bool(os.environ.get("KBENCH_TRACE"))
    res = run_bass_kernel_spmd(nc, in_maps, core_ids=list(range(N_CORES)),
                               trace=trace)
    _cache['exec_time_ns'] = res.exec_time_ns
    _cache['results'] = res
    out = np.zeros((B, C, N), np.float32)
    for c in range(N_CORES):
        out[c * 2:(c + 1) * 2] = res.results[c]['out']
    return out.reshape(B, C, H, W)


# revision 46
# speedup vs baseline: 2.5382x; 1.0008x over previous
"""Trainium2 Bass kernel for nn_Block_17033840296551 (GNN message passing block).

Data-parallel over batch: 16 images -> 8 cores x 2 images. Each core runs the
full block (g1 conv -> kNN top-9 -> EdgeConv max-agg -> g2 -> FFN -> bottleneck
-> final BN) on its 2 images with no cross-core communication.

v2 design (LSE EdgeConv — no neighbor gather):
  * All BNs folded into conv weights/biases on host.
  * EdgeConv decomposed: e[n,k] = p[n] + q[idx[n,k]], p = (Wa-Wb)@feat + b,
    q = Wb@feat; max_k relu(e) = relu(p + max_k q).
  * max_k q replaced by log-sum-exp: max_k q ~= c + ln(sum_k exp(t(q_k-c)))/t
    with t=30, c=2.0. The sum over the 9-hot neighbor set is a matmul
    S^T.T @ expq on the PE (S built by gpsimd local_scatter from the top-8
    indices; S^T via tiled xbar dma transpose). Kills the descriptor-
    generation-bound dma_gather (~160us/core) plus the DVE max-fold tree.
  * kNN: scores s[n,m] = <feat_n, feat_m/||feat_m||> rank-equivalent to cosine
    per row; self removed via -BIG diagonal (extra identity matmul into PSUM);
    DVE Max8/FindIndex8 read scores straight from PSUM (no SBUF sim buffer).
  * f16 matmul inputs (bf16 for the exp path: f16 overflows at e^11), f32
    PSUM, f16 residual stream, [128,1024] two-bank PSUM drains.
  * Two-phase emission (head: g1..sim..sel..q..p, tail: agg..FFN..bottleneck)
    interleaved across the 2 images so PE/DVE/ACT/DMA overlap.
"""

import os
import numpy as np

# problem constants (hardcoded per harness contract)
B, C, H, W = 16, 256, 32, 32
N = H * W           # 1024 pixels per image
K = 9
EPS = 1e-5
IMGS_PER_CORE = 2
N_CORES = 8
NEG_BIG = -30000.0
T_LSE = 30.0
C_LSE = 2.0
LN2 = 0.6931471805599453
# ln(x) ~= LN2 * (bitcast_int32(x) * 2^-23 - 126.957)  (max err ~0.03 in ln)
LN_ALPHA = LN2 / T_LSE / (1 << 23)
LN_BETA = -126.957 * LN2 / T_LSE

_cache = {}


# --------------------------------------------------------------------------
# host-side preprocessing
# --------------------------------------------------------------------------
def _bn_fold(p):
    g, b, m, v = np.asarray(p, np.float32)
    s = g / np.sqrt(v + EPS)
    t = b - m * s
    return s, t


def _pack_kxm(w_t, part=128):
    """[K, M] -> [part, K//part, M] (partition-major K tiling)."""
    Kd, M = w_t.shape
    kt = Kd // part
    return np.ascontiguousarray(w_t.reshape(kt, part, M).transpose(1, 0, 2))


def _pack_bias(b, part=128):
    n = b.shape[0]
    t = n // part
    return np.ascontiguousarray(b.reshape(t, part).T)  # [part, t]


def _make_selftpl():
    """ixbuf init template [128, 8, 10] uint16: col0 = self node id, col9 =
    0xFFFF (-1 as int16: ignored by local_scatter), cols 1..8 overwritten by
    find_index8."""
    tpl = np.zeros((128, 8, 10), np.uint16)
    for I in range(8):
        tpl[:, I, 0] = I * 128 + np.arange(128)
        tpl[:, I, 9] = 0xFFFF
    return np.ascontiguousarray(tpl.reshape(128, 80))


def _prep_weights(inp):
    f16 = np.float16
    s1, t1 = _bn_fold(inp['g1_bn'])
    Wg1 = s1[:, None] * inp['g1_w']
    s2, t2 = _bn_fold(inp['g2_bn'])
    Wg2 = s2[:, None] * inp['g2_w']
    sf1, tf1 = _bn_fold(inp['f1_bn'])
    Wf1 = sf1[:, None] * inp['f1_w']
    bf1 = sf1 * inp['f1_b'] + tf1
    sf2, tf2 = _bn_fold(inp['f2_bn'])
    Wf2 = sf2[:, None] * inp['f2_w']
    bf2 = sf2 * inp['f2_b'] + tf2
    sb1, tb1 = _bn_fold(inp['b1_bn'])
    Wb1 = sb1[:, None] * inp['b1_w']
    sb2, tb2 = _bn_fold(inp['b2_bn'])
    Wb2 = sb2[:, None, None, None] * inp['b2_w']
    sb3, tb3 = _bn_fold(inp['b3_bn'])
    Wb3 = sb3[:, None] * inp['b3_w']
    sf, tf = _bn_fold(inp['bnf'])

    A = inp['edge_w'][:, :C]
    Bm = inp['edge_w'][:, C:]
    Wp = A - Bm
    Wq = Bm
    bp = inp['edge_b'] + C_LSE          # LSE shift folded into the p bias

    wb2_t = np.zeros((64, 9, 64), f16)
    for dy in range(3):
        for dx in range(3):
            wb2_t[:, dy * 3 + dx, :] = Wb2[:, :, dy, dx].T.astype(f16)

    return {
        'wg1': _pack_kxm(Wg1.T.astype(f16)),                # [128,2,256]
        'wp': _pack_kxm(Wp.T.astype(f16)),                  # [128,2,512]
        'wq': _pack_kxm((T_LSE * Wq).T.astype(f16)),        # [128,2,512] (t*Wq)
        'wg2': _pack_kxm(Wg2.T.astype(f16)),                # [128,4,256]
        'wf1': _pack_kxm(Wf1.T.astype(f16)),                # [128,2,1024]
        'wf2': _pack_kxm(Wf2.T.astype(f16)),                # [128,8,256]
        'wb1': _pack_kxm(Wb1.T.astype(f16)),                # [128,2,64]
        'wb2': wb2_t,                                        # [64,9,64]
        'wb3': Wb3.T.astype(f16),                            # [64,256]
        'bt1': _pack_bias(t1),                               # [128,2] f32
        'bt2': _pack_bias(t2),
        'bbp': _pack_bias(bp),                               # [128,4]
        'bbf1': _pack_bias(bf1),                             # [128,8]
        'bbf2': _pack_bias(bf2),
        'btb1': np.ascontiguousarray(tb1[:, None].astype(np.float32)),  # [64,1]
        'btb2': np.ascontiguousarray(tb2[:, None].astype(np.float32)),
        'btb3': _pack_bias(tb3),
        'bsf': _pack_bias(sf),
        'btf': _pack_bias(tf),
        'expb': np.full((128, 1), -T_LSE * C_LSE, np.float32),
        'lnb': np.full((128, 1), 1e-30, np.float32),
        'selftpl': _make_selftpl(),                          # [128,80] u16
    }


# --------------------------------------------------------------------------
# device kernel builder
# --------------------------------------------------------------------------
def _build_bass():
    import concourse.bass as bass
    import concourse.mybir as mybir
    from concourse import bacc
    from concourse.tile import TileContext
    from concourse.masks import make_identity

    dt = mybir.dt
    F16 = dt.float16
    BF16 = dt.bfloat16
    F32 = dt.float32
    AF = mybir.ActivationFunctionType

    nc = bacc.Bacc()

    # ---- DRAM parameters ----
    x_d = nc.declare_dram_parameter("x", [IMGS_PER_CORE, C, N], F32, isOutput=False)
    wg1_d = nc.declare_dram_parameter("wg1", [128, 2, 256], F16, isOutput=False)
    wp_d = nc.declare_dram_parameter("wp", [128, 2, 512], F16, isOutput=False)
    wq_d = nc.declare_dram_parameter("wq", [128, 2, 512], F16, isOutput=False)
    wg2_d = nc.declare_dram_parameter("wg2", [128, 4, 256], F16, isOutput=False)
    wf1_d = nc.declare_dram_parameter("wf1", [128, 2, 1024], F16, isOutput=False)
    wf2_d = nc.declare_dram_parameter("wf2", [128, 8, 256], F16, isOutput=False)
    wb1_d = nc.declare_dram_parameter("wb1", [128, 2, 64], F16, isOutput=False)
    wb2_d = nc.declare_dram_parameter("wb2", [64, 9, 64], F16, isOutput=False)
    wb3_d = nc.declare_dram_parameter("wb3", [64, 256], F16, isOutput=False)
    bt1_d = nc.declare_dram_parameter("bt1", [128, 2], F32, isOutput=False)
    bt2_d = nc.declare_dram_parameter("bt2", [128, 2], F32, isOutput=False)
    bbp_d = nc.declare_dram_parameter("bbp", [128, 4], F32, isOutput=False)
    bbf1_d = nc.declare_dram_parameter("bbf1", [128, 8], F32, isOutput=False)
    bbf2_d = nc.declare_dram_parameter("bbf2", [128, 2], F32, isOutput=False)
    btb1_d = nc.declare_dram_parameter("btb1", [64, 1], F32, isOutput=False)
    btb2_d = nc.declare_dram_parameter("btb2", [64, 1], F32, isOutput=False)
    btb3_d = nc.declare_dram_parameter("btb3", [128, 2], F32, isOutput=False)
    bsf_d = nc.declare_dram_parameter("bsf", [128, 2], F32, isOutput=False)
    btf_d = nc.declare_dram_parameter("btf", [128, 2], F32, isOutput=False)
    expb_d = nc.declare_dram_parameter("expb", [128, 1], F32, isOutput=False)
    lnb_d = nc.declare_dram_parameter("lnb", [128, 1], F32, isOutput=False)
    selftpl_d = nc.declare_dram_parameter("selftpl", [128, 80], dt.uint16,
                                          isOutput=False)
    out_d = nc.declare_dram_parameter("out", [IMGS_PER_CORE, C, N], F32,
                                      isOutput=True)
    dbg = bool(os.environ.get("KBENCH_DEBUG"))
    if dbg:
        dbg_ix = nc.declare_dram_parameter("dbg_ix", [128, 80], dt.uint16,
                                           isOutput=True)
        dbg_S = nc.declare_dram_parameter("dbg_S", [128, N], BF16, isOutput=True)
        dbg_STt = nc.declare_dram_parameter("dbg_STt", [128, 8, 8, 128], BF16,
                                            isOutput=True)
        dbg_e = nc.declare_dram_parameter("dbg_e", [128, 4, N], F16,
                                          isOutput=True)
        dbg_expq = nc.declare_dram_parameter("dbg_expq", [128, 4, N], BF16,
                                             isOutput=True)
        dbg_lnq = nc.declare_dram_parameter("dbg_lnq", [128, 4, N], F16,
                                            isOutput=True)
        dbg_feat = nc.declare_dram_parameter("dbg_feat", [128, 2, N], F16,
                                             isOutput=True)

    with TileContext(nc) as tc:
        import contextlib
        ctx = contextlib.ExitStack()
        with ctx:
            consts = ctx.enter_context(tc.tile_pool(name="consts", bufs=1))
            p_xc = ctx.enter_context(tc.tile_pool(name="xc", bufs=2))
            p_feat = ctx.enter_context(tc.tile_pool(name="feat", bufs=2))
            p_xn = ctx.enter_context(tc.tile_pool(name="xn", bufs=2))
            p_sc = ctx.enter_context(tc.tile_pool(name="sc", bufs=3))
            p_qe = ctx.enter_context(tc.tile_pool(name="qe", bufs=2))
            p_S = ctx.enter_context(tc.tile_pool(name="S", bufs=2))
            p_STt = ctx.enter_context(tc.tile_pool(name="STt", bufs=2))
            p_lnq = ctx.enter_context(tc.tile_pool(name="lnq", bufs=2))
            p_p = ctx.enter_context(tc.tile_pool(name="p", bufs=2))
            p_h = ctx.enter_context(tc.tile_pool(name="h", bufs=4))
            p_f1o = ctx.enter_context(tc.tile_pool(name="f1o", bufs=1))
            p_b = ctx.enter_context(tc.tile_pool(name="b", bufs=3))
            p_out = ctx.enter_context(tc.tile_pool(name="out", bufs=2))
            p_ix = ctx.enter_context(tc.tile_pool(name="ix", bufs=2))
            p_mx = ctx.enter_context(tc.tile_pool(name="mx", bufs=2))
            p_simsb = ctx.enter_context(tc.tile_pool(name="simsb", bufs=2))
            ps1024 = ctx.enter_context(
                tc.tile_pool(name="ps1024", bufs=3, space="PSUM"))
            psmall = ctx.enter_context(
                tc.tile_pool(name="psmall", bufs=2, space="PSUM"))

            # ---- constants / weights (loaded once) ----
            # alternate the two HWDGE rings (sync/scalar) so the ~21 weight
            # loads don't serialize on one ring at startup
            _ld = [0]

            def load(name, shape, dtype, src):
                t = consts.tile(shape, dtype, name=name)
                eng = nc.sync if _ld[0] % 2 == 0 else nc.scalar
                _ld[0] += 1
                eng.dma_start(out=t[:], in_=src[:])
                return t

            wg1 = load("wg1s", [128, 2, 256], F16, wg1_d)
            wp = load("wps", [128, 2, 512], F16, wp_d)
            wq = load("wqs", [128, 2, 512], F16, wq_d)
            wg2 = load("wg2s", [128, 4, 256], F16, wg2_d)
            wf1 = load("wf1s", [128, 2, 1024], F16, wf1_d)
            wf2 = load("wf2s", [128, 8, 256], F16, wf2_d)
            wb1 = load("wb1s", [128, 2, 64], F16, wb1_d)
            wb2 = load("wb2s", [64, 9, 64], F16, wb2_d)
            wb3 = load("wb3s", [64, 256], F16, wb3_d)
            bt1 = load("bt1s", [128, 2], F32, bt1_d)
            bt2 = load("bt2s", [128, 2], F32, bt2_d)
            bbp = load("bbps", [128, 4], F32, bbp_d)
            bbf1 = load("bbf1s", [128, 8], F32, bbf1_d)
            bbf2 = load("bbf2s", [128, 2], F32, bbf2_d)
            btb1 = load("btb1s", [64, 1], F32, btb1_d)
            btb2 = load("btb2s", [64, 1], F32, btb2_d)
            btb3 = load("btb3s", [128, 2], F32, btb3_d)
            bsf = load("bsfs", [128, 2], F32, bsf_d)
            btf = load("btfs", [128, 2], F32, btf_d)
            expb = load("expbs", [128, 1], F32, expb_d)
            lnb = load("lnbs", [128, 1], F32, lnb_d)
            selftpl = load("selftpls", [128, 80], dt.uint16, selftpl_d)

            ident = consts.tile([128, 128], F16, name="ident")
            make_identity(nc, ident[:])
            negid = consts.tile([128, 128], F16, name="negid")
            nc.scalar.activation(out=negid[:], in_=ident[:], func=AF.Copy,
                                 scale=NEG_BIG)
            ones = consts.tile([128, 128], F16, name="ones")
            nc.gpsimd.memset(ones[:], 1.0)
            onesk = consts.tile([128, 16], BF16, name="onesk")
            nc.gpsimd.memset(onesk[:], 1.0)
            # idbig[k, f] = 1 iff f == k + 384 (shifted identity for diag-kill)
            idbig = consts.tile([128, 1024], F16, name="idbig")
            nc.gpsimd.memset(idbig[:], 0.0)
            nc.gpsimd.affine_select(
                out=idbig[:], in_=idbig[:],
                compare_op=mybir.AluOpType.not_equal, fill=1.0,
                base=384, pattern=[[-1, 1024]], channel_multiplier=1)

            # per-image state carried from head to tail
            st = [{} for _ in range(IMGS_PER_CORE)]

            # ============== HEAD: load, g1, norms, sim/top8/S, q, p =======
            def head(img):
                s = st[img]
                xc = p_xc.tile([128, 2, N], F16, name="xc")
                for t in range(2):
                    # cast f32->f16 during DMA (SWDGE)
                    nc.gpsimd.dma_start(out=xc[:, t, :],
                                        in_=x_d[img, t * 128:(t + 1) * 128, :])
                s['xc'] = xc

                # ---- g1: featT [128, 2, N] f16 ----
                # (kt-outer loops everywhere: one LDWEIGHTS serves both
                # nb-halves, so matmuls stream back-to-back)
                featT = p_feat.tile([128, 2, N], F16, name="featT")
                for to in range(2):
                    ps = ps1024.tile([128, N], F32, name="ps_g1", tag="ps")
                    for kt in range(2):
                        for nb in range(2):
                            nc.tensor.matmul(
                                ps[:, nb * 512:(nb + 1) * 512],
                                lhsT=wg1[:, kt, to * 128:(to + 1) * 128],
                                rhs=xc[:, kt, nb * 512:(nb + 1) * 512],
                                start=(kt == 0), stop=(kt == 1))
                    nc.scalar.activation(out=featT[:, to, :], in_=ps[:],
                                         func=AF.Identity, bias=bt1[:, to:to + 1])

                # ---- row norms first: the rsqrt ACT-table swap and the n2
                # matmuls run while the PE then chews q/p, so invnb is ready
                # by the time the bcast matmul needs it ----
                fsq = p_sc.tile([128, 2, N], F16, name="fsq", tag="sc")
                nc.vector.tensor_mul(fsq[:], featT[:], featT[:])
                invn = p_mx.tile([1, N], F16, name="invn", tag="invn")
                for nb in range(2):
                    ps1 = psmall.tile([1, 512], F32, name="ps_n2")
                    for kt in range(2):
                        nc.tensor.matmul(
                            ps1[:], lhsT=ones[:, 0:1],
                            rhs=fsq[:, kt, nb * 512:(nb + 1) * 512],
                            start=(kt == 0), stop=(kt == 1))
                    # rank-only use; the gated-accuracy LUT is fine here
                    nc.scalar.activation(out=invn[:, nb * 512:(nb + 1) * 512],
                                         in_=ps1[:], func=AF.Abs_reciprocal_sqrt)

                # ---- q -> expq (t*Wq folded; exp bias = -t*c) ----
                expq = p_qe.tile([128, 4, N], BF16, name="expq", tag="qe")
                for pair in range(4):
                    ps = ps1024.tile([128, N], F32, name="ps_q", tag="ps")
                    for sub in range(2):
                        nt = 2 * pair + sub
                        for kt in range(2):
                            nc.tensor.matmul(
                                ps[:, sub * 512:(sub + 1) * 512],
                                lhsT=featT[:, kt, nt * 128:(nt + 1) * 128],
                                rhs=wq[:, kt, :], start=(kt == 0), stop=(kt == 1))
                    nc.scalar.activation(out=expq[:, pair, :], in_=ps[:],
                                         func=AF.Exp, bias=expb[:, 0:1])
                if dbg and img == 0:
                    nc.sync.dma_start(out=dbg_expq[:], in_=expq[:])
                s['expq'] = expq

                # ---- p^T [128, 4, N] f16 (ch-part, bias + c folded) ----
                pT = p_p.tile([128, 4, N], F16, name="pT")
                for to in range(4):
                    ps = ps1024.tile([128, N], F32, name="ps_p", tag="ps")
                    for kt in range(2):
                        for nb in range(2):
                            nc.tensor.matmul(
                                ps[:, nb * 512:(nb + 1) * 512],
                                lhsT=wp[:, kt, to * 128:(to + 1) * 128],
                                rhs=featT[:, kt, nb * 512:(nb + 1) * 512],
                                start=(kt == 0), stop=(kt == 1))
                    # split PSUM drains between ACT and DVE: the PE fills a
                    # two-bank tile in ~0.9-1.8us; one ACT alone (1.3us/tile)
                    # is the pipeline bottleneck
                    if to < 2:
                        nc.scalar.activation(out=pT[:, to, :], in_=ps[:],
                                             func=AF.Identity,
                                             bias=bbp[:, to:to + 1])
                    else:
                        nc.vector.tensor_scalar(
                            out=pT[:, to, :], in0=ps[:],
                            scalar1=bbp[:, to:to + 1], scalar2=None,
                            op0=mybir.AluOpType.add)
                s['pT'] = pT

                # ---- broadcast invn -> xnT (rhs-side normalized) ----
                invnb = p_sc.tile([128, N], F16, name="invnb", tag="sc")
                psb = ps1024.tile([128, N], F32, name="ps_bc", tag="ps")
                for nb in range(2):
                    nc.tensor.matmul(psb[:, nb * 512:(nb + 1) * 512],
                                     lhsT=ones[0:1, :],
                                     rhs=invn[:, nb * 512:(nb + 1) * 512],
                                     start=True, stop=True)
                nc.scalar.activation(out=invnb[:], in_=psb[:], func=AF.Copy)
                xnT = p_xn.tile([128, 2, N], F16, name="xnT")
                for t in range(2):
                    nc.vector.tensor_mul(xnT[:, t, :], featT[:, t, :], invnb[:])

                # ---- sim + top8 + S + S^T, per 128-node block ----
                ixbuf = p_ix.tile([128, 8, 10], dt.uint16, name="ixbuf")
                nc.vector.tensor_copy(
                    ixbuf[:].rearrange("p a b -> p (a b)"), selftpl[:])
                STt = p_STt.tile([128, 8, 8, 128], BF16, name="STt")
                s['STt'] = STt
                for I in range(8):
                    ps = ps1024.tile([128, N], F32, name="ps_sim", tag="ps")
                    dcb = I // 4
                    for kt in range(2):
                        for cb in range(2):
                            nc.tensor.matmul(
                                ps[:, cb * 512:(cb + 1) * 512],
                                lhsT=featT[:, kt, I * 128:(I + 1) * 128],
                                rhs=xnT[:, kt, cb * 512:(cb + 1) * 512],
                                start=(kt == 0),
                                stop=(kt == 1 and cb != dcb))
                    off2 = (I % 4) * 128
                    nc.tensor.matmul(
                        ps[:, dcb * 512:(dcb + 1) * 512], lhsT=negid[:],
                        rhs=idbig[:, 384 - off2:896 - off2],
                        start=False, stop=True)
                    # drain scores to SBUF f16 on the (idle-here) ACT first:
                    # DVE per-block drops 3.0->2.3us and the PSUM pair frees
                    # ~1.5us earlier. max8/find_index8 both read the same f16
                    # tile, so the exact-value match still holds.
                    simsb = p_simsb.tile([128, N], F16, name="simsb")
                    nc.scalar.activation(out=simsb[:], in_=ps[:], func=AF.Copy)
                    mx = p_mx.tile([128, 8], F16, name="mx", tag="mx")
                    nc.vector.max(out=mx[:], in_=simsb[:])
                    nc.vector.max_index(out=ixbuf[:, I, 1:9], in_max=mx[:],
                                        in_values=simsb[:])
                    S_I = p_S.tile([128, N], BF16, name="S_I")
                    nc.gpsimd.local_scatter(
                        out_ap=S_I[:], data_ap=onesk[:, 0:10],
                        idxs_ap=ixbuf[:, I, :].bitcast(dt.int16),
                        channels=128, num_elems=N, num_idxs=10)
                    nc.sync.dma_start_transpose(out=STt[:, I], in_=S_I[:])
                    if dbg and img == 0 and I == 0:
                        nc.sync.dma_start(out=dbg_S[:], in_=S_I[:])
                if dbg and img == 0:
                    nc.sync.dma_start(out=dbg_STt[:], in_=STt[:])
                    nc.sync.dma_start(out=dbg_ix[:],
                                      in_=ixbuf[:].rearrange("p a b -> p (a b)"))
                    nc.sync.dma_start(out=dbg_feat[:], in_=featT[:])

            # ============== TAIL phase 1: agg + e =========================
            def agg_phase(img):
                s = st[img]
                expq, pT, STt = s['expq'], s['pT'], s['STt']

                # ---- agg: lnqT [128, 4, N] f16 = ln(expq^T @ S^T) ----
                # lnqT holds ln(agg)/t, computed on the DVE from the f32
                # exponent bits (the ACT Ln LUT clamps below ~2^-66, which
                # floors 20% of entries)
                lnqT = p_lnq.tile([128, 4, N], F16, name="lnqT")
                for cb in range(4):
                    ps = ps1024.tile([128, N], F32, name="ps_agg", tag="ps")
                    for kt in range(8):
                        for half in range(2):
                            nc.tensor.matmul(
                                ps[:, half * 512:(half + 1) * 512],
                                lhsT=expq[:, kt // 2,
                                          (kt % 2) * 512 + cb * 128:
                                          (kt % 2) * 512 + cb * 128 + 128],
                                rhs=STt[:, half * 4:(half + 1) * 4, kt, :],
                                start=(kt == 0), stop=(kt == 7))
                    nc.vector.tensor_scalar(
                        out=lnqT[:, cb, :], in0=ps[:].bitcast(dt.int32),
                        scalar1=LN_ALPHA, scalar2=LN_BETA,
                        op0=mybir.AluOpType.mult, op1=mybir.AluOpType.add)

                if dbg and img == 0:
                    nc.sync.dma_start(out=dbg_lnq[:], in_=lnqT[:])
                # ---- e = relu(pT + lnqT) (lnqT already scaled by 1/t) ----
                eT = p_qe.tile([128, 4, N], F16, name="eT", tag="qe")
                nc.vector.tensor_add(eT[:], lnqT[:], pT[:])
                nc.vector.tensor_scalar_max(eT[:], eT[:], 0.0)
                if dbg and img == 0:
                    nc.sync.dma_start(out=dbg_e[:], in_=eT[:])
                s['eT'] = eT

            # ============== TAIL phase 2: g2 + FFN ========================
            def ffn_phase(img):
                s = st[img]
                xc, eT = s['xc'], s['eT']

                # ---- g2 + residual -> h f16 ----
                h = p_h.tile([128, 2, N], F16, name="h", tag="h")
                for to in range(2):
                    ps = ps1024.tile([128, N], F32, name="ps_g2", tag="ps")
                    for kt in range(4):
                        for nb in range(2):
                            nc.tensor.matmul(
                                ps[:, nb * 512:(nb + 1) * 512],
                                lhsT=wg2[:, kt, to * 128:(to + 1) * 128],
                                rhs=eT[:, kt, nb * 512:(nb + 1) * 512],
                                start=(kt == 0), stop=(kt == 3))
                    tmp = p_sc.tile([128, N], F32, name="g2tmp", tag="sc")
                    nc.scalar.activation(out=tmp[:], in_=ps[:],
                                         func=AF.Identity, bias=bt2[:, to:to + 1])
                    nc.vector.tensor_add(h[:, to, :], tmp[:], xc[:, to, :])

                # ---- FFN ----
                f1o = p_f1o.tile([128, 8, N], F16, name="f1o")
                for to in range(8):
                    ps = ps1024.tile([128, N], F32, name="ps_f1", tag="ps")
                    for kt in range(2):
                        for nb in range(2):
                            nc.tensor.matmul(
                                ps[:, nb * 512:(nb + 1) * 512],
                                lhsT=wf1[:, kt, to * 128:(to + 1) * 128],
                                rhs=h[:, kt, nb * 512:(nb + 1) * 512],
                                start=(kt == 0), stop=(kt == 1))
                    if to % 2 == 0:
                        nc.scalar.activation(out=f1o[:, to, :], in_=ps[:],
                                             func=AF.Relu, bias=bbf1[:, to:to + 1])
                    else:
                        nc.vector.tensor_scalar(
                            out=f1o[:, to, :], in0=ps[:],
                            scalar1=bbf1[:, to:to + 1], scalar2=0.0,
                            op0=mybir.AluOpType.add, op1=mybir.AluOpType.max)
                h2 = p_h.tile([128, 2, N], F16, name="h2", tag="h")
                for to in range(2):
                    ps = ps1024.tile([128, N], F32, name="ps_f2", tag="ps")
                    for kt in range(8):
                        for nb in range(2):
                            nc.tensor.matmul(
                                ps[:, nb * 512:(nb + 1) * 512],
                                lhsT=wf2[:, kt, to * 128:(to + 1) * 128],
                                rhs=f1o[:, kt, nb * 512:(nb + 1) * 512],
                                start=(kt == 0), stop=(kt == 7))
                    tmp = p_sc.tile([128, N], F32, name="f2tmp", tag="sc")
                    nc.scalar.activation(out=tmp[:], in_=ps[:],
                                         func=AF.Identity, bias=bbf2[:, to:to + 1])
                    nc.vector.tensor_add(h2[:, to, :], tmp[:], h[:, to, :])
                s['h2'] = h2

            # ============== TAIL phase 3: bottleneck + out ================
            def bott_phase(img):
                s = st[img]
                xc, h2 = s['xc'], s['h2']

                b1o = p_b.tile([64, N], F16, name="b1o", tag="b")
                psb1 = ps1024.tile([128, N], F32, name="ps_b1", tag="ps")
                for kt in range(2):
                    for nb in range(2):
                        nc.tensor.matmul(
                            psb1[0:64, nb * 512:(nb + 1) * 512],
                            lhsT=wb1[:, kt, :],
                            rhs=h2[:, kt, nb * 512:(nb + 1) * 512],
                            start=(kt == 0), stop=(kt == 1))
                nc.scalar.activation(out=b1o[:], in_=psb1[0:64, :],
                                     func=AF.Relu, bias=btb1[:, 0:1])
                pad = p_b.tile([64, 34 * 34], F16, name="pad", tag="b")
                nc.vector.memset(pad[:], 0.0)
                pad3 = pad[:].rearrange("p (r c) -> p r c", r=34)
                b1v = b1o[:].rearrange("p (r c) -> p r c", r=32)
                nc.vector.tensor_copy(pad3[:, 1:33, 1:33], b1v)
                b2o = p_b.tile([64, N], F16, name="b2o", tag="b")
                psb2 = ps1024.tile([128, N], F32, name="ps_b2", tag="ps")
                for tap in range(9):
                    dy, dx = tap // 3, tap % 3
                    for nb in range(2):
                        rhs = pad3[:, 16 * nb + dy:16 * nb + dy + 16, dx:dx + 32]
                        nc.tensor.matmul(psb2[0:64, nb * 512:(nb + 1) * 512],
                                         lhsT=wb2[:, tap, :], rhs=rhs,
                                         start=(tap == 0), stop=(tap == 8))
                nc.scalar.activation(out=b2o[:], in_=psb2[0:64, :],
                                     func=AF.Relu, bias=btb2[:, 0:1])
                b3o = p_sc.tile([128, 2, N], F16, name="b3o", tag="sc")
                for to in range(2):
                    ps = ps1024.tile([128, N], F32, name="ps_b3", tag="ps")
                    for nb in range(2):
                        nc.tensor.matmul(
                            ps[:, nb * 512:(nb + 1) * 512],
                            lhsT=wb3[:, to * 128:(to + 1) * 128],
                            rhs=b2o[:, nb * 512:(nb + 1) * 512],
                            start=True, stop=True)
                    nc.scalar.activation(out=b3o[:, to, :], in_=ps[:],
                                         func=AF.Identity, bias=btb3[:, to:to + 1])

                # ---- o3 = b3o + h2; fin = o3 + x; out = sf*fin + tf ----
                o3 = p_h.tile([128, 2, N], F16, name="o3", tag="h")
                nc.vector.tensor_add(o3[:], b3o[:], h2[:])
                fin = p_lnq.tile([128, 2, N], F16, name="fin", tag="fin")
                nc.vector.tensor_add(fin[:], o3[:], xc[:])
                for t in range(2):
                    out32 = p_out.tile([128, N], F32, name="out32")
                    nc.scalar.activation(out=out32[:], in_=fin[:, t, :],
                                         func=AF.Identity, scale=bsf[:, t:t + 1],
                                         bias=btf[:, t:t + 1])
                    nc.sync.dma_start(out=out_d[img, t * 128:(t + 1) * 128, :],
                                      in_=out32[:])

            for img in range(IMGS_PER_CORE):
                head(img)
            for img in range(IMGS_PER_CORE):
                agg_phase(img)
            for img in range(IMGS_PER_CORE):
                ffn_phase(img)
            for img in range(IMGS_PER_CORE):
                bott_phase(img)

    nc.finalize()
    return nc


# --------------------------------------------------------------------------
# entry point
# --------------------------------------------------------------------------
def kernel(**inputs):
    inp = {k: np.asarray(v) for k, v in inputs.items()}
    w = _prep_weights(inp)

    if 'nc' not in _cache:
        _cache['nc'] = _build_bass()
    nc = _cache['nc']

    x = inp['x'].astype(np.float32).reshape(B, C, N)
    in_maps = []
    for c in range(N_CORES):
        m = {'x': np.ascontiguousarray(x[c * 2:(c + 1) * 2])}
        m.update({k: v for k, v in w.items()})
        in_maps.append(m)

    from concourse.bass_utils import run_bass_kernel_spmd
    trace = bool(os.environ.get("KBENCH_TRACE"))
    res = run_bass_kernel_spmd(nc, in_maps, core_ids=list(range(N_CORES)),
                               trace=trace)
    _cache['exec_time_ns'] = res.exec_time_ns
    _cache['results'] = res
    out = np.zeros((B, C, N), np.float32)
    for c in range(N_CORES):
        out[c * 2:(c + 1) * 2] = res.results[c]['out']
    return out.reshape(B, C, H, W)


# revision 50
# speedup vs baseline: 2.5694x; 1.0123x over previous
"""Trainium2 Bass kernel for nn_Block_17033840296551 (GNN message passing block).

Data-parallel over batch: 16 images -> 8 cores x 2 images. Each core runs the
full block (g1 conv -> kNN top-9 -> EdgeConv max-agg -> g2 -> FFN -> bottleneck
-> final BN) on its 2 images with no cross-core communication.

v2 design (LSE EdgeConv — no neighbor gather):
  * All BNs folded into conv weights/biases on host.
  * EdgeConv decomposed: e[n,k] = p[n] + q[idx[n,k]], p = (Wa-Wb)@feat + b,
    q = Wb@feat; max_k relu(e) = relu(p + max_k q).
  * max_k q replaced by log-sum-exp: max_k q ~= c + ln(sum_k exp(t(q_k-c)))/t
    with t=30, c=2.0. The sum over the 9-hot neighbor set is a matmul
    S^T.T @ expq on the PE (S built by gpsimd local_scatter from the top-8
    indices; S^T via tiled xbar dma transpose). Kills the descriptor-
    generation-bound dma_gather (~160us/core) plus the DVE max-fold tree.
  * kNN: scores s[n,m] = <feat_n, feat_m/||feat_m||> rank-equivalent to cosine
    per row; self removed via -BIG diagonal (extra identity matmul into PSUM);
    DVE Max8/FindIndex8 read scores straight from PSUM (no SBUF sim buffer).
  * f16 matmul inputs (bf16 for the exp path: f16 overflows at e^11), f32
    PSUM, f16 residual stream, [128,1024] two-bank PSUM drains.
  * Two-phase emission (head: g1..sim..sel..q..p, tail: agg..FFN..bottleneck)
    interleaved across the 2 images so PE/DVE/ACT/DMA overlap.
"""

import os
import numpy as np

# problem constants (hardcoded per harness contract)
B, C, H, W = 16, 256, 32, 32
N = H * W           # 1024 pixels per image
K = 9
EPS = 1e-5
IMGS_PER_CORE = 2
N_CORES = 8
NEG_BIG = -30000.0
T_LSE = 30.0
C_LSE = 2.0
LN2 = 0.6931471805599453
# ln(x) ~= LN2 * (bitcast_int32(x) * 2^-23 - 126.957)  (max err ~0.03 in ln)
LN_ALPHA = LN2 / T_LSE / (1 << 23)
LN_BETA = -126.957 * LN2 / T_LSE

_cache = {}


# --------------------------------------------------------------------------
# host-side preprocessing
# --------------------------------------------------------------------------
def _bn_fold(p):
    g, b, m, v = np.asarray(p, np.float32)
    s = g / np.sqrt(v + EPS)
    t = b - m * s
    return s, t


def _pack_kxm(w_t, part=128):
    """[K, M] -> [part, K//part, M] (partition-major K tiling)."""
    Kd, M = w_t.shape
    kt = Kd // part
    return np.ascontiguousarray(w_t.reshape(kt, part, M).transpose(1, 0, 2))


def _pack_bias(b, part=128):
    n = b.shape[0]
    t = n // part
    return np.ascontiguousarray(b.reshape(t, part).T)  # [part, t]


def _make_selftpl():
    """ixbuf init template [128, 8, 10] uint16: col0 = self node id, col9 =
    0xFFFF (-1 as int16: ignored by local_scatter), cols 1..8 overwritten by
    find_index8."""
    tpl = np.zeros((128, 8, 10), np.uint16)
    for I in range(8):
        tpl[:, I, 0] = I * 128 + np.arange(128)
        tpl[:, I, 9] = 0xFFFF
    return np.ascontiguousarray(tpl.reshape(128, 80))


def _prep_weights(inp):
    f16 = np.float16
    s1, t1 = _bn_fold(inp['g1_bn'])
    Wg1 = s1[:, None] * inp['g1_w']
    s2, t2 = _bn_fold(inp['g2_bn'])
    Wg2 = s2[:, None] * inp['g2_w']
    sf1, tf1 = _bn_fold(inp['f1_bn'])
    Wf1 = sf1[:, None] * inp['f1_w']
    bf1 = sf1 * inp['f1_b'] + tf1
    sf2, tf2 = _bn_fold(inp['f2_bn'])
    Wf2 = sf2[:, None] * inp['f2_w']
    bf2 = sf2 * inp['f2_b'] + tf2
    sb1, tb1 = _bn_fold(inp['b1_bn'])
    Wb1 = sb1[:, None] * inp['b1_w']
    sb2, tb2 = _bn_fold(inp['b2_bn'])
    Wb2 = sb2[:, None, None, None] * inp['b2_w']
    sb3, tb3 = _bn_fold(inp['b3_bn'])
    Wb3 = sb3[:, None] * inp['b3_w']
    sf, tf = _bn_fold(inp['bnf'])

    A = inp['edge_w'][:, :C]
    Bm = inp['edge_w'][:, C:]
    Wp = A - Bm
    Wq = Bm
    bp = inp['edge_b'] + C_LSE          # LSE shift folded into the p bias

    wb2_t = np.zeros((64, 9, 64), f16)
    for dy in range(3):
        for dx in range(3):
            wb2_t[:, dy * 3 + dx, :] = Wb2[:, :, dy, dx].T.astype(f16)

    return {
        'wg1': _pack_kxm(Wg1.T.astype(f16)),                # [128,2,256]
        'wp': _pack_kxm(Wp.T.astype(f16)),                  # [128,2,512]
        'wq': _pack_kxm((T_LSE * Wq).T.astype(f16)),        # [128,2,512] (t*Wq)
        'wg2': _pack_kxm(Wg2.T.astype(f16)),                # [128,4,256]
        'wf1': _pack_kxm(Wf1.T.astype(f16)),                # [128,2,1024]
        'wf2': _pack_kxm(Wf2.T.astype(f16)),                # [128,8,256]
        'wb1': _pack_kxm(Wb1.T.astype(f16)),                # [128,2,64]
        'wb2': wb2_t,                                        # [64,9,64]
        'wb3': Wb3.T.astype(f16),                            # [64,256]
        'bt1': _pack_bias(t1),                               # [128,2] f32
        'bt2': _pack_bias(t2),
        'bbp': _pack_bias(bp),                               # [128,4]
        'bbf1': _pack_bias(bf1),                             # [128,8]
        'bbf2': _pack_bias(bf2),
        'btb1': np.ascontiguousarray(tb1[:, None].astype(np.float32)),  # [64,1]
        'btb2': np.ascontiguousarray(tb2[:, None].astype(np.float32)),
        'btb3': _pack_bias(tb3),
        'bsf': _pack_bias(sf),
        'btf': _pack_bias(tf),
        'expb': np.full((128, 1), -T_LSE * C_LSE, np.float32),
        'lnb': np.full((128, 1), 1e-30, np.float32),
        'selftpl': _make_selftpl(),                          # [128,80] u16
    }


# --------------------------------------------------------------------------
# device kernel builder
# --------------------------------------------------------------------------
def _build_bass():
    import concourse.bass as bass
    import concourse.mybir as mybir
    from concourse import bacc
    from concourse.tile import TileContext
    from concourse.masks import make_identity

    dt = mybir.dt
    F16 = dt.float16
    BF16 = dt.bfloat16
    F32 = dt.float32
    AF = mybir.ActivationFunctionType

    nc = bacc.Bacc()

    # ---- DRAM parameters ----
    x_d = nc.declare_dram_parameter("x", [IMGS_PER_CORE, C, N], F32, isOutput=False)
    wg1_d = nc.declare_dram_parameter("wg1", [128, 2, 256], F16, isOutput=False)
    wp_d = nc.declare_dram_parameter("wp", [128, 2, 512], F16, isOutput=False)
    wq_d = nc.declare_dram_parameter("wq", [128, 2, 512], F16, isOutput=False)
    wg2_d = nc.declare_dram_parameter("wg2", [128, 4, 256], F16, isOutput=False)
    wf1_d = nc.declare_dram_parameter("wf1", [128, 2, 1024], F16, isOutput=False)
    wf2_d = nc.declare_dram_parameter("wf2", [128, 8, 256], F16, isOutput=False)
    wb1_d = nc.declare_dram_parameter("wb1", [128, 2, 64], F16, isOutput=False)
    wb2_d = nc.declare_dram_parameter("wb2", [64, 9, 64], F16, isOutput=False)
    wb3_d = nc.declare_dram_parameter("wb3", [64, 256], F16, isOutput=False)
    bt1_d = nc.declare_dram_parameter("bt1", [128, 2], F32, isOutput=False)
    bt2_d = nc.declare_dram_parameter("bt2", [128, 2], F32, isOutput=False)
    bbp_d = nc.declare_dram_parameter("bbp", [128, 4], F32, isOutput=False)
    bbf1_d = nc.declare_dram_parameter("bbf1", [128, 8], F32, isOutput=False)
    bbf2_d = nc.declare_dram_parameter("bbf2", [128, 2], F32, isOutput=False)
    btb1_d = nc.declare_dram_parameter("btb1", [64, 1], F32, isOutput=False)
    btb2_d = nc.declare_dram_parameter("btb2", [64, 1], F32, isOutput=False)
    btb3_d = nc.declare_dram_parameter("btb3", [128, 2], F32, isOutput=False)
    bsf_d = nc.declare_dram_parameter("bsf", [128, 2], F32, isOutput=False)
    btf_d = nc.declare_dram_parameter("btf", [128, 2], F32, isOutput=False)
    expb_d = nc.declare_dram_parameter("expb", [128, 1], F32, isOutput=False)
    lnb_d = nc.declare_dram_parameter("lnb", [128, 1], F32, isOutput=False)
    selftpl_d = nc.declare_dram_parameter("selftpl", [128, 80], dt.uint16,
                                          isOutput=False)
    out_d = nc.declare_dram_parameter("out", [IMGS_PER_CORE, C, N], F32,
                                      isOutput=True)
    dbg = bool(os.environ.get("KBENCH_DEBUG"))
    if dbg:
        dbg_ix = nc.declare_dram_parameter("dbg_ix", [128, 80], dt.uint16,
                                           isOutput=True)
        dbg_S = nc.declare_dram_parameter("dbg_S", [128, N], BF16, isOutput=True)
        dbg_STt = nc.declare_dram_parameter("dbg_STt", [128, 8, 8, 128], BF16,
                                            isOutput=True)
        dbg_e = nc.declare_dram_parameter("dbg_e", [128, 4, N], F16,
                                          isOutput=True)
        dbg_expq = nc.declare_dram_parameter("dbg_expq", [128, 4, N], BF16,
                                             isOutput=True)
        dbg_lnq = nc.declare_dram_parameter("dbg_lnq", [128, 4, N], F16,
                                            isOutput=True)
        dbg_feat = nc.declare_dram_parameter("dbg_feat", [128, 2, N], F16,
                                             isOutput=True)

    with TileContext(nc) as tc:
        import contextlib
        ctx = contextlib.ExitStack()
        with ctx:
            consts = ctx.enter_context(tc.tile_pool(name="consts", bufs=1))
            p_xc = ctx.enter_context(tc.tile_pool(name="xc", bufs=2))
            p_feat = ctx.enter_context(tc.tile_pool(name="feat", bufs=2))
            p_xn = ctx.enter_context(tc.tile_pool(name="xn", bufs=2))
            p_sc = ctx.enter_context(tc.tile_pool(name="sc", bufs=3))
            p_qe = ctx.enter_context(tc.tile_pool(name="qe", bufs=2))
            p_S = ctx.enter_context(tc.tile_pool(name="S", bufs=2))
            p_STt = ctx.enter_context(tc.tile_pool(name="STt", bufs=2))
            p_lnq = ctx.enter_context(tc.tile_pool(name="lnq", bufs=2))
            p_p = ctx.enter_context(tc.tile_pool(name="p", bufs=2))
            p_h = ctx.enter_context(tc.tile_pool(name="h", bufs=4))
            p_f1o = ctx.enter_context(tc.tile_pool(name="f1o", bufs=1))
            p_b = ctx.enter_context(tc.tile_pool(name="b", bufs=3))
            p_out = ctx.enter_context(tc.tile_pool(name="out", bufs=2))
            p_ix = ctx.enter_context(tc.tile_pool(name="ix", bufs=2))
            p_mx = ctx.enter_context(tc.tile_pool(name="mx", bufs=2))
            # all 8 PSUM banks in one 4-deep two-bank rotation (the n2 row
            # vector borrows row 0 of a ps1024 tile instead of its own pool)
            ps1024 = ctx.enter_context(
                tc.tile_pool(name="ps1024", bufs=4, space="PSUM"))

            # ---- constants / weights (loaded once) ----
            # alternate the two HWDGE rings (sync/scalar) so the ~21 weight
            # loads don't serialize on one ring at startup
            _ld = [0]

            def load(name, shape, dtype, src):
                t = consts.tile(shape, dtype, name=name)
                eng = nc.sync if _ld[0] % 2 == 0 else nc.scalar
                _ld[0] += 1
                eng.dma_start(out=t[:], in_=src[:])
                return t

            wg1 = load("wg1s", [128, 2, 256], F16, wg1_d)
            wp = load("wps", [128, 2, 512], F16, wp_d)
            wq = load("wqs", [128, 2, 512], F16, wq_d)
            wg2 = load("wg2s", [128, 4, 256], F16, wg2_d)
            wf1 = load("wf1s", [128, 2, 1024], F16, wf1_d)
            wf2 = load("wf2s", [128, 8, 256], F16, wf2_d)
            wb1 = load("wb1s", [128, 2, 64], F16, wb1_d)
            wb2 = load("wb2s", [64, 9, 64], F16, wb2_d)
            wb3 = load("wb3s", [64, 256], F16, wb3_d)
            bt1 = load("bt1s", [128, 2], F32, bt1_d)
            bt2 = load("bt2s", [128, 2], F32, bt2_d)
            bbp = load("bbps", [128, 4], F32, bbp_d)
            bbf1 = load("bbf1s", [128, 8], F32, bbf1_d)
            bbf2 = load("bbf2s", [128, 2], F32, bbf2_d)
            btb1 = load("btb1s", [64, 1], F32, btb1_d)
            btb2 = load("btb2s", [64, 1], F32, btb2_d)
            btb3 = load("btb3s", [128, 2], F32, btb3_d)
            bsf = load("bsfs", [128, 2], F32, bsf_d)
            btf = load("btfs", [128, 2], F32, btf_d)
            expb = load("expbs", [128, 1], F32, expb_d)
            lnb = load("lnbs", [128, 1], F32, lnb_d)
            selftpl = load("selftpls", [128, 80], dt.uint16, selftpl_d)

            ident = consts.tile([128, 128], F16, name="ident")
            make_identity(nc, ident[:])
            negid = consts.tile([128, 128], F16, name="negid")
            nc.scalar.activation(out=negid[:], in_=ident[:], func=AF.Copy,
                                 scale=NEG_BIG)
            ones = consts.tile([128, 128], F16, name="ones")
            nc.gpsimd.memset(ones[:], 1.0)
            onesk = consts.tile([128, 16], BF16, name="onesk")
            nc.gpsimd.memset(onesk[:], 1.0)
            # idbig[k, f] = 1 iff f == k + 384 (shifted identity for diag-kill)
            idbig = consts.tile([128, 1024], F16, name="idbig")
            nc.gpsimd.memset(idbig[:], 0.0)
            nc.gpsimd.affine_select(
                out=idbig[:], in_=idbig[:],
                compare_op=mybir.AluOpType.not_equal, fill=1.0,
                base=384, pattern=[[-1, 1024]], channel_multiplier=1)

            # per-image state carried from head to tail
            st = [{} for _ in range(IMGS_PER_CORE)]

            # ============== HEAD: load, g1, norms, sim/top8/S, q, p =======
            def head(img):
                s = st[img]
                xc = p_xc.tile([128, 2, N], F16, name="xc")
                for t in range(2):
                    # cast f32->f16 during DMA (SWDGE)
                    nc.gpsimd.dma_start(out=xc[:, t, :],
                                        in_=x_d[img, t * 128:(t + 1) * 128, :])
                s['xc'] = xc

                # ---- g1: featT [128, 2, N] f16 ----
                # (kt-outer loops everywhere: one LDWEIGHTS serves both
                # nb-halves, so matmuls stream back-to-back)
                featT = p_feat.tile([128, 2, N], F16, name="featT")
                for to in range(2):
                    ps = ps1024.tile([128, N], F32, name="ps_g1", tag="ps")
                    for kt in range(2):
                        for nb in range(2):
                            nc.tensor.matmul(
                                ps[:, nb * 512:(nb + 1) * 512],
                                lhsT=wg1[:, kt, to * 128:(to + 1) * 128],
                                rhs=xc[:, kt, nb * 512:(nb + 1) * 512],
                                start=(kt == 0), stop=(kt == 1))
                    nc.scalar.activation(out=featT[:, to, :], in_=ps[:],
                                         func=AF.Identity, bias=bt1[:, to:to + 1])

                # ---- row norms first: the rsqrt ACT-table swap and the n2
                # matmuls run while the PE then chews q/p, so invnb is ready
                # by the time the bcast matmul needs it ----
                fsq = p_sc.tile([128, 2, N], F16, name="fsq", tag="sc")
                nc.vector.tensor_mul(fsq[:], featT[:], featT[:])
                invn = p_mx.tile([1, N], F16, name="invn", tag="invn")
                ps_nb = ps1024.tile([128, N], F32, name="ps_nb", tag="ps")
                for nb in range(2):
                    for kt in range(2):
                        nc.tensor.matmul(
                            ps_nb[0:1, nb * 512:(nb + 1) * 512],
                            lhsT=ones[:, 0:1],
                            rhs=fsq[:, kt, nb * 512:(nb + 1) * 512],
                            start=(kt == 0), stop=(kt == 1))
                    # rank-only use; the gated-accuracy LUT is fine here
                    nc.scalar.activation(out=invn[:, nb * 512:(nb + 1) * 512],
                                         in_=ps_nb[0:1, nb * 512:(nb + 1) * 512],
                                         func=AF.Abs_reciprocal_sqrt)

                # ---- q -> expq (t*Wq folded; exp bias = -t*c) ----
                expq = p_qe.tile([128, 4, N], BF16, name="expq", tag="qe")
                for pair in range(4):
                    ps = ps1024.tile([128, N], F32, name="ps_q", tag="ps")
                    for sub in range(2):
                        nt = 2 * pair + sub
                        for kt in range(2):
                            nc.tensor.matmul(
                                ps[:, sub * 512:(sub + 1) * 512],
                                lhsT=featT[:, kt, nt * 128:(nt + 1) * 128],
                                rhs=wq[:, kt, :], start=(kt == 0), stop=(kt == 1))
                    nc.scalar.activation(out=expq[:, pair, :], in_=ps[:],
                                         func=AF.Exp, bias=expb[:, 0:1])
                if dbg and img == 0:
                    nc.sync.dma_start(out=dbg_expq[:], in_=expq[:])
                s['expq'] = expq

                # ---- p^T [128, 4, N] f16 (ch-part, bias + c folded) ----
                pT = p_p.tile([128, 4, N], F16, name="pT")
                for to in range(4):
                    ps = ps1024.tile([128, N], F32, name="ps_p", tag="ps")
                    for kt in range(2):
                        for nb in range(2):
                            nc.tensor.matmul(
                                ps[:, nb * 512:(nb + 1) * 512],
                                lhsT=wp[:, kt, to * 128:(to + 1) * 128],
                                rhs=featT[:, kt, nb * 512:(nb + 1) * 512],
                                start=(kt == 0), stop=(kt == 1))
                    # split PSUM drains between ACT and DVE: the PE fills a
                    # two-bank tile in ~0.9-1.8us; one ACT alone (1.3us/tile)
                    # is the pipeline bottleneck
                    if to < 2:
                        nc.scalar.activation(out=pT[:, to, :], in_=ps[:],
                                             func=AF.Identity,
                                             bias=bbp[:, to:to + 1])
                    else:
                        nc.vector.tensor_scalar(
                            out=pT[:, to, :], in0=ps[:],
                            scalar1=bbp[:, to:to + 1], scalar2=None,
                            op0=mybir.AluOpType.add)
                s['pT'] = pT

                # ---- broadcast invn -> xnT (rhs-side normalized) ----
                invnb = p_sc.tile([128, N], F16, name="invnb", tag="sc")
                psb = ps1024.tile([128, N], F32, name="ps_bc", tag="ps")
                for nb in range(2):
                    nc.tensor.matmul(psb[:, nb * 512:(nb + 1) * 512],
                                     lhsT=ones[0:1, :],
                                     rhs=invn[:, nb * 512:(nb + 1) * 512],
                                     start=True, stop=True)
                nc.scalar.activation(out=invnb[:], in_=psb[:], func=AF.Copy)
                xnT = p_xn.tile([128, 2, N], F16, name="xnT")
                for t in range(2):
                    nc.vector.tensor_mul(xnT[:, t, :], featT[:, t, :], invnb[:])

                # ---- sim + top8 + S + S^T, per 128-node block ----
                ixbuf = p_ix.tile([128, 8, 10], dt.uint16, name="ixbuf")
                nc.vector.tensor_copy(
                    ixbuf[:].rearrange("p a b -> p (a b)"), selftpl[:])
                STt = p_STt.tile([128, 8, 8, 128], BF16, name="STt")
                s['STt'] = STt
                for I in range(8):
                    ps = ps1024.tile([128, N], F32, name="ps_sim", tag="ps")
                    dcb = I // 4
                    for kt in range(2):
                        for cb in range(2):
                            nc.tensor.matmul(
                                ps[:, cb * 512:(cb + 1) * 512],
                                lhsT=featT[:, kt, I * 128:(I + 1) * 128],
                                rhs=xnT[:, kt, cb * 512:(cb + 1) * 512],
                                start=(kt == 0),
                                stop=(kt == 1 and cb != dcb))
                    off2 = (I % 4) * 128
                    nc.tensor.matmul(
                        ps[:, dcb * 512:(dcb + 1) * 512], lhsT=negid[:],
                        rhs=idbig[:, 384 - off2:896 - off2],
                        start=False, stop=True)
                    # mx must be f32: find_index8 matches exact values, so
                    # in_max and in_values (PSUM f32) must share precision
                    mx = p_mx.tile([128, 8], F32, name="mx", tag="mx")
                    nc.vector.max(out=mx[:], in_=ps[:])
                    nc.vector.max_index(out=ixbuf[:, I, 1:9], in_max=mx[:],
                                        in_values=ps[:])
                    S_I = p_S.tile([128, N], BF16, name="S_I")
                    nc.gpsimd.local_scatter(
                        out_ap=S_I[:], data_ap=onesk[:, 0:10],
                        idxs_ap=ixbuf[:, I, :].bitcast(dt.int16),
                        channels=128, num_elems=N, num_idxs=10)
                    nc.sync.dma_start_transpose(out=STt[:, I], in_=S_I[:])
                    if dbg and img == 0 and I == 0:
                        nc.sync.dma_start(out=dbg_S[:], in_=S_I[:])
                if dbg and img == 0:
                    nc.sync.dma_start(out=dbg_STt[:], in_=STt[:])
                    nc.sync.dma_start(out=dbg_ix[:],
                                      in_=ixbuf[:].rearrange("p a b -> p (a b)"))
                    nc.sync.dma_start(out=dbg_feat[:], in_=featT[:])

            # ============== TAIL phase 1: agg + e =========================
            def agg_phase(img):
                s = st[img]
                expq, pT, STt = s['expq'], s['pT'], s['STt']

                # ---- agg: lnqT [128, 4, N] f16 = ln(expq^T @ S^T) ----
                # lnqT holds ln(agg)/t, computed on the DVE from the f32
                # exponent bits (the ACT Ln LUT clamps below ~2^-66, which
                # floors 20% of entries)
                lnqT = p_lnq.tile([128, 4, N], F16, name="lnqT")
                for cb in range(4):
                    ps = ps1024.tile([128, N], F32, name="ps_agg", tag="ps")
                    for kt in range(8):
                        for half in range(2):
                            nc.tensor.matmul(
                                ps[:, half * 512:(half + 1) * 512],
                                lhsT=expq[:, kt // 2,
                                          (kt % 2) * 512 + cb * 128:
                                          (kt % 2) * 512 + cb * 128 + 128],
                                rhs=STt[:, half * 4:(half + 1) * 4, kt, :],
                                start=(kt == 0), stop=(kt == 7))
                    nc.vector.tensor_scalar(
                        out=lnqT[:, cb, :], in0=ps[:].bitcast(dt.int32),
                        scalar1=LN_ALPHA, scalar2=LN_BETA,
                        op0=mybir.AluOpType.mult, op1=mybir.AluOpType.add)

                if dbg and img == 0:
                    nc.sync.dma_start(out=dbg_lnq[:], in_=lnqT[:])
                # ---- e = relu(pT + lnqT) (lnqT already scaled by 1/t) ----
                eT = p_qe.tile([128, 4, N], F16, name="eT", tag="qe")
                nc.vector.tensor_add(eT[:], lnqT[:], pT[:])
                nc.vector.tensor_scalar_max(eT[:], eT[:], 0.0)
                if dbg and img == 0:
                    nc.sync.dma_start(out=dbg_e[:], in_=eT[:])
                s['eT'] = eT

            # ============== TAIL phase 2: g2 + FFN ========================
            def ffn_phase(img):
                s = st[img]
                xc, eT = s['xc'], s['eT']

                # ---- g2 + residual -> h f16 ----
                h = p_h.tile([128, 2, N], F16, name="h", tag="h")
                for to in range(2):
                    ps = ps1024.tile([128, N], F32, name="ps_g2", tag="ps")
                    for kt in range(4):
                        for nb in range(2):
                            nc.tensor.matmul(
                                ps[:, nb * 512:(nb + 1) * 512],
                                lhsT=wg2[:, kt, to * 128:(to + 1) * 128],
                                rhs=eT[:, kt, nb * 512:(nb + 1) * 512],
                                start=(kt == 0), stop=(kt == 3))
                    tmp = p_sc.tile([128, N], F32, name="g2tmp", tag="sc")
                    nc.scalar.activation(out=tmp[:], in_=ps[:],
                                         func=AF.Identity, bias=bt2[:, to:to + 1])
                    nc.vector.tensor_add(h[:, to, :], tmp[:], xc[:, to, :])

                # ---- FFN ----
                f1o = p_f1o.tile([128, 8, N], F16, name="f1o")
                for to in range(8):
                    ps = ps1024.tile([128, N], F32, name="ps_f1", tag="ps")
                    for kt in range(2):
                        for nb in range(2):
                            nc.tensor.matmul(
                                ps[:, nb * 512:(nb + 1) * 512],
                                lhsT=wf1[:, kt, to * 128:(to + 1) * 128],
                                rhs=h[:, kt, nb * 512:(nb + 1) * 512],
                                start=(kt == 0), stop=(kt == 1))
                    if to % 2 == 0:
                        nc.scalar.activation(out=f1o[:, to, :], in_=ps[:],
                                             func=AF.Relu, bias=bbf1[:, to:to + 1])
                    else:
                        nc.vector.tensor_scalar(
                            out=f1o[:, to, :], in0=ps[:],
                            scalar1=bbf1[:, to:to + 1], scalar2=0.0,
                            op0=mybir.AluOpType.add, op1=mybir.AluOpType.max)
                h2 = p_h.tile([128, 2, N], F16, name="h2", tag="h")
                for to in range(2):
                    ps = ps1024.tile([128, N], F32, name="ps_f2", tag="ps")
                    for kt in range(8):
                        for nb in range(2):
                            nc.tensor.matmul(
                                ps[:, nb * 512:(nb + 1) * 512],
                                lhsT=wf2[:, kt, to * 128:(to + 1) * 128],
                                rhs=f1o[:, kt, nb * 512:(nb + 1) * 512],
                                start=(kt == 0), stop=(kt == 7))
                    tmp = p_sc.tile([128, N], F32, name="f2tmp", tag="sc")
                    nc.scalar.activation(out=tmp[:], in_=ps[:],
                                         func=AF.Identity, bias=bbf2[:, to:to + 1])
                    nc.vector.tensor_add(h2[:, to, :], tmp[:], h[:, to, :])
                s['h2'] = h2

            # ============== TAIL phase 3: bottleneck + out ================
            def bott_phase(img):
                s = st[img]
                xc, h2 = s['xc'], s['h2']

                b1o = p_b.tile([64, N], F16, name="b1o", tag="b")
                psb1 = ps1024.tile([128, N], F32, name="ps_b1", tag="ps")
                for kt in range(2):
                    for nb in range(2):
                        nc.tensor.matmul(
                            psb1[0:64, nb * 512:(nb + 1) * 512],
                            lhsT=wb1[:, kt, :],
                            rhs=h2[:, kt, nb * 512:(nb + 1) * 512],
                            start=(kt == 0), stop=(kt == 1))
                nc.scalar.activation(out=b1o[:], in_=psb1[0:64, :],
                                     func=AF.Relu, bias=btb1[:, 0:1])
                pad = p_b.tile([64, 34 * 34], F16, name="pad", tag="b")
                nc.vector.memset(pad[:], 0.0)
                pad3 = pad[:].rearrange("p (r c) -> p r c", r=34)
                b1v = b1o[:].rearrange("p (r c) -> p r c", r=32)
                nc.vector.tensor_copy(pad3[:, 1:33, 1:33], b1v)
                b2o = p_b.tile([64, N], F16, name="b2o", tag="b")
                psb2 = ps1024.tile([128, N], F32, name="ps_b2", tag="ps")
                for tap in range(9):
                    dy, dx = tap // 3, tap % 3
                    for nb in range(2):
                        rhs = pad3[:, 16 * nb + dy:16 * nb + dy + 16, dx:dx + 32]
                        nc.tensor.matmul(psb2[0:64, nb * 512:(nb + 1) * 512],
                                         lhsT=wb2[:, tap, :], rhs=rhs,
                                         start=(tap == 0), stop=(tap == 8))
                nc.scalar.activation(out=b2o[:], in_=psb2[0:64, :],
                                     func=AF.Relu, bias=btb2[:, 0:1])
                b3o = p_sc.tile([128, 2, N], F16, name="b3o", tag="sc")
                for to in range(2):
                    ps = ps1024.tile([128, N], F32, name="ps_b3", tag="ps")
                    for nb in range(2):
                        nc.tensor.matmul(
                            ps[:, nb * 512:(nb + 1) * 512],
                            lhsT=wb3[:, to * 128:(to + 1) * 128],
                            rhs=b2o[:, nb * 512:(nb + 1) * 512],
                            start=True, stop=True)
                    nc.scalar.activation(out=b3o[:, to, :], in_=ps[:],
                                         func=AF.Identity, bias=btb3[:, to:to + 1])

                # ---- o3 = b3o + h2; fin = o3 + x; out = sf*fin + tf ----
                o3 = p_h.tile([128, 2, N], F16, name="o3", tag="h")
                nc.vector.tensor_add(o3[:], b3o[:], h2[:])
                fin = p_lnq.tile([128, 2, N], F16, name="fin", tag="fin")
                nc.vector.tensor_add(fin[:], o3[:], xc[:])
                for t in range(2):
                    out32 = p_out.tile([128, N], F32, name="out32")
                    nc.scalar.activation(out=out32[:], in_=fin[:, t, :],
                                         func=AF.Identity, scale=bsf[:, t:t + 1],
                                         bias=btf[:, t:t + 1])
                    nc.sync.dma_start(out=out_d[img, t * 128:(t + 1) * 128, :],
                                      in_=out32[:])

            for img in range(IMGS_PER_CORE):
                head(img)
            for img in range(IMGS_PER_CORE):
                agg_phase(img)
            for img in range(IMGS_PER_CORE):
                ffn_phase(img)
            for img in range(IMGS_PER_CORE):
                bott_phase(img)

    nc.finalize()
    return nc


# --------------------------------------------------------------------------
# entry point
# --------------------------------------------------------------------------
def kernel(**inputs):
    inp = {k: np.asarray(v) for k, v in inputs.items()}
    w = _prep_weights(inp)

    if 'nc' not in _cache:
        _cache['nc'] = _build_bass()
    nc = _cache['nc']

    x = inp['x'].astype(np.float32).reshape(B, C, N)
    in_maps = []
    for c in range(N_CORES):
        m = {'x': np.ascontiguousarray(x[c * 2:(c + 1) * 2])}
        m.update({k: v for k, v in w.items()})
        in_maps.append(m)

    from concourse.bass_utils import run_bass_kernel_spmd
    trace = bool(os.environ.get("KBENCH_TRACE"))
    res = run_bass_kernel_spmd(nc, in_maps, core_ids=list(range(N_CORES)),
                               trace=trace)
    _cache['exec_time_ns'] = res.exec_time_ns
    _cache['results'] = res
    out = np.zeros((B, C, N), np.float32)
    for c in range(N_CORES):
        out[c * 2:(c + 1) * 2] = res.results[c]['out']
    return out.reshape(B, C, H, W)


# revision 53
# speedup vs baseline: 2.6398x; 1.0274x over previous
"""Trainium2 Bass kernel for nn_Block_17033840296551 (GNN message passing block).

Data-parallel over batch: 16 images -> 8 cores x 2 images. Each core runs the
full block (g1 conv -> kNN top-9 -> EdgeConv max-agg -> g2 -> FFN -> bottleneck
-> final BN) on its 2 images with no cross-core communication.

v2 design (LSE EdgeConv — no neighbor gather):
  * All BNs folded into conv weights/biases on host.
  * EdgeConv decomposed: e[n,k] = p[n] + q[idx[n,k]], p = (Wa-Wb)@feat + b,
    q = Wb@feat; max_k relu(e) = relu(p + max_k q).
  * max_k q replaced by log-sum-exp: max_k q ~= c + ln(sum_k exp(t(q_k-c)))/t
    with t=30, c=2.0. The sum over the 9-hot neighbor set is a matmul
    S^T.T @ expq on the PE (S built by gpsimd local_scatter from the top-8
    indices; S^T via tiled xbar dma transpose). Kills the descriptor-
    generation-bound dma_gather (~160us/core) plus the DVE max-fold tree.
  * kNN: scores s[n,m] = <feat_n, feat_m/||feat_m||> rank-equivalent to cosine
    per row; self removed via -BIG diagonal (extra identity matmul into PSUM);
    DVE Max8/FindIndex8 read scores straight from PSUM (no SBUF sim buffer).
  * f16 matmul inputs (bf16 for the exp path: f16 overflows at e^11), f32
    PSUM, f16 residual stream, [128,1024] two-bank PSUM drains.
  * Two-phase emission (head: g1..sim..sel..q..p, tail: agg..FFN..bottleneck)
    interleaved across the 2 images so PE/DVE/ACT/DMA overlap.
"""

import os
import numpy as np

# problem constants (hardcoded per harness contract)
B, C, H, W = 16, 256, 32, 32
N = H * W           # 1024 pixels per image
K = 9
EPS = 1e-5
IMGS_PER_CORE = 2
N_CORES = 8
NEG_BIG = -30000.0
T_LSE = 30.0
C_LSE = 2.0
LN2 = 0.6931471805599453
# ln(x) ~= LN2 * (bitcast_int32(x) * 2^-23 - 126.957)  (max err ~0.03 in ln)
LN_ALPHA = LN2 / T_LSE / (1 << 23)
LN_BETA = -126.957 * LN2 / T_LSE

_cache = {}


# --------------------------------------------------------------------------
# host-side preprocessing
# --------------------------------------------------------------------------
def _bn_fold(p):
    g, b, m, v = np.asarray(p, np.float32)
    s = g / np.sqrt(v + EPS)
    t = b - m * s
    return s, t


def _pack_kxm(w_t, part=128):
    """[K, M] -> [part, K//part, M] (partition-major K tiling)."""
    Kd, M = w_t.shape
    kt = Kd // part
    return np.ascontiguousarray(w_t.reshape(kt, part, M).transpose(1, 0, 2))


def _pack_bias(b, part=128):
    n = b.shape[0]
    t = n // part
    return np.ascontiguousarray(b.reshape(t, part).T)  # [part, t]


def _make_selftpl():
    """ixbuf init template [128, 8, 10] uint16: col0 = self node id, col9 =
    0xFFFF (-1 as int16: ignored by local_scatter), cols 1..8 overwritten by
    find_index8."""
    tpl = np.zeros((128, 8, 10), np.uint16)
    for I in range(8):
        tpl[:, I, 0] = I * 128 + np.arange(128)
        tpl[:, I, 9] = 0xFFFF
    return np.ascontiguousarray(tpl.reshape(128, 80))


def _prep_weights(inp):
    f16 = np.float16
    s1, t1 = _bn_fold(inp['g1_bn'])
    Wg1 = s1[:, None] * inp['g1_w']
    s2, t2 = _bn_fold(inp['g2_bn'])
    Wg2 = s2[:, None] * inp['g2_w']
    sf1, tf1 = _bn_fold(inp['f1_bn'])
    Wf1 = sf1[:, None] * inp['f1_w']
    bf1 = sf1 * inp['f1_b'] + tf1
    sf2, tf2 = _bn_fold(inp['f2_bn'])
    Wf2 = sf2[:, None] * inp['f2_w']
    bf2 = sf2 * inp['f2_b'] + tf2
    sb1, tb1 = _bn_fold(inp['b1_bn'])
    Wb1 = sb1[:, None] * inp['b1_w']
    sb2, tb2 = _bn_fold(inp['b2_bn'])
    Wb2 = sb2[:, None, None, None] * inp['b2_w']
    sb3, tb3 = _bn_fold(inp['b3_bn'])
    Wb3 = sb3[:, None] * inp['b3_w']
    sf, tf = _bn_fold(inp['bnf'])

    A = inp['edge_w'][:, :C]
    Bm = inp['edge_w'][:, C:]
    Wp = A - Bm
    Wq = Bm
    bp = inp['edge_b'] + C_LSE          # LSE shift folded into the p bias

    wb2_t = np.zeros((64, 9, 64), f16)
    for dy in range(3):
        for dx in range(3):
            wb2_t[:, dy * 3 + dx, :] = Wb2[:, :, dy, dx].T.astype(f16)

    return {
        'wg1': _pack_kxm(Wg1.T.astype(f16)),                # [128,2,256]
        'wp': _pack_kxm(Wp.T.astype(f16)),                  # [128,2,512]
        'wq': _pack_kxm((T_LSE * Wq).T.astype(f16)),        # [128,2,512] (t*Wq)
        'wg2': _pack_kxm(Wg2.T.astype(f16)),                # [128,4,256]
        'wf1': _pack_kxm(Wf1.T.astype(f16)),                # [128,2,1024]
        'wf2': _pack_kxm(Wf2.T.astype(f16)),                # [128,8,256]
        'wb1': _pack_kxm(Wb1.T.astype(f16)),                # [128,2,64]
        'wb2': wb2_t,                                        # [64,9,64]
        'wb3': Wb3.T.astype(f16),                            # [64,256]
        'bt1': _pack_bias(t1),                               # [128,2] f32
        'bt2': _pack_bias(t2),
        'bbp': _pack_bias(bp),                               # [128,4]
        'bbf1': _pack_bias(bf1),                             # [128,8]
        'bbf2': _pack_bias(bf2),
        'btb1': np.ascontiguousarray(tb1[:, None].astype(np.float32)),  # [64,1]
        'btb2': np.ascontiguousarray(tb2[:, None].astype(np.float32)),
        'btb3': _pack_bias(tb3),
        'bsf': _pack_bias(sf),
        'btf': _pack_bias(tf),
        'expb': np.full((128, 1), -T_LSE * C_LSE, np.float32),
        'lnb': np.full((128, 1), 1e-30, np.float32),
        'selftpl': _make_selftpl(),                          # [128,80] u16
    }


# --------------------------------------------------------------------------
# device kernel builder
# --------------------------------------------------------------------------
def _build_bass():
    import concourse.bass as bass
    import concourse.mybir as mybir
    from concourse import bacc
    from concourse.tile import TileContext
    from concourse.masks import make_identity

    dt = mybir.dt
    F16 = dt.float16
    BF16 = dt.bfloat16
    F32 = dt.float32
    AF = mybir.ActivationFunctionType

    nc = bacc.Bacc()

    # ---- DRAM parameters ----
    x_d = nc.declare_dram_parameter("x", [IMGS_PER_CORE, C, N], F32, isOutput=False)
    wg1_d = nc.declare_dram_parameter("wg1", [128, 2, 256], F16, isOutput=False)
    wp_d = nc.declare_dram_parameter("wp", [128, 2, 512], F16, isOutput=False)
    wq_d = nc.declare_dram_parameter("wq", [128, 2, 512], F16, isOutput=False)
    wg2_d = nc.declare_dram_parameter("wg2", [128, 4, 256], F16, isOutput=False)
    wf1_d = nc.declare_dram_parameter("wf1", [128, 2, 1024], F16, isOutput=False)
    wf2_d = nc.declare_dram_parameter("wf2", [128, 8, 256], F16, isOutput=False)
    wb1_d = nc.declare_dram_parameter("wb1", [128, 2, 64], F16, isOutput=False)
    wb2_d = nc.declare_dram_parameter("wb2", [64, 9, 64], F16, isOutput=False)
    wb3_d = nc.declare_dram_parameter("wb3", [64, 256], F16, isOutput=False)
    bt1_d = nc.declare_dram_parameter("bt1", [128, 2], F32, isOutput=False)
    bt2_d = nc.declare_dram_parameter("bt2", [128, 2], F32, isOutput=False)
    bbp_d = nc.declare_dram_parameter("bbp", [128, 4], F32, isOutput=False)
    bbf1_d = nc.declare_dram_parameter("bbf1", [128, 8], F32, isOutput=False)
    bbf2_d = nc.declare_dram_parameter("bbf2", [128, 2], F32, isOutput=False)
    btb1_d = nc.declare_dram_parameter("btb1", [64, 1], F32, isOutput=False)
    btb2_d = nc.declare_dram_parameter("btb2", [64, 1], F32, isOutput=False)
    btb3_d = nc.declare_dram_parameter("btb3", [128, 2], F32, isOutput=False)
    bsf_d = nc.declare_dram_parameter("bsf", [128, 2], F32, isOutput=False)
    btf_d = nc.declare_dram_parameter("btf", [128, 2], F32, isOutput=False)
    expb_d = nc.declare_dram_parameter("expb", [128, 1], F32, isOutput=False)
    lnb_d = nc.declare_dram_parameter("lnb", [128, 1], F32, isOutput=False)
    selftpl_d = nc.declare_dram_parameter("selftpl", [128, 80], dt.uint16,
                                          isOutput=False)
    out_d = nc.declare_dram_parameter("out", [IMGS_PER_CORE, C, N], F32,
                                      isOutput=True)
    dbg = bool(os.environ.get("KBENCH_DEBUG"))
    if dbg:
        dbg_ix = nc.declare_dram_parameter("dbg_ix", [128, 80], dt.uint16,
                                           isOutput=True)
        dbg_S = nc.declare_dram_parameter("dbg_S", [128, N], BF16, isOutput=True)
        dbg_STt = nc.declare_dram_parameter("dbg_STt", [128, 8, 8, 128], BF16,
                                            isOutput=True)
        dbg_e = nc.declare_dram_parameter("dbg_e", [128, 4, N], F16,
                                          isOutput=True)
        dbg_expq = nc.declare_dram_parameter("dbg_expq", [128, 4, N], BF16,
                                             isOutput=True)
        dbg_lnq = nc.declare_dram_parameter("dbg_lnq", [128, 4, N], F16,
                                            isOutput=True)
        dbg_feat = nc.declare_dram_parameter("dbg_feat", [128, 2, N], F16,
                                             isOutput=True)

    with TileContext(nc) as tc:
        import contextlib
        ctx = contextlib.ExitStack()
        with ctx:
            consts = ctx.enter_context(tc.tile_pool(name="consts", bufs=1))
            p_xc = ctx.enter_context(tc.tile_pool(name="xc", bufs=2))
            p_feat = ctx.enter_context(tc.tile_pool(name="feat", bufs=2))
            p_xn = ctx.enter_context(tc.tile_pool(name="xn", bufs=2))
            p_sc = ctx.enter_context(tc.tile_pool(name="sc", bufs=3))
            p_qe = ctx.enter_context(tc.tile_pool(name="qe", bufs=2))
            p_S = ctx.enter_context(tc.tile_pool(name="S", bufs=2))
            p_STt = ctx.enter_context(tc.tile_pool(name="STt", bufs=2))
            p_lnq = ctx.enter_context(tc.tile_pool(name="lnq", bufs=2))
            p_p = ctx.enter_context(tc.tile_pool(name="p", bufs=2))
            p_h = ctx.enter_context(tc.tile_pool(name="h", bufs=4))
            p_f1o = ctx.enter_context(tc.tile_pool(name="f1o", bufs=1))
            p_b = ctx.enter_context(tc.tile_pool(name="b", bufs=3))
            p_out = ctx.enter_context(tc.tile_pool(name="out", bufs=2))
            p_ix = ctx.enter_context(tc.tile_pool(name="ix", bufs=2))
            p_mx = ctx.enter_context(tc.tile_pool(name="mx", bufs=2))
            # all 8 PSUM banks in one 4-deep two-bank rotation (the n2 row
            # vector borrows row 0 of a ps1024 tile instead of its own pool)
            ps1024 = ctx.enter_context(
                tc.tile_pool(name="ps1024", bufs=4, space="PSUM"))

            # ---- constants / weights (loaded once) ----
            # alternate the two HWDGE rings (sync/scalar) so the ~21 weight
            # loads don't serialize on one ring at startup
            _ld = [0]

            def load(name, shape, dtype, src):
                t = consts.tile(shape, dtype, name=name)
                eng = nc.sync if _ld[0] % 2 == 0 else nc.scalar
                _ld[0] += 1
                eng.dma_start(out=t[:], in_=src[:])
                return t

            wg1 = load("wg1s", [128, 2, 256], F16, wg1_d)
            wp = load("wps", [128, 2, 512], F16, wp_d)
            wq = load("wqs", [128, 2, 512], F16, wq_d)
            wg2 = load("wg2s", [128, 4, 256], F16, wg2_d)
            wf1 = load("wf1s", [128, 2, 1024], F16, wf1_d)
            wf2 = load("wf2s", [128, 8, 256], F16, wf2_d)
            wb1 = load("wb1s", [128, 2, 64], F16, wb1_d)
            wb2 = load("wb2s", [64, 9, 64], F16, wb2_d)
            wb3 = load("wb3s", [64, 256], F16, wb3_d)
            bt1 = load("bt1s", [128, 2], F32, bt1_d)
            bt2 = load("bt2s", [128, 2], F32, bt2_d)
            bbp = load("bbps", [128, 4], F32, bbp_d)
            bbf1 = load("bbf1s", [128, 8], F32, bbf1_d)
            bbf2 = load("bbf2s", [128, 2], F32, bbf2_d)
            btb1 = load("btb1s", [64, 1], F32, btb1_d)
            btb2 = load("btb2s", [64, 1], F32, btb2_d)
            btb3 = load("btb3s", [128, 2], F32, btb3_d)
            bsf = load("bsfs", [128, 2], F32, bsf_d)
            btf = load("btfs", [128, 2], F32, btf_d)
            expb = load("expbs", [128, 1], F32, expb_d)
            lnb = load("lnbs", [128, 1], F32, lnb_d)
            selftpl = load("selftpls", [128, 80], dt.uint16, selftpl_d)

            ident = consts.tile([128, 128], F16, name="ident")
            make_identity(nc, ident[:])
            negid = consts.tile([128, 128], F16, name="negid")
            nc.scalar.activation(out=negid[:], in_=ident[:], func=AF.Copy,
                                 scale=NEG_BIG)
            ones = consts.tile([128, 128], F16, name="ones")
            nc.gpsimd.memset(ones[:], 1.0)
            onesk = consts.tile([128, 16], BF16, name="onesk")
            nc.gpsimd.memset(onesk[:], 1.0)
            # idbig[k, f] = 1 iff f == k + 384 (shifted identity for diag-kill)
            idbig = consts.tile([128, 1024], F16, name="idbig")
            nc.gpsimd.memset(idbig[:], 0.0)
            nc.gpsimd.affine_select(
                out=idbig[:], in_=idbig[:],
                compare_op=mybir.AluOpType.not_equal, fill=1.0,
                base=384, pattern=[[-1, 1024]], channel_multiplier=1)

            # per-image state carried from head to tail
            st = [{} for _ in range(IMGS_PER_CORE)]

            # ============== HEAD: load, g1, norms, sim/top8/S, q, p =======
            def head(img):
                s = st[img]
                xc = p_xc.tile([128, 2, N], F16, name="xc")
                for t in range(2):
                    # cast f32->f16 during DMA (SWDGE)
                    nc.gpsimd.dma_start(out=xc[:, t, :],
                                        in_=x_d[img, t * 128:(t + 1) * 128, :])
                s['xc'] = xc

                # ---- g1: featT [128, 2, N] f16 ----
                # (kt-outer loops everywhere: one LDWEIGHTS serves both
                # nb-halves, so matmuls stream back-to-back)
                featT = p_feat.tile([128, 2, N], F16, name="featT")
                for to in range(2):
                    ps = ps1024.tile([128, N], F32, name="ps_g1", tag="ps")
                    for kt in range(2):
                        for nb in range(2):
                            nc.tensor.matmul(
                                ps[:, nb * 512:(nb + 1) * 512],
                                lhsT=wg1[:, kt, to * 128:(to + 1) * 128],
                                rhs=xc[:, kt, nb * 512:(nb + 1) * 512],
                                start=(kt == 0), stop=(kt == 1))
                    nc.scalar.activation(out=featT[:, to, :], in_=ps[:],
                                         func=AF.Identity, bias=bt1[:, to:to + 1])

                # ---- row norms first: the rsqrt ACT-table swap and the n2
                # matmuls run while the PE then chews q/p, so invnb is ready
                # by the time the bcast matmul needs it ----
                fsq = p_sc.tile([128, 2, N], F16, name="fsq", tag="sc")
                nc.vector.tensor_mul(fsq[:], featT[:], featT[:])
                invn = p_mx.tile([1, N], F16, name="invn", tag="invn")
                ps_nb = ps1024.tile([128, N], F32, name="ps_nb", tag="ps")
                for nb in range(2):
                    for kt in range(2):
                        nc.tensor.matmul(
                            ps_nb[0:1, nb * 512:(nb + 1) * 512],
                            lhsT=ones[:, 0:1],
                            rhs=fsq[:, kt, nb * 512:(nb + 1) * 512],
                            start=(kt == 0), stop=(kt == 1))
                    # rank-only use; the gated-accuracy LUT is fine here
                    nc.scalar.activation(out=invn[:, nb * 512:(nb + 1) * 512],
                                         in_=ps_nb[0:1, nb * 512:(nb + 1) * 512],
                                         func=AF.Abs_reciprocal_sqrt)

                # ---- q -> expq (t*Wq folded; exp bias = -t*c) ----
                expq = p_qe.tile([128, 4, N], BF16, name="expq", tag="qe")
                for pair in range(4):
                    ps = ps1024.tile([128, N], F32, name="ps_q", tag="ps")
                    for sub in range(2):
                        nt = 2 * pair + sub
                        for kt in range(2):
                            nc.tensor.matmul(
                                ps[:, sub * 512:(sub + 1) * 512],
                                lhsT=featT[:, kt, nt * 128:(nt + 1) * 128],
                                rhs=wq[:, kt, :], start=(kt == 0), stop=(kt == 1))
                    nc.scalar.activation(out=expq[:, pair, :], in_=ps[:],
                                         func=AF.Exp, bias=expb[:, 0:1])
                if dbg and img == 0:
                    nc.sync.dma_start(out=dbg_expq[:], in_=expq[:])
                s['expq'] = expq

                # ---- p^T [128, 4, N] f16 (ch-part, bias + c folded) ----
                pT = p_p.tile([128, 4, N], F16, name="pT")
                for to in range(4):
                    ps = ps1024.tile([128, N], F32, name="ps_p", tag="ps")
                    for kt in range(2):
                        for nb in range(2):
                            nc.tensor.matmul(
                                ps[:, nb * 512:(nb + 1) * 512],
                                lhsT=wp[:, kt, to * 128:(to + 1) * 128],
                                rhs=featT[:, kt, nb * 512:(nb + 1) * 512],
                                start=(kt == 0), stop=(kt == 1))
                    # split PSUM drains between ACT and DVE: the PE fills a
                    # two-bank tile in ~0.9-1.8us; one ACT alone (1.3us/tile)
                    # is the pipeline bottleneck
                    if to < 2:
                        nc.scalar.activation(out=pT[:, to, :], in_=ps[:],
                                             func=AF.Identity,
                                             bias=bbp[:, to:to + 1])
                    else:
                        nc.vector.tensor_scalar(
                            out=pT[:, to, :], in0=ps[:],
                            scalar1=bbp[:, to:to + 1], scalar2=None,
                            op0=mybir.AluOpType.add)
                s['pT'] = pT

                # ---- broadcast invn -> xnT (rhs-side normalized) ----
                invnb = p_sc.tile([128, N], F16, name="invnb", tag="sc")
                psb = ps1024.tile([128, N], F32, name="ps_bc", tag="ps")
                for nb in range(2):
                    nc.tensor.matmul(psb[:, nb * 512:(nb + 1) * 512],
                                     lhsT=ones[0:1, :],
                                     rhs=invn[:, nb * 512:(nb + 1) * 512],
                                     start=True, stop=True)
                nc.scalar.activation(out=invnb[:], in_=psb[:], func=AF.Copy)
                xnT = p_xn.tile([128, 2, N], F16, name="xnT")
                for t in range(2):
                    nc.vector.tensor_mul(xnT[:, t, :], featT[:, t, :], invnb[:])
                s['featT'] = featT
                s['xnT'] = xnT

            # ============== HEAD B: sim + top8 + S + S^T =================
            # (separate phase: img1's sim matmuls cover img0's selection
            # tail, which otherwise holds all PSUM slots and idles the PE)
            def head_b(img):
                s = st[img]
                featT, xnT = s['featT'], s['xnT']
                ixbuf = p_ix.tile([128, 8, 10], dt.uint16, name="ixbuf")
                nc.vector.tensor_copy(
                    ixbuf[:].rearrange("p a b -> p (a b)"), selftpl[:])
                STt = p_STt.tile([128, 8, 8, 128], BF16, name="STt")
                s['STt'] = STt
                for I in range(8):
                    ps = ps1024.tile([128, N], F32, name="ps_sim", tag="ps")
                    dcb = I // 4
                    for kt in range(2):
                        for cb in range(2):
                            nc.tensor.matmul(
                                ps[:, cb * 512:(cb + 1) * 512],
                                lhsT=featT[:, kt, I * 128:(I + 1) * 128],
                                rhs=xnT[:, kt, cb * 512:(cb + 1) * 512],
                                start=(kt == 0),
                                stop=(kt == 1 and cb != dcb))
                    off2 = (I % 4) * 128
                    nc.tensor.matmul(
                        ps[:, dcb * 512:(dcb + 1) * 512], lhsT=negid[:],
                        rhs=idbig[:, 384 - off2:896 - off2],
                        start=False, stop=True)
                    # mx must be f32: find_index8 matches exact values, so
                    # in_max and in_values (PSUM f32) must share precision
                    mx = p_mx.tile([128, 8], F32, name="mx", tag="mx")
                    nc.vector.max(out=mx[:], in_=ps[:])
                    nc.vector.max_index(out=ixbuf[:, I, 1:9], in_max=mx[:],
                                        in_values=ps[:])
                    S_I = p_S.tile([128, N], BF16, name="S_I")
                    nc.gpsimd.local_scatter(
                        out_ap=S_I[:], data_ap=onesk[:, 0:10],
                        idxs_ap=ixbuf[:, I, :].bitcast(dt.int16),
                        channels=128, num_elems=N, num_idxs=10)
                    nc.sync.dma_start_transpose(out=STt[:, I], in_=S_I[:])
                    if dbg and img == 0 and I == 0:
                        nc.sync.dma_start(out=dbg_S[:], in_=S_I[:])
                if dbg and img == 0:
                    nc.sync.dma_start(out=dbg_STt[:], in_=STt[:])
                    nc.sync.dma_start(out=dbg_ix[:],
                                      in_=ixbuf[:].rearrange("p a b -> p (a b)"))
                    nc.sync.dma_start(out=dbg_feat[:], in_=featT[:])

            # ============== TAIL phase 1: agg + e =========================
            def agg_phase(img):
                s = st[img]
                expq, pT, STt = s['expq'], s['pT'], s['STt']

                # ---- agg: lnqT [128, 4, N] f16 = ln(expq^T @ S^T) ----
                # lnqT holds ln(agg)/t, computed on the DVE from the f32
                # exponent bits (the ACT Ln LUT clamps below ~2^-66, which
                # floors 20% of entries)
                lnqT = p_lnq.tile([128, 4, N], F16, name="lnqT")
                for cb in range(4):
                    ps = ps1024.tile([128, N], F32, name="ps_agg", tag="ps")
                    for kt in range(8):
                        for half in range(2):
                            nc.tensor.matmul(
                                ps[:, half * 512:(half + 1) * 512],
                                lhsT=expq[:, kt // 2,
                                          (kt % 2) * 512 + cb * 128:
                                          (kt % 2) * 512 + cb * 128 + 128],
                                rhs=STt[:, half * 4:(half + 1) * 4, kt, :],
                                start=(kt == 0), stop=(kt == 7))
                    nc.vector.tensor_scalar(
                        out=lnqT[:, cb, :], in0=ps[:].bitcast(dt.int32),
                        scalar1=LN_ALPHA, scalar2=LN_BETA,
                        op0=mybir.AluOpType.mult, op1=mybir.AluOpType.add)

                if dbg and img == 0:
                    nc.sync.dma_start(out=dbg_lnq[:], in_=lnqT[:])
                # ---- e = relu(pT + lnqT) (lnqT already scaled by 1/t) ----
                eT = p_qe.tile([128, 4, N], F16, name="eT", tag="qe")
                nc.vector.tensor_add(eT[:], lnqT[:], pT[:])
                nc.vector.tensor_scalar_max(eT[:], eT[:], 0.0)
                if dbg and img == 0:
                    nc.sync.dma_start(out=dbg_e[:], in_=eT[:])
                s['eT'] = eT

            # ============== TAIL phase 2: g2 + FFN ========================
            def ffn_phase(img):
                s = st[img]
                xc, eT = s['xc'], s['eT']

                # ---- g2 + residual -> h f16 ----
                h = p_h.tile([128, 2, N], F16, name="h", tag="h")
                for to in range(2):
                    ps = ps1024.tile([128, N], F32, name="ps_g2", tag="ps")
                    for kt in range(4):
                        for nb in range(2):
                            nc.tensor.matmul(
                                ps[:, nb * 512:(nb + 1) * 512],
                                lhsT=wg2[:, kt, to * 128:(to + 1) * 128],
                                rhs=eT[:, kt, nb * 512:(nb + 1) * 512],
                                start=(kt == 0), stop=(kt == 3))
                    tmp = p_sc.tile([128, N], F32, name="g2tmp", tag="sc")
                    nc.scalar.activation(out=tmp[:], in_=ps[:],
                                         func=AF.Identity, bias=bt2[:, to:to + 1])
                    nc.vector.tensor_add(h[:, to, :], tmp[:], xc[:, to, :])

                # ---- FFN ----
                f1o = p_f1o.tile([128, 8, N], F16, name="f1o")
                for to in range(8):
                    ps = ps1024.tile([128, N], F32, name="ps_f1", tag="ps")
                    for kt in range(2):
                        for nb in range(2):
                            nc.tensor.matmul(
                                ps[:, nb * 512:(nb + 1) * 512],
                                lhsT=wf1[:, kt, to * 128:(to + 1) * 128],
                                rhs=h[:, kt, nb * 512:(nb + 1) * 512],
                                start=(kt == 0), stop=(kt == 1))
                    if to % 2 == 0:
                        nc.scalar.activation(out=f1o[:, to, :], in_=ps[:],
                                             func=AF.Relu, bias=bbf1[:, to:to + 1])
                    else:
                        nc.vector.tensor_scalar(
                            out=f1o[:, to, :], in0=ps[:],
                            scalar1=bbf1[:, to:to + 1], scalar2=0.0,
                            op0=mybir.AluOpType.add, op1=mybir.AluOpType.max)
                h2 = p_h.tile([128, 2, N], F16, name="h2", tag="h")
                for to in range(2):
                    ps = ps1024.tile([128, N], F32, name="ps_f2", tag="ps")
                    for kt in range(8):
                        for nb in range(2):
                            nc.tensor.matmul(
                                ps[:, nb * 512:(nb + 1) * 512],
                                lhsT=wf2[:, kt, to * 128:(to + 1) * 128],
                                rhs=f1o[:, kt, nb * 512:(nb + 1) * 512],
                                start=(kt == 0), stop=(kt == 7))
                    tmp = p_sc.tile([128, N], F32, name="f2tmp", tag="sc")
                    nc.scalar.activation(out=tmp[:], in_=ps[:],
                                         func=AF.Identity, bias=bbf2[:, to:to + 1])
                    nc.vector.tensor_add(h2[:, to, :], tmp[:], h[:, to, :])
                s['h2'] = h2

            # ============== TAIL phase 3: bottleneck + out ================
            def bott_phase(img):
                s = st[img]
                xc, h2 = s['xc'], s['h2']

                b1o = p_b.tile([64, N], F16, name="b1o", tag="b")
                psb1 = ps1024.tile([128, N], F32, name="ps_b1", tag="ps")
                for kt in range(2):
                    for nb in range(2):
                        nc.tensor.matmul(
                            psb1[0:64, nb * 512:(nb + 1) * 512],
                            lhsT=wb1[:, kt, :],
                            rhs=h2[:, kt, nb * 512:(nb + 1) * 512],
                            start=(kt == 0), stop=(kt == 1))
                nc.scalar.activation(out=b1o[:], in_=psb1[0:64, :],
                                     func=AF.Relu, bias=btb1[:, 0:1])
                pad = p_b.tile([64, 34 * 34], F16, name="pad", tag="b")
                nc.vector.memset(pad[:], 0.0)
                pad3 = pad[:].rearrange("p (r c) -> p r c", r=34)
                b1v = b1o[:].rearrange("p (r c) -> p r c", r=32)
                nc.vector.tensor_copy(pad3[:, 1:33, 1:33], b1v)
                b2o = p_b.tile([64, N], F16, name="b2o", tag="b")
                psb2 = ps1024.tile([128, N], F32, name="ps_b2", tag="ps")
                for tap in range(9):
                    dy, dx = tap // 3, tap % 3
                    for nb in range(2):
                        rhs = pad3[:, 16 * nb + dy:16 * nb + dy + 16, dx:dx + 32]
                        nc.tensor.matmul(psb2[0:64, nb * 512:(nb + 1) * 512],
                                         lhsT=wb2[:, tap, :], rhs=rhs,
                                         start=(tap == 0), stop=(tap == 8))
                nc.scalar.activation(out=b2o[:], in_=psb2[0:64, :],
                                     func=AF.Relu, bias=btb2[:, 0:1])
                # ---- b3 + o3/fin/out, pipelined per channel half so the
                # t=1 compute hides under the t=0 output DMA (this chain is
                # the kernel's exit tail — nothing else overlaps it) ----
                b3o = p_sc.tile([128, 2, N], F16, name="b3o", tag="sc")
                o3 = p_h.tile([128, 2, N], F16, name="o3", tag="h")
                fin = p_lnq.tile([128, 2, N], F16, name="fin", tag="fin")
                for t in range(2):
                    ps = ps1024.tile([128, N], F32, name="ps_b3", tag="ps")
                    for nb in range(2):
                        nc.tensor.matmul(
                            ps[:, nb * 512:(nb + 1) * 512],
                            lhsT=wb3[:, t * 128:(t + 1) * 128],
                            rhs=b2o[:, nb * 512:(nb + 1) * 512],
                            start=True, stop=True)
                    nc.scalar.activation(out=b3o[:, t, :], in_=ps[:],
                                         func=AF.Identity, bias=btb3[:, t:t + 1])
                    nc.vector.tensor_add(o3[:, t, :], b3o[:, t, :], h2[:, t, :])
                    nc.vector.tensor_add(fin[:, t, :], o3[:, t, :], xc[:, t, :])
                    out32 = p_out.tile([128, N], F32, name="out32")
                    nc.scalar.activation(out=out32[:], in_=fin[:, t, :],
                                         func=AF.Identity, scale=bsf[:, t:t + 1],
                                         bias=btf[:, t:t + 1])
                    nc.sync.dma_start(out=out_d[img, t * 128:(t + 1) * 128, :],
                                      in_=out32[:])

            for img in range(IMGS_PER_CORE):
                head(img)
            for img in range(IMGS_PER_CORE):
                head_b(img)
            for img in range(IMGS_PER_CORE):
                agg_phase(img)
            for img in range(IMGS_PER_CORE):
                ffn_phase(img)
            for img in range(IMGS_PER_CORE):
                bott_phase(img)

    nc.finalize()
    return nc


# --------------------------------------------------------------------------
# entry point
# --------------------------------------------------------------------------
def kernel(**inputs):
    inp = {k: np.asarray(v) for k, v in inputs.items()}
    w = _prep_weights(inp)

    if 'nc' not in _cache:
        _cache['nc'] = _build_bass()
    nc = _cache['nc']

    x = inp['x'].astype(np.float32).reshape(B, C, N)
    in_maps = []
    for c in range(N_CORES):
        m = {'x': np.ascontiguousarray(x[c * 2:(c + 1) * 2])}
        m.update({k: v for k, v in w.items()})
        in_maps.append(m)

    from concourse.bass_utils import run_bass_kernel_spmd
    trace = bool(os.environ.get("KBENCH_TRACE"))
    res = run_bass_kernel_spmd(nc, in_maps, core_ids=list(range(N_CORES)),
                               trace=trace)
    _cache['exec_time_ns'] = res.exec_time_ns
    _cache['results'] = res
    out = np.zeros((B, C, N), np.float32)
    for c in range(N_CORES):
        out[c * 2:(c + 1) * 2] = res.results[c]['out']
    return out.reshape(B, C, H, W)
